# revision 17
# baseline (speedup 1.0000x reference)
"""Trainium2 Bass kernel for nn_PhongRender (DIB-R style Phong renderer).

kernel(**inputs) takes FULL unsharded inputs (as from setup_inputs()) and
returns (imrender [2,128,128,3], improb [2,128,128,1], normal1 [2,800,3]).

Sharding: 8 cores = 2 batches x 4 row-bands of 32 image rows each.
Each core rasterizes its 32x128 pixel band against all 800 faces.

Per-core device pipeline (f-layout [128 faces, 512 px] tiles):
  - w0n/w1n/w2n/zi maps: exact elementwise evaluation (ACT fma + DVE STT/TS)
    with the same f32 rounding sequence as the reference-validated host sim
    (hard decisions: inside tests, z-buffer winner).
  - S/Q soft-silhouette maps: PE matmul (K=4 affine basis), smooth path.
  - zmax / sum(log) partition reductions: GPSIMD all-reduce (exact f32).
  - winner one-hot (zbuf == zmax) -> feature interpolation via PE matmul
    against a precomputed per-face G table.
  - fragment shader + bilinear texture (indirect DMA gather) on-device.
"""
import numpy as np

f32 = np.float32

B, P, F, H, W, TEX = 2, 500, 800, 128, 128, 512
NCORES = 8
BANDS = 4                 # row-bands per batch
ROWS = H // BANDS         # 32 rows per core
NPX = ROWS * W            # 4096 pixels per core
CH = 512                  # pixels per chunk (4 image rows)
NCHUNK = NPX // CH        # 8
RCH = CH // W             # rows per chunk = 4
NT = (F + 127) // 128     # 7 face tiles
FP = NT * 128             # 896 padded faces
EPS = f32(1e-15)
MULT = 1000.0
DELTA = 7000.0
NEG_BIG = f32(-1e30)
ZFILL = f32(-1e10)
LOG1EM7 = float(np.log(np.float32(1e-7)))
PMAX = float(np.float32(1.0) - np.float32(1e-7))
TEXN = TEX * TEX * 3 + 16


def _fma(a, b, c):
    """f32 fused multiply-add (matches XLA CPU's fma contraction)."""
    return (a.astype(np.float64) * b.astype(np.float64) + c.astype(np.float64)).astype(f32)


def _cross_fma(a, b):
    """cross product with XLA-CPU's fma pattern: fma(a1,b2, -(a2*b1))."""
    return np.stack([
        _fma(a[:, 1], b[:, 2], -(a[:, 2] * b[:, 1]).astype(f32)),
        _fma(a[:, 2], b[:, 0], -(a[:, 0] * b[:, 2]).astype(f32)),
        _fma(a[:, 0], b[:, 1], -(a[:, 1] * b[:, 0]).astype(f32)),
    ], -1)


def host_prep(inputs):
    """Build per-core input tensor maps + host-computed normal1."""
    points = np.asarray(inputs['points'], dtype=f32)
    faces = np.asarray(inputs['faces'])
    camera_rot = np.asarray(inputs['camera_rot'], dtype=f32)
    camera_pos = np.asarray(inputs['camera_pos'], dtype=f32)
    camera_proj = np.asarray(inputs['camera_proj'], dtype=f32).reshape(3)
    uv = np.asarray(inputs['uv'], dtype=f32)
    ft = np.asarray(inputs['ft'])
    texture = np.asarray(inputs['texture'], dtype=f32)
    lightdirect = np.asarray(inputs['lightdirect'], dtype=f32)
    material = np.asarray(inputs['material'], dtype=f32)
    shininess = np.asarray(inputs['shininess'], dtype=f32)

    px = ((f32(2.0) * (np.arange(W, dtype=f32) + f32(0.5)) - f32(W)) / f32(W)).astype(f32)
    py = ((f32(H) - f32(2.0) * (np.arange(H, dtype=f32) + f32(0.5))) / f32(H)).astype(f32)

    cc = np.float64(MULT) ** 2 / np.float64(DELTA)

    normal1_out = np.zeros((B, F, 3), dtype=f32)
    per_batch = []
    for b in range(B):
        pts = points[b]
        pos = camera_pos[b]
        rot = camera_rot[b]
        pc = np.einsum('pj,ij->pi', (pts - pos[None, :]).astype(f32), rot).astype(f32)
        xy = (pc * camera_proj[None, :]).astype(f32)
        xy2 = (xy[:, :2] / xy[:, 2:3]).astype(f32)
        p3 = pc[faces]
        p2 = xy2[faces]
        v10 = (p3[:, 1] - p3[:, 0]).astype(f32)
        v20 = (p3[:, 2] - p3[:, 0]).astype(f32)
        normal = _cross_fma(v10, v20)
        normalz = normal[:, 2]
        nn = np.sqrt((normal * normal).sum(-1, keepdims=True).astype(f32)).astype(f32)
        normal1_out[b] = (normal / (nn + EPS)).astype(f32)

        ax, ay = p2[:, 0, 0], p2[:, 0, 1]
        bx, by = p2[:, 1, 0], p2[:, 1, 1]
        cx, cy = p2[:, 2, 0], p2[:, 2, 1]
        area = ((bx - ax) * (cy - ay) - (by - ay) * (cx - ax)).astype(f32)
        ok = np.abs(area) > f32(1e-10)
        inv = (f32(1.0) / np.where(ok, area, f32(1.0))).astype(f32)

        def edge_coef(ux, uy, vx, vy):
            ex = (vx - ux).astype(f32)
            ey = (vy - uy).astype(f32)
            return np.stack([(-ey).astype(f32), ex,
                             (ey * ux - ex * uy).astype(f32)], 0)

        cw0 = edge_coef(bx, by, cx, cy)
        cw1 = edge_coef(cx, cy, ax, ay)
        cw2 = edge_coef(ax, ay, bx, by)
        cw0n = (cw0 * inv[None]).astype(f32)
        cw1n = (cw1 * inv[None]).astype(f32)
        cw2n = (cw2 * inv[None]).astype(f32)
        z = p3[:, :, 2]
        czi = (cw0n * z[:, 0][None] + cw1n * z[:, 1][None] + cw2n * z[:, 2][None]).astype(f32)

        uvf = uv[b][ft]
        feat = np.concatenate([
            np.tile(normal[:, None, :], (1, 3, 1)),
            -p3, uvf, np.ones((F, 3, 1), dtype=f32)], axis=-1).astype(f32)
        cw = np.stack([cw0n, cw1n, cw2n], 0)  # [3k,3j,F]
        G = np.einsum('kjf,fkd->fjd', cw.astype(np.float64),
                      feat.astype(np.float64)).astype(f32)  # [F,3,9]

        def seg_tables(ux, uy, vx, vy):
            ex = (vx - ux).astype(np.float64); ey = (vy - uy).astype(np.float64)
            uxd = ux.astype(np.float64); uyd = uy.astype(np.float64)
            E = ex * ex + ey * ey
            S = np.stack([np.zeros_like(ex), 2 * cc * ex, 2 * cc * ey,
                          2 * cc * (-ex * uxd - ey * uyd)], 0).astype(f32)
            Q = np.stack([np.full_like(ex, cc), -2 * cc * uxd, -2 * cc * uyd,
                          cc * (uxd * uxd + uyd * uyd)], 0).astype(f32)
            IE = (1.0 / (2 * cc * (E + 1e-12))).astype(f32)
            CE = (cc * E).astype(f32)
            return S, Q, IE, CE

        Sab, Qab, IEab, CEab = seg_tables(ax, ay, bx, by)
        Sbc, Qbc, IEbc, CEbc = seg_tables(bx, by, cx, cy)
        Sca, Qca, IEca, CEca = seg_tables(cx, cy, ax, ay)

        okm1 = np.where(ok, f32(0.0), NEG_BIG).astype(f32)
        okm2 = np.where(ok & (normalz > 0), f32(0.0), NEG_BIG).astype(f32)

        def padF(a, fill=0.0):
            shp = list(a.shape); shp[-1] = FP
            out = np.full(shp, fill, dtype=f32)
            out[..., :F] = a
            return out

        SQ = np.stack([padF(Sab), padF(Sbc), padF(Sca),
                       padF(Qab), padF(Qbc), padF(Qca)], 0)  # [6,4,FP]
        SQ[3:, 3, F:] = f32(1e30)  # pad faces: a = huge -> lg = 0
        coefs = SQ.reshape(6, 4, NT, 128).transpose(2, 0, 1, 3).copy()  # [NT,6,4,128]

        WC = np.stack([padF(cw0n[0]), padF(cw0n[1]), padF(cw0n[2]),
                       padF(cw1n[0]), padF(cw1n[1]), padF(cw1n[2]),
                       padF(cw2n[0]), padF(cw2n[1]), padF(cw2n[2]),
                       padF(czi[0]), padF(czi[1]), padF(czi[2])], 0)  # [12, FP]
        wcols = WC.reshape(12, NT, 128).transpose(2, 1, 0).reshape(128, NT * 12).copy()

        FC = np.stack([padF(IEab), padF(IEbc), padF(IEca),
                       padF(CEab), padF(CEbc), padF(CEca),
                       padF(okm1, float(NEG_BIG)), padF(okm2, float(NEG_BIG))], 0)
        fcols = FC.reshape(8, NT, 128).transpose(2, 1, 0).reshape(128, NT * 8).copy()

        Gp = padF(G.reshape(F, 27).T).T.reshape(FP, 3, 9)
        gtab = Gp.reshape(NT, 128, 27)
        gtab = np.concatenate([gtab, np.zeros((NT, 128, 9), dtype=f32)], -1)
        gtab = gtab.transpose(1, 0, 2).reshape(128, NT * 36).copy()

        misc = np.zeros((128, 16), dtype=f32)
        mat = material[b]
        ld = lightdirect[b]
        ln = np.sqrt((ld * ld).sum().astype(f32)).astype(f32)
        ldn = (ld / (ln + EPS)).astype(f32)
        misc[:, 0:3] = mat[0][None, :]
        misc[:, 3:6] = mat[1][None, :]
        misc[:, 6:9] = mat[2][None, :]
        misc[:, 9:12] = ldn[None, :]
        misc[:, 12] = shininess[b, 0]

        ti = np.zeros(TEXN, dtype=f32)
        ti[:TEX * TEX * 3] = texture[b].transpose(1, 2, 0).reshape(-1)

        per_batch.append(dict(coefs=coefs, wcols=wcols, fcols=fcols, gtab=gtab,
                              misc=misc, texi=ti.reshape(TEXN, 1)))

    PXg = np.broadcast_to(px[None, :], (H, W)).reshape(-1).astype(f32)
    PYg = np.broadcast_to(py[:, None], (H, W)).reshape(-1).astype(f32)
    Rg = (PXg * PXg + PYg * PYg).astype(f32)

    in_maps = []
    for core in range(NCORES):
        b = core // BANDS
        band = core % BANDS
        r0 = band * ROWS
        sl = slice(r0 * W, (r0 + ROWS) * W)
        basis = np.stack([Rg[sl], PXg[sl], PYg[sl], np.ones(NPX, dtype=f32)], 0).copy()
        pyexp = np.broadcast_to(py[r0:r0 + ROWS][None, :, None], (128, ROWS, 9)).reshape(128, ROWS * 9).copy()
        pxcol = px[:, None].copy()
        t = per_batch[b]
        in_maps.append({
            'coefs': t['coefs'], 'wcols': t['wcols'], 'fcols': t['fcols'],
            'gtab': t['gtab'], 'misc': t['misc'], 'texi': t['texi'],
            'basis': basis, 'pyexp': pyexp, 'pxcol': pxcol,
            'pxb': np.broadcast_to(PXg[sl][None, :], (128, NPX)).copy(),
            'pyb': np.broadcast_to(PYg[sl][None, :], (128, NPX)).copy(),
        })
    return in_maps, normal1_out


_NC_CACHE = {}


def _register_seg_a():
    """Register the fused soft-prob custom DVE op:
    out = t*(t*CE - S) + Q with t = clip01(S*IE)   (S=in0, Q=in1, IE=s0, CE=s1)."""
    import concourse.dve_ops as dve_ops
    if any(o.name == "SEG_A_ANT" for o in dve_ops.OPS):
        return (next(o for o in dve_ops.OPS if o.name == "SEG_A_ANT"),
                next(o for o in dve_ops.OPS if o.name == "ZSEL_ANT"))
    from concourse.dve_spec import Spec, Src0, Src1, C0, C1, C2, relu, minn
    from concourse.dve_table_gen import dve_ver_for

    def ref_seg_a(in0, in1, s0, s1, imm2):
        t = np.minimum(np.maximum(in0.astype(np.float32) * s0, 0), imm2).astype(np.float32)
        return (t * (t * s1 - in0) + in1).astype(np.float32)

    t_expr = minn(relu(Src0 * C0), C2)
    spec = Spec(body=t_expr * (t_expr * C1 - Src0) + Src1, reference=ref_seg_a)
    row = dve_ops._CUSTOM_DVE_ROW_BASE + len(dve_ops.OPS)
    op = dve_ops.DveOp("SEG_A_ANT", spec, subdim=False,
                       uops_sha={"v3": "a415bb456e75ca33"})
    dve_ops.OPS.append(op)
    dve_ops._SUB_OPCODE_FOR_NAME[op.name] = row
    dve_ops.CUSTOM_DVE_SPECS[op.name] = spec

    def ref_zsel(in0, in1, s0, s1, imm2):
        return (in0.astype(np.float32) * in1 + (in0 - imm2) * s0).astype(np.float32)

    spec2 = Spec(body=Src0 * Src1 + (Src0 - C2) * C0, reference=ref_zsel)
    row2 = dve_ops._CUSTOM_DVE_ROW_BASE + len(dve_ops.OPS)
    op2 = dve_ops.DveOp("ZSEL_ANT", spec2, subdim=False,
                        uops_sha={"v3": "4c6249d316d2ba3d"})
    dve_ops.OPS.append(op2)
    dve_ops._SUB_OPCODE_FOR_NAME[op2.name] = row2
    dve_ops.CUSTOM_DVE_SPECS[op2.name] = spec2
    return op, op2


def build_nc():
    import concourse.bacc as bacc
    import concourse.bass as bass
    import concourse.mybir as mybir
    import concourse.bass_isa as bass_isa
    from concourse.tile import TileContext

    dt = mybir.dt
    Alu = mybir.AluOpType
    Act = mybir.ActivationFunctionType

    seg_a_op, zsel_op = _register_seg_a()
    nc = bacc.Bacc(trn_type="TRN2")
    coefs_d = nc.dram_tensor("coefs", [NT, 6, 4, 128], dt.float32, kind="ExternalInput")
    wcols_d = nc.dram_tensor("wcols", [128, NT * 12], dt.float32, kind="ExternalInput")
    fcols_d = nc.dram_tensor("fcols", [128, NT * 8], dt.float32, kind="ExternalInput")
    gtab_d = nc.dram_tensor("gtab", [128, NT * 36], dt.float32, kind="ExternalInput")
    misc_d = nc.dram_tensor("misc", [128, 16], dt.float32, kind="ExternalInput")
    texi_d = nc.dram_tensor("texi", [TEXN, 1], dt.float32, kind="ExternalInput")
    basis_d = nc.dram_tensor("basis", [4, NPX], dt.float32, kind="ExternalInput")
    pxb_d = nc.dram_tensor("pxb", [128, NPX], dt.float32, kind="ExternalInput")
    pyb_d = nc.dram_tensor("pyb", [128, NPX], dt.float32, kind="ExternalInput")
    pyexp_d = nc.dram_tensor("pyexp", [128, ROWS * 9], dt.float32, kind="ExternalInput")
    pxcol_d = nc.dram_tensor("pxcol", [128, 1], dt.float32, kind="ExternalInput")
    imr_d = nc.dram_tensor("imr", [ROWS, W, 3], dt.float32, kind="ExternalOutput")
    imp_d = nc.dram_tensor("imp", [ROWS, W], dt.float32, kind="ExternalOutput")

    with TileContext(nc) as tc:
        with tc.tile_pool(name="cb", bufs=1) as cb, \
             tc.tile_pool(name="zb", bufs=1) as zbp, \
             tc.tile_pool(name="wk", bufs=2) as wk, \
             tc.tile_pool(name="ep", bufs=1) as ep, \
             tc.tile_pool(name="ps1", bufs=1, space="PSUM") as ps1, \
             tc.tile_pool(name="ps2", bufs=2, space="PSUM") as ps2, \
             tc.tile_pool(name="dr", bufs=1, space="DRAM") as dr:

            # ---- constants into SBUF ----
            coefs_sb = cb.tile([4, NT * 6 * 128], dt.float32, tag="coefs")
            nc.sync.dma_start(out=coefs_sb[:].rearrange("k (t m j) -> k t m j", t=NT, m=6),
                              in_=coefs_d[:].rearrange("t m k j -> k t m j"))
            wcols = cb.tile([128, NT * 12], dt.float32, tag="wcols")
            nc.sync.dma_start(out=wcols[:], in_=wcols_d[:])
            fcols = cb.tile([128, NT * 8], dt.float32, tag="fcols")
            nc.sync.dma_start(out=fcols[:], in_=fcols_d[:])
            gtab = cb.tile([128, NT * 36], dt.float32, tag="gtab")
            nc.sync.dma_start(out=gtab[:], in_=gtab_d[:])
            misc = cb.tile([128, 16], dt.float32, tag="misc")
            nc.sync.dma_start(out=misc[:], in_=misc_d[:])
            pxb = cb.tile([128, NPX], dt.float32, tag="pxb")
            nc.sync.dma_start(out=pxb[:], in_=pxb_d[:])
            pyb = cb.tile([128, NPX], dt.float32, tag="pyb")
            nc.sync.dma_start(out=pyb[:], in_=pyb_d[:])
            pyexp = cb.tile([128, ROWS * 9], dt.float32, tag="pyexp")
            nc.sync.dma_start(out=pyexp[:], in_=pyexp_d[:])
            pxcol = cb.tile([128, 1], dt.float32, tag="pxcol")
            nc.sync.dma_start(out=pxcol[:], in_=pxcol_d[:])
            m16 = cb.tile([128, CH], dt.float32, tag="m16")
            nc.vector.memset(m16[:], LOG1EM7)
            ones = cb.tile([128, 1], dt.float32, tag="ones")
            nc.vector.memset(ones[:], 1.0)

            imfeat = ep.tile([128, ROWS * 36], dt.float32, tag="imfeat")
            covT = ep.tile([128, ROWS], dt.float32, tag="covT")
            covd = dr.tile([ROWS, W], dt.float32, tag="covd")
            gathA = ep.tile([128, ROWS * 2 * 6], dt.float32, tag="gathA")
            uA = ep.tile([128, ROWS], dt.float32, tag="uA")
            vA = ep.tile([128, ROWS], dt.float32, tag="vA")

            def fcol(t, j):
                return fcols[:, t * 8 + j:t * 8 + j + 1]

            def wcol(t, j):
                return wcols[:, t * 12 + j:t * 12 + j + 1]

            # ---------------- main per-chunk loop ----------------
            for ch in range(NCHUNK):
                pxs = slice(ch * CH, (ch + 1) * CH)
                bas = wk.tile([4, CH], dt.float32, tag="bas")
                nc.sync.dma_start(out=bas[:], in_=basis_d[:, pxs])

                zts = []
                sg = ps1.tile([1, CH], dt.float32, tag="sg", space="PSUM")
                in_tiles = []
                for t in range(NT):
                    # ---- exact w/zi maps: s_m = PX*c0 + PY*c1 (ACT fma + STT) ----
                    ss = []
                    for m in range(4):
                        u = wk.tile([128, CH], dt.float32, tag=f"u{m}", name=f"u{m}")
                        nc.scalar.activation(u[:], pxb[:, pxs], Act.Copy,
                                             scale=wcol(t, m * 3 + 0))
                        nc.vector.scalar_tensor_tensor(
                            out=u[:], in0=pyb[:, pxs], scalar=wcol(t, m * 3 + 1),
                            in1=u[:], op0=Alu.mult, op1=Alu.add)
                        ss.append(u)
                    w1 = wk.tile([128, CH], dt.float32, tag="w1")
                    nc.vector.tensor_scalar(out=w1[:], in0=ss[1][:],
                                            scalar1=wcol(t, 1 * 3 + 2), scalar2=None,
                                            op0=Alu.add)
                    zi = wk.tile([128, CH], dt.float32, tag="zi")
                    nc.vector.tensor_scalar(out=zi[:], in0=ss[3][:],
                                            scalar1=wcol(t, 3 * 3 + 2), scalar2=None,
                                            op0=Alu.add)
                    m1 = wk.tile([128, CH], dt.float32, tag="m1")
                    nc.vector.scalar_tensor_tensor(
                        out=m1[:], in0=ss[0][:], scalar=wcol(t, 0 * 3 + 2),
                        in1=w1[:], op0=Alu.add, op1=Alu.min)
                    m3 = wk.tile([128, CH], dt.float32, tag="m3")
                    nc.vector.scalar_tensor_tensor(
                        out=m3[:], in0=ss[2][:], scalar=wcol(t, 2 * 3 + 2),
                        in1=m1[:], op0=Alu.add, op1=Alu.min)
                    v01 = wk.tile([128, CH], dt.float32, tag="v01")
                    nc.vector.tensor_scalar(out=v01[:], in0=m3[:], scalar1=fcol(t, 7),
                                            scalar2=0.0, op0=Alu.add, op1=Alu.is_ge)
                    in01 = wk.tile([128, CH], dt.uint8, tag="in01")
                    nc.vector.tensor_scalar(out=in01[:], in0=m3[:], scalar1=fcol(t, 6),
                                            scalar2=0.0, op0=Alu.add, op1=Alu.is_ge)
                    zb = zbp.tile([128, CH], dt.float32, tag=f"z{t}")
                    nc.vector._custom_dve(zsel_op, out=zb[:], in0=v01[:], in1=zi[:],
                                          s0=float(-ZFILL), s1=0.0, imm2=1.0)
                    zts.append(zb)

                    # ---- soft-prob path (PE maps) ----
                    a_list = []
                    for e in range(3):
                        Sp = ps2.tile([128, CH], dt.float32, tag="qS", space="PSUM")
                        nc.tensor.matmul(
                            Sp[:], coefs_sb[:, (t * 6 + e) * 128:(t * 6 + e + 1) * 128],
                            bas[:], start=True, stop=True)
                        Qp = ps2.tile([128, CH], dt.float32, tag="qQ", space="PSUM")
                        nc.tensor.matmul(
                            Qp[:], coefs_sb[:, (t * 6 + e + 3) * 128:(t * 6 + e + 4) * 128],
                            bas[:], start=True, stop=True)
                        Qs = wk.tile([128, CH], dt.float32, tag="Qs")
                        nc.scalar.activation(Qs[:], Qp[:], Act.Copy)
                        a = wk.tile([128, CH], dt.float32, tag=f"a{e}")
                        nc.vector._custom_dve(seg_a_op, out=a[:], in0=Sp[:], in1=Qs[:],
                                              s0=fcol(t, e), s1=fcol(t, 3 + e), imm2=1.0)
                        a_list.append(a)
                    am = wk.tile([128, CH], dt.float32, tag="am")
                    nc.vector.tensor_tensor(out=am[:], in0=a_list[0][:], in1=a_list[1][:], op=Alu.min)
                    am2 = wk.tile([128, CH], dt.float32, tag="am2")
                    nc.vector.tensor_tensor(out=am2[:], in0=am[:], in1=a_list[2][:], op=Alu.min)
                    pp = wk.tile([128, CH], dt.float32, tag="pp")
                    nc.scalar.activation(pp[:], am2[:], Act.Exp, scale=-1.0)
                    pc = wk.tile([128, CH], dt.float32, tag="pc")
                    nc.vector.tensor_scalar(out=pc[:], in0=pp[:], scalar1=PMAX,
                                            scalar2=None, op0=Alu.min)
                    lg = wk.tile([128, CH], dt.float32, tag="lg")
                    nc.scalar.activation(lg[:], pc[:], Act.Ln, scale=-1.0, bias=1.0)
                    nc.vector.copy_predicated(out=lg[:], mask=in01[:], data=m16[:])
                    nc.tensor.matmul(sg[:], ones[:], lg[:], start=(t == 0),
                                     stop=(t == NT - 1), skip_group_check=True)

                # ---- zmax + sum(lg) partition reductions ----
                zmt = wk.tile([128, CH], dt.float32, tag="zmt")
                nc.vector.tensor_tensor(out=zmt[:], in0=zts[0][:], in1=zts[1][:], op=Alu.max)
                for t in range(2, NT):
                    nc.vector.tensor_tensor(out=zmt[:], in0=zmt[:], in1=zts[t][:], op=Alu.max)
                zmaxb = wk.tile([128, CH], dt.float32, tag="zmaxb")
                nc.gpsimd.partition_all_reduce(zmaxb[:], zmt[:], 128, bass_isa.ReduceOp.max)

                # ---- one-hot winner + feature matmul (accumulate over tiles) ----
                gall = ps1.tile([128, RCH * 36], dt.float32, tag="gall", space="PSUM")
                ohs = []
                for t in range(NT):
                    oh = zbp.tile([128, CH], dt.float32, tag=f"oh{t}", name=f"oh{t}")
                    nc.vector.tensor_tensor(out=oh[:], in0=zts[t][:], in1=zmaxb[:], op=Alu.is_equal)
                    ohs.append(oh)
                for blk in range(RCH):
                    for t in range(NT):
                        nc.tensor.matmul(gall[:, blk * 36:(blk + 1) * 36],
                                         ohs[t][:, blk * 128:(blk + 1) * 128],
                                         gtab[:, t * 36:(t + 1) * 36],
                                         start=(t == 0), stop=(t == NT - 1),
                                         skip_group_check=True)
                nc.scalar.activation(
                    imfeat[:, ch * RCH * 36:(ch + 1) * RCH * 36], gall[:], Act.Copy)

                # --- texcoords + texture gathers for this chunk's 4 rows ---
                ut = wk.tile([128, RCH], dt.float32, tag="ut")
                vt = wk.tile([128, RCH], dt.float32, tag="vt")
                tt1 = wk.tile([128, RCH], dt.float32, tag="tt1")
                idf = wk.tile([128, RCH * 2], dt.float32, tag="idf")
                idi = wk.tile([128, RCH * 2], dt.int32, tag="idi")
                imfc = imfeat[:, ch * RCH * 36:(ch + 1) * RCH * 36].rearrange(
                    "p (r j d) -> p r j d", j=4, d=9)
                for j, dst in ((6, ut), (7, vt)):
                    nc.vector.tensor_scalar(out=tt1[:], in0=imfc[:, :, 0, j],
                                            scalar1=pxcol[:, 0:1], scalar2=None, op0=Alu.mult)
                    nc.vector.tensor_tensor(out=dst[:], in0=imfc[:, :, 1, j],
                                            in1=pyexp[:].rearrange("p (r d) -> p r d", d=9)[
                                                :, ch * RCH:(ch + 1) * RCH, 0],
                                            op=Alu.mult)
                    nc.vector.tensor_tensor(out=dst[:], in0=dst[:], in1=tt1[:], op=Alu.add)
                    nc.vector.tensor_tensor(out=dst[:], in0=dst[:], in1=imfc[:, :, 2, j], op=Alu.add)
                # u = clip(u,0,1)*511 ; v = (1-clip(v,0,1))*511
                nc.vector.tensor_scalar(out=ut[:], in0=ut[:], scalar1=0.0, scalar2=1.0,
                                        op0=Alu.max, op1=Alu.min)
                nc.vector.tensor_scalar(out=ut[:], in0=ut[:], scalar1=float(TEX - 1),
                                        scalar2=None, op0=Alu.mult)
                nc.vector.tensor_scalar(out=vt[:], in0=vt[:], scalar1=0.0, scalar2=1.0,
                                        op0=Alu.max, op1=Alu.min)
                nc.vector.tensor_scalar(out=vt[:], in0=vt[:], scalar1=-1.0, scalar2=1.0,
                                        op0=Alu.mult, op1=Alu.add)
                nc.vector.tensor_scalar(out=vt[:], in0=vt[:], scalar1=float(TEX - 1),
                                        scalar2=None, op0=Alu.mult)
                nc.vector.tensor_copy(uA[:, ch * RCH:(ch + 1) * RCH], ut[:])
                nc.vector.tensor_copy(vA[:, ch * RCH:(ch + 1) * RCH], vt[:])
                # floor (exact, rounding-agnostic)
                ci = wk.tile([128, RCH], dt.int32, tag="ci")
                fx = wk.tile([128, RCH], dt.float32, tag="fx")
                x0c = wk.tile([128, RCH], dt.float32, tag="x0c")
                y0c = wk.tile([128, RCH], dt.float32, tag="y0c")
                for srcv, dstv in ((ut, x0c), (vt, y0c)):
                    nc.vector.tensor_copy(ci[:], srcv[:])
                    nc.vector.tensor_copy(dstv[:], ci[:])
                    nc.vector.tensor_tensor(out=fx[:], in0=dstv[:], in1=srcv[:], op=Alu.is_gt)
                    nc.vector.tensor_tensor(out=dstv[:], in0=dstv[:], in1=fx[:], op=Alu.subtract)
                y1c = wk.tile([128, RCH], dt.float32, tag="y1c")
                nc.vector.tensor_scalar(out=y1c[:], in0=y0c[:], scalar1=1.0,
                                        scalar2=float(TEX - 1), op0=Alu.add, op1=Alu.min)
                idv = idf[:].rearrange("p (r d) -> p r d", d=2)
                nc.vector.scalar_tensor_tensor(out=idv[:, :, 0], in0=y0c[:], scalar=float(TEX),
                                               in1=x0c[:], op0=Alu.mult, op1=Alu.add)
                nc.vector.scalar_tensor_tensor(out=idv[:, :, 1], in0=y1c[:], scalar=float(TEX),
                                               in1=x0c[:], op0=Alu.mult, op1=Alu.add)
                nc.vector.tensor_scalar(out=idf[:], in0=idf[:], scalar1=3.0, scalar2=None,
                                        op0=Alu.mult)
                nc.vector.tensor_copy(idi[:], idf[:])
                for r in range(RCH):
                    for wch in range(2):
                        gi = (ch * RCH + r) * 2 + wch
                        nc.gpsimd.indirect_dma_start(
                            out=gathA[:, gi * 6:(gi + 1) * 6], out_offset=None,
                            in_=texi_d[:],
                            in_offset=bass.IndirectOffsetOnAxis(
                                ap=idi[:, r * 2 + wch:r * 2 + wch + 1], axis=0))

                # ---- coverage + improb for this chunk ----
                cr = ep.tile([1, CH], dt.float32, tag="cr")
                nc.vector.tensor_scalar(out=cr[:], in0=zmaxb[0:1, :], scalar1=float(ZFILL),
                                        scalar2=None, op0=Alu.is_gt)
                nc.gpsimd.dma_start(
                    out=covd[ch * RCH:(ch + 1) * RCH, :], in_=cr[:])
                exr = ep.tile([1, CH], dt.float32, tag="exr")
                nc.scalar.activation(exr[:], sg[:], Act.Exp)
                impc = ep.tile([1, CH], dt.float32, tag="impc")
                nc.vector.tensor_scalar(out=impc[:], in0=exr[:], scalar1=-1.0,
                                        scalar2=1.0, op0=Alu.mult, op1=Alu.add)
                nc.sync.dma_start(
                    out=imp_d[ch * RCH:(ch + 1) * RCH, :].rearrange("r c -> r c")[None, :, :],
                    in_=impc[:].rearrange("a (r c) -> a r c", c=W))

            # ---------------- core epilogue ([col, row] layout) ----------------
            NR = ROWS
            nc.gpsimd.dma_start(out=covT[:], in_=covd[:].rearrange("r c -> c r"))
            i3 = imfeat[:].rearrange("p (r j d) -> p r j d", j=4, d=9)
            imf9 = ep.tile([128, NR * 9], dt.float32, tag="imf9")
            v9 = imf9[:].rearrange("p (r d) -> p r d", d=9)
            tmp9 = ep.tile([128, NR * 9], dt.float32, tag="tmp9")
            t9 = tmp9[:].rearrange("p (r d) -> p r d", d=9)
            nc.vector.tensor_scalar(out=t9[:, :, :], in0=i3[:, :, 0, :],
                                    scalar1=pxcol[:, 0:1], scalar2=None, op0=Alu.mult)
            nc.vector.tensor_tensor(out=v9[:, :, :], in0=i3[:, :, 1, :],
                                    in1=pyexp[:].rearrange("p (r d) -> p r d", d=9),
                                    op=Alu.mult)
            nc.vector.tensor_tensor(out=v9[:, :, :], in0=v9[:, :, :], in1=t9[:, :, :], op=Alu.add)
            nc.vector.tensor_tensor(out=v9[:, :, :], in0=v9[:, :, :], in1=i3[:, :, 2, :], op=Alu.add)

            def vsl(j, k=1):
                return imf9[:].rearrange("p (r d) -> p r d", d=9)[:, :, j:j + k]

            nc.vector.tensor_tensor(out=vsl(8)[:, :, 0], in0=vsl(8)[:, :, 0],
                                    in1=covT[:], op=Alu.mult)

            def normalize3(dst_tag, j0):
                sq = ep.tile([128, NR * 3], dt.float32, tag=dst_tag + "sq")
                s3 = sq[:].rearrange("p (r d) -> p r d", d=3)
                nc.vector.tensor_tensor(out=s3[:], in0=vsl(j0, 3)[:, :, :],
                                        in1=vsl(j0, 3)[:, :, :], op=Alu.mult)
                n2 = ep.tile([128, NR], dt.float32, tag=dst_tag + "n2")
                nc.vector.tensor_reduce(out=n2[:], in_=s3[:], axis=mybir.AxisListType.X,
                                        op=Alu.add)
                sn = ep.tile([128, NR], dt.float32, tag=dst_tag + "sn")
                nc.scalar.activation(sn[:], n2[:], Act.Sqrt)
                nc.vector.tensor_scalar(out=sn[:], in0=sn[:], scalar1=float(EPS),
                                        scalar2=None, op0=Alu.add)
                rc = ep.tile([128, NR], dt.float32, tag=dst_tag + "rc")
                nc.vector.reciprocal(rc[:], sn[:])
                out = ep.tile([128, NR * 3], dt.float32, tag=dst_tag)
                o3 = out[:].rearrange("p (r d) -> p r d", d=3)
                for k in range(3):
                    nc.vector.tensor_tensor(out=o3[:, :, k], in0=vsl(j0 + k)[:, :, 0],
                                            in1=rc[:], op=Alu.mult)
                return out, o3

            nrm, nrm3 = normalize3("nrm", 0)
            eye, eye3 = normalize3("eye", 3)

            def mcol(j):
                return misc[:, j:j + 1]

            cosT = ep.tile([128, NR], dt.float32, tag="cosT")
            nc.vector.tensor_scalar(out=cosT[:], in0=nrm3[:, :, 0], scalar1=mcol(9),
                                    scalar2=None, op0=Alu.mult)
            nc.vector.scalar_tensor_tensor(out=cosT[:], in0=nrm3[:, :, 1], scalar=mcol(10),
                                           in1=cosT[:], op0=Alu.mult, op1=Alu.add)
            nc.vector.scalar_tensor_tensor(out=cosT[:], in0=nrm3[:, :, 2], scalar=mcol(11),
                                           in1=cosT[:], op0=Alu.mult, op1=Alu.add)
            nc.vector.tensor_scalar(out=cosT[:], in0=cosT[:], scalar1=0.0, scalar2=1.0,
                                    op0=Alu.max, op1=Alu.min)
            cosA = ep.tile([128, NR], dt.float32, tag="cosA")
            rk = ep.tile([128, NR], dt.float32, tag="rk")
            for k in range(3):
                nc.vector.tensor_tensor(out=rk[:], in0=cosT[:], in1=nrm3[:, :, k], op=Alu.mult)
                nc.vector.tensor_scalar(out=rk[:], in0=rk[:], scalar1=2.0, scalar2=None,
                                        op0=Alu.mult)
                nc.vector.tensor_scalar(out=rk[:], in0=rk[:], scalar1=mcol(9 + k),
                                        scalar2=None, op0=Alu.subtract)
                nc.vector.tensor_tensor(out=rk[:], in0=rk[:], in1=eye3[:, :, k], op=Alu.mult)
                if k == 0:
                    nc.vector.tensor_copy(cosA[:], rk[:])
                else:
                    nc.vector.tensor_tensor(out=cosA[:], in0=cosA[:], in1=rk[:], op=Alu.add)
            nc.vector.tensor_scalar(out=cosA[:], in0=cosA[:], scalar1=1e-5, scalar2=1.0,
                                    op0=Alu.max, op1=Alu.min)
            nc.scalar.activation(cosA[:], cosA[:], Act.Ln)
            nc.scalar.activation(cosA[:], cosA[:], Act.Exp, scale=mcol(12))

            # texture lookup (u/v + gathers were produced per-chunk)
            uu = uA
            vv = vA
            # exact floor: cast to int and back, then subtract 1 where it rounded up
            cint = ep.tile([128, NR], dt.int32, tag="cint")
            fixt = ep.tile([128, NR], dt.float32, tag="fixt")

            def floorf(dst_tag, src):
                nc.vector.tensor_copy(cint[:], src[:])
                dst = ep.tile([128, NR], dt.float32, tag=dst_tag, name=dst_tag)
                nc.vector.tensor_copy(dst[:], cint[:])
                nc.vector.tensor_tensor(out=fixt[:], in0=dst[:], in1=src[:], op=Alu.is_gt)
                nc.vector.tensor_tensor(out=dst[:], in0=dst[:], in1=fixt[:], op=Alu.subtract)
                return dst

            x0f = floorf("x0f", uu)
            y0f = floorf("y0f", vv)
            wx = ep.tile([128, NR], dt.float32, tag="wx")
            nc.vector.tensor_tensor(out=wx[:], in0=uu[:], in1=x0f[:], op=Alu.subtract)
            wy = ep.tile([128, NR], dt.float32, tag="wy")
            nc.vector.tensor_tensor(out=wy[:], in0=vv[:], in1=y0f[:], op=Alu.subtract)
            gv = gathA[:].rearrange("p (r w d) -> p r w d", w=2, d=6)

            wxc = ep.tile([128, NR], dt.float32, tag="wxc")
            nc.vector.tensor_scalar(out=wxc[:], in0=wx[:], scalar1=-1.0, scalar2=1.0,
                                    op0=Alu.mult, op1=Alu.add)
            wyc = ep.tile([128, NR], dt.float32, tag="wyc")
            nc.vector.tensor_scalar(out=wyc[:], in0=wy[:], scalar1=-1.0, scalar2=1.0,
                                    op0=Alu.mult, op1=Alu.add)
            w00 = ep.tile([128, NR], dt.float32, tag="w00")
            nc.vector.tensor_tensor(out=w00[:], in0=wxc[:], in1=wyc[:], op=Alu.mult)
            w01 = ep.tile([128, NR], dt.float32, tag="w01")
            nc.vector.tensor_tensor(out=w01[:], in0=wx[:], in1=wyc[:], op=Alu.mult)
            w10 = ep.tile([128, NR], dt.float32, tag="w10")
            nc.vector.tensor_tensor(out=w10[:], in0=wxc[:], in1=wy[:], op=Alu.mult)
            w11 = ep.tile([128, NR], dt.float32, tag="w11")
            nc.vector.tensor_tensor(out=w11[:], in0=wx[:], in1=wy[:], op=Alu.mult)

            colorT = ep.tile([128, NR * 3], dt.float32, tag="colorT")
            c3v = colorT[:].rearrange("p (r d) -> p r d", d=3)
            tcc = ep.tile([128, NR], dt.float32, tag="tcc")
            acc = ep.tile([128, NR], dt.float32, tag="acc")
            mm = ep.tile([128, NR], dt.float32, tag="mm")
            for c in range(3):
                nc.vector.tensor_tensor(out=tcc[:], in0=gv[:, :, 0, c], in1=w00[:], op=Alu.mult)
                nc.vector.tensor_tensor(out=mm[:], in0=gv[:, :, 0, 3 + c], in1=w01[:], op=Alu.mult)
                nc.vector.tensor_tensor(out=tcc[:], in0=tcc[:], in1=mm[:], op=Alu.add)
                nc.vector.tensor_tensor(out=mm[:], in0=gv[:, :, 1, c], in1=w10[:], op=Alu.mult)
                nc.vector.tensor_tensor(out=tcc[:], in0=tcc[:], in1=mm[:], op=Alu.add)
                nc.vector.tensor_tensor(out=mm[:], in0=gv[:, :, 1, 3 + c], in1=w11[:], op=Alu.mult)
                nc.vector.tensor_tensor(out=tcc[:], in0=tcc[:], in1=mm[:], op=Alu.add)
                nc.vector.tensor_tensor(out=mm[:], in0=cosT[:], in1=tcc[:], op=Alu.mult)
                nc.vector.tensor_scalar(out=acc[:], in0=tcc[:], scalar1=mcol(c),
                                        scalar2=None, op0=Alu.mult)
                nc.vector.scalar_tensor_tensor(out=acc[:], in0=mm[:], scalar=mcol(3 + c),
                                               in1=acc[:], op0=Alu.mult, op1=Alu.add)
                nc.vector.scalar_tensor_tensor(out=acc[:], in0=cosA[:], scalar=mcol(6 + c),
                                               in1=acc[:], op0=Alu.mult, op1=Alu.add)
                nc.vector.tensor_tensor(out=acc[:], in0=acc[:], in1=vsl(8)[:, :, 0], op=Alu.mult)
                nc.vector.tensor_scalar(out=c3v[:, :, c], in0=acc[:], scalar1=0.0, scalar2=1.0,
                                        op0=Alu.max, op1=Alu.min)

            nc.sync.dma_start(out=imr_d[:].rearrange("r c k -> c r k"),
                              in_=colorT[:].rearrange("p (r k) -> p r k", k=3))

    nc.finalize()
    return nc


def kernel(points, faces, camera_rot, camera_pos, camera_proj, uv, ft, texture,
           lightdirect, material, shininess, height, width):
    from concourse.bass_utils import run_bass_kernel_spmd

    inputs = dict(points=points, faces=faces, camera_rot=camera_rot,
                  camera_pos=camera_pos, camera_proj=camera_proj, uv=uv, ft=ft,
                  texture=texture, lightdirect=lightdirect, material=material,
                  shininess=shininess)
    in_maps, normal1 = host_prep(inputs)

    if 'nc' not in _NC_CACHE:
        _NC_CACHE['nc'] = build_nc()
    nc = _NC_CACHE['nc']

    res = run_bass_kernel_spmd(nc, in_maps, core_ids=list(range(NCORES)))

    imrender = np.zeros((B, H, W, 3), dtype=f32)
    improb = np.zeros((B, H, W, 1), dtype=f32)
    for core in range(NCORES):
        b = core // BANDS
        r0 = (core % BANDS) * ROWS
        imrender[b, r0:r0 + ROWS] = res.results[core]['imr']
        improb[b, r0:r0 + ROWS, :, 0] = res.results[core]['imp']
    return imrender, improb, normal1


# revision 18
# speedup vs baseline: 1.0907x; 1.0907x over previous
"""Trainium2 Bass kernel for nn_PhongRender (DIB-R style Phong renderer).

kernel(**inputs) takes FULL unsharded inputs (as from setup_inputs()) and
returns (imrender [2,128,128,3], improb [2,128,128,1], normal1 [2,800,3]).

Sharding: 8 cores = 2 batches x 4 row-bands of 32 image rows each.
Each core rasterizes its 32x128 pixel band against all 800 faces.

Per-core device pipeline (f-layout [128 faces, 512 px] tiles):
  - w0n/w1n/w2n/zi maps: exact elementwise evaluation (ACT fma + DVE STT/TS)
    with the same f32 rounding sequence as the reference-validated host sim
    (hard decisions: inside tests, z-buffer winner).
  - S/Q soft-silhouette maps: PE matmul (K=4 affine basis), smooth path.
  - zmax / sum(log) partition reductions: GPSIMD all-reduce (exact f32).
  - winner one-hot (zbuf == zmax) -> feature interpolation via PE matmul
    against a precomputed per-face G table.
  - fragment shader + bilinear texture (indirect DMA gather) on-device.
"""
import numpy as np

f32 = np.float32

B, P, F, H, W, TEX = 2, 500, 800, 128, 128, 512
NCORES = 8
BANDS = 4                 # row-bands per batch
ROWS = H // BANDS         # 32 rows per core
NPX = ROWS * W            # 4096 pixels per core
CH = 512                  # pixels per chunk (4 image rows)
NCHUNK = NPX // CH        # 8
RCH = CH // W             # rows per chunk = 4
NT = (F + 127) // 128     # 7 face tiles
FP = NT * 128             # 896 padded faces
EPS = f32(1e-15)
MULT = 1000.0
DELTA = 7000.0
NEG_BIG = f32(-1e30)
ZFILL = f32(-1e10)
LOG1EM7 = float(np.log(np.float32(1e-7)))
PMAX = float(np.float32(1.0) - np.float32(1e-7))
TEXN = TEX * TEX * 3 + 16


def _fma(a, b, c):
    """f32 fused multiply-add (matches XLA CPU's fma contraction)."""
    return (a.astype(np.float64) * b.astype(np.float64) + c.astype(np.float64)).astype(f32)


def _cross_fma(a, b):
    """cross product with XLA-CPU's fma pattern: fma(a1,b2, -(a2*b1))."""
    return np.stack([
        _fma(a[:, 1], b[:, 2], -(a[:, 2] * b[:, 1]).astype(f32)),
        _fma(a[:, 2], b[:, 0], -(a[:, 0] * b[:, 2]).astype(f32)),
        _fma(a[:, 0], b[:, 1], -(a[:, 1] * b[:, 0]).astype(f32)),
    ], -1)


def host_prep(inputs):
    """Build per-core input tensor maps + host-computed normal1."""
    points = np.asarray(inputs['points'], dtype=f32)
    faces = np.asarray(inputs['faces'])
    camera_rot = np.asarray(inputs['camera_rot'], dtype=f32)
    camera_pos = np.asarray(inputs['camera_pos'], dtype=f32)
    camera_proj = np.asarray(inputs['camera_proj'], dtype=f32).reshape(3)
    uv = np.asarray(inputs['uv'], dtype=f32)
    ft = np.asarray(inputs['ft'])
    texture = np.asarray(inputs['texture'], dtype=f32)
    lightdirect = np.asarray(inputs['lightdirect'], dtype=f32)
    material = np.asarray(inputs['material'], dtype=f32)
    shininess = np.asarray(inputs['shininess'], dtype=f32)

    px = ((f32(2.0) * (np.arange(W, dtype=f32) + f32(0.5)) - f32(W)) / f32(W)).astype(f32)
    py = ((f32(H) - f32(2.0) * (np.arange(H, dtype=f32) + f32(0.5))) / f32(H)).astype(f32)

    cc = np.float64(MULT) ** 2 / np.float64(DELTA)

    normal1_out = np.zeros((B, F, 3), dtype=f32)
    per_batch = []
    for b in range(B):
        pts = points[b]
        pos = camera_pos[b]
        rot = camera_rot[b]
        pc = np.einsum('pj,ij->pi', (pts - pos[None, :]).astype(f32), rot).astype(f32)
        xy = (pc * camera_proj[None, :]).astype(f32)
        xy2 = (xy[:, :2] / xy[:, 2:3]).astype(f32)
        p3 = pc[faces]
        p2 = xy2[faces]
        v10 = (p3[:, 1] - p3[:, 0]).astype(f32)
        v20 = (p3[:, 2] - p3[:, 0]).astype(f32)
        normal = _cross_fma(v10, v20)
        normalz = normal[:, 2]
        nn = np.sqrt((normal * normal).sum(-1, keepdims=True).astype(f32)).astype(f32)
        normal1_out[b] = (normal / (nn + EPS)).astype(f32)

        ax, ay = p2[:, 0, 0], p2[:, 0, 1]
        bx, by = p2[:, 1, 0], p2[:, 1, 1]
        cx, cy = p2[:, 2, 0], p2[:, 2, 1]
        area = ((bx - ax) * (cy - ay) - (by - ay) * (cx - ax)).astype(f32)
        ok = np.abs(area) > f32(1e-10)
        inv = (f32(1.0) / np.where(ok, area, f32(1.0))).astype(f32)

        def edge_coef(ux, uy, vx, vy):
            ex = (vx - ux).astype(f32)
            ey = (vy - uy).astype(f32)
            return np.stack([(-ey).astype(f32), ex,
                             (ey * ux - ex * uy).astype(f32)], 0)

        cw0 = edge_coef(bx, by, cx, cy)
        cw1 = edge_coef(cx, cy, ax, ay)
        cw2 = edge_coef(ax, ay, bx, by)
        cw0n = (cw0 * inv[None]).astype(f32)
        cw1n = (cw1 * inv[None]).astype(f32)
        cw2n = (cw2 * inv[None]).astype(f32)
        z = p3[:, :, 2]
        czi = (cw0n * z[:, 0][None] + cw1n * z[:, 1][None] + cw2n * z[:, 2][None]).astype(f32)

        uvf = uv[b][ft]
        feat = np.concatenate([
            np.tile(normal[:, None, :], (1, 3, 1)),
            -p3, uvf, np.ones((F, 3, 1), dtype=f32)], axis=-1).astype(f32)
        cw = np.stack([cw0n, cw1n, cw2n], 0)  # [3k,3j,F]
        G = np.einsum('kjf,fkd->fjd', cw.astype(np.float64),
                      feat.astype(np.float64)).astype(f32)  # [F,3,9]

        def seg_tables(ux, uy, vx, vy):
            ex = (vx - ux).astype(np.float64); ey = (vy - uy).astype(np.float64)
            uxd = ux.astype(np.float64); uyd = uy.astype(np.float64)
            E = ex * ex + ey * ey
            S = np.stack([np.zeros_like(ex), 2 * cc * ex, 2 * cc * ey,
                          2 * cc * (-ex * uxd - ey * uyd)], 0).astype(f32)
            Q = np.stack([np.full_like(ex, cc), -2 * cc * uxd, -2 * cc * uyd,
                          cc * (uxd * uxd + uyd * uyd)], 0).astype(f32)
            IE = (1.0 / (2 * cc * (E + 1e-12))).astype(f32)
            CE = (cc * E).astype(f32)
            return S, Q, IE, CE

        Sab, Qab, IEab, CEab = seg_tables(ax, ay, bx, by)
        Sbc, Qbc, IEbc, CEbc = seg_tables(bx, by, cx, cy)
        Sca, Qca, IEca, CEca = seg_tables(cx, cy, ax, ay)

        okm1 = np.where(ok, f32(0.0), NEG_BIG).astype(f32)
        okm2 = np.where(ok & (normalz > 0), f32(0.0), NEG_BIG).astype(f32)

        def padF(a, fill=0.0):
            shp = list(a.shape); shp[-1] = FP
            out = np.full(shp, fill, dtype=f32)
            out[..., :F] = a
            return out

        SQ = np.stack([padF(Sab), padF(Sbc), padF(Sca),
                       padF(Qab), padF(Qbc), padF(Qca)], 0)  # [6,4,FP]
        SQ[3:, 3, F:] = f32(1e30)  # pad faces: a = huge -> lg = 0
        coefs = SQ.reshape(6, 4, NT, 128).transpose(2, 0, 1, 3).copy()  # [NT,6,4,128]

        WC = np.stack([padF(cw0n[0]), padF(cw0n[1]), padF(cw0n[2]),
                       padF(cw1n[0]), padF(cw1n[1]), padF(cw1n[2]),
                       padF(cw2n[0]), padF(cw2n[1]), padF(cw2n[2]),
                       padF(czi[0]), padF(czi[1]), padF(czi[2])], 0)  # [12, FP]
        wcols = WC.reshape(12, NT, 128).transpose(2, 1, 0).reshape(128, NT * 12).copy()

        FC = np.stack([padF(IEab), padF(IEbc), padF(IEca),
                       padF(CEab), padF(CEbc), padF(CEca),
                       padF(okm1, float(NEG_BIG)), padF(okm2, float(NEG_BIG))], 0)
        fcols = FC.reshape(8, NT, 128).transpose(2, 1, 0).reshape(128, NT * 8).copy()

        Gp = padF(G.reshape(F, 27).T).T.reshape(FP, 3, 9)
        gtab = Gp.reshape(NT, 128, 27)
        gtab = np.concatenate([gtab, np.zeros((NT, 128, 9), dtype=f32)], -1)
        gtab = gtab.transpose(1, 0, 2).reshape(128, NT * 36).copy()

        misc = np.zeros((128, 16), dtype=f32)
        mat = material[b]
        ld = lightdirect[b]
        ln = np.sqrt((ld * ld).sum().astype(f32)).astype(f32)
        ldn = (ld / (ln + EPS)).astype(f32)
        misc[:, 0:3] = mat[0][None, :]
        misc[:, 3:6] = mat[1][None, :]
        misc[:, 6:9] = mat[2][None, :]
        misc[:, 9:12] = ldn[None, :]
        misc[:, 12] = shininess[b, 0]

        ti = np.zeros(TEXN, dtype=f32)
        ti[:TEX * TEX * 3] = texture[b].transpose(1, 2, 0).reshape(-1)

        per_batch.append(dict(coefs=coefs, wcols=wcols, fcols=fcols, gtab=gtab,
                              misc=misc, texi=ti.reshape(TEXN, 1)))

    PXg = np.broadcast_to(px[None, :], (H, W)).reshape(-1).astype(f32)
    PYg = np.broadcast_to(py[:, None], (H, W)).reshape(-1).astype(f32)
    Rg = (PXg * PXg + PYg * PYg).astype(f32)

    in_maps = []
    for core in range(NCORES):
        b = core // BANDS
        band = core % BANDS
        r0 = band * ROWS
        sl = slice(r0 * W, (r0 + ROWS) * W)
        basis = np.stack([Rg[sl], PXg[sl], PYg[sl], np.ones(NPX, dtype=f32)], 0).copy()
        pyexp = np.broadcast_to(py[r0:r0 + ROWS][None, :, None], (128, ROWS, 9)).reshape(128, ROWS * 9).copy()
        pxcol = px[:, None].copy()
        t = per_batch[b]
        in_maps.append({
            'coefs': t['coefs'], 'wcols': t['wcols'], 'fcols': t['fcols'],
            'gtab': t['gtab'], 'misc': t['misc'], 'texi': t['texi'],
            'basis': basis, 'pyexp': pyexp, 'pxcol': pxcol,
            'pxb': np.broadcast_to(PXg[sl][None, :], (128, NPX)).copy(),
            'pyb': np.broadcast_to(PYg[sl][None, :], (128, NPX)).copy(),
        })
    return in_maps, normal1_out


_NC_CACHE = {}


def _register_seg_a():
    """Register the fused soft-prob custom DVE op:
    out = t*(t*CE - S) + Q with t = clip01(S*IE)   (S=in0, Q=in1, IE=s0, CE=s1)."""
    import concourse.dve_ops as dve_ops
    if any(o.name == "SEG_A_ANT" for o in dve_ops.OPS):
        return (next(o for o in dve_ops.OPS if o.name == "SEG_A_ANT"),
                next(o for o in dve_ops.OPS if o.name == "ZSEL_ANT"),
                next(o for o in dve_ops.OPS if o.name == "ZSEL2_ANT"))
    from concourse.dve_spec import Spec, Src0, Src1, C0, C1, C2, relu, minn
    from concourse.dve_table_gen import dve_ver_for

    def ref_seg_a(in0, in1, s0, s1, imm2):
        t = np.minimum(np.maximum(in0.astype(np.float32) * s0, 0), imm2).astype(np.float32)
        return (t * (t * s1 - in0) + in1).astype(np.float32)

    t_expr = minn(relu(Src0 * C0), C2)
    spec = Spec(body=t_expr * (t_expr * C1 - Src0) + Src1, reference=ref_seg_a)
    row = dve_ops._CUSTOM_DVE_ROW_BASE + len(dve_ops.OPS)
    op = dve_ops.DveOp("SEG_A_ANT", spec, subdim=False,
                       uops_sha={"v3": "a415bb456e75ca33"})
    dve_ops.OPS.append(op)
    dve_ops._SUB_OPCODE_FOR_NAME[op.name] = row
    dve_ops.CUSTOM_DVE_SPECS[op.name] = spec

    def ref_zsel(in0, in1, s0, s1, imm2):
        return (in0.astype(np.float32) * in1 + (in0 - imm2) * s0).astype(np.float32)

    spec2 = Spec(body=Src0 * Src1 + (Src0 - C2) * C0, reference=ref_zsel)
    row2 = dve_ops._CUSTOM_DVE_ROW_BASE + len(dve_ops.OPS)
    op2 = dve_ops.DveOp("ZSEL_ANT", spec2, subdim=False,
                        uops_sha={"v3": "4c6249d316d2ba3d"})
    dve_ops.OPS.append(op2)
    dve_ops._SUB_OPCODE_FOR_NAME[op2.name] = row2
    dve_ops.CUSTOM_DVE_SPECS[op2.name] = spec2

    from concourse.dve_spec import Zero, select

    def ref_zsel2(in0, in1, s0, s1, imm2):
        return np.where((in0.astype(np.float32) + s0) >= 0,
                        (in1.astype(np.float32) + s1).astype(np.float32),
                        np.float32(imm2)).astype(np.float32)

    spec3 = Spec(body=select((Src0 + C0) >= Zero, Src1 + C1, C2), reference=ref_zsel2)
    row3 = dve_ops._CUSTOM_DVE_ROW_BASE + len(dve_ops.OPS)
    op3 = dve_ops.DveOp("ZSEL2_ANT", spec3, subdim=False,
                        uops_sha={"v3": "c08a5da8e56941a5"})
    dve_ops.OPS.append(op3)
    dve_ops._SUB_OPCODE_FOR_NAME[op3.name] = row3
    dve_ops.CUSTOM_DVE_SPECS[op3.name] = spec3
    return op, op2, op3


def build_nc():
    import concourse.bacc as bacc
    import concourse.bass as bass
    import concourse.mybir as mybir
    import concourse.bass_isa as bass_isa
    from concourse.tile import TileContext

    dt = mybir.dt
    Alu = mybir.AluOpType
    Act = mybir.ActivationFunctionType

    seg_a_op, zsel_op, zsel2_op = _register_seg_a()
    nc = bacc.Bacc(trn_type="TRN2")
    coefs_d = nc.dram_tensor("coefs", [NT, 6, 4, 128], dt.float32, kind="ExternalInput")
    wcols_d = nc.dram_tensor("wcols", [128, NT * 12], dt.float32, kind="ExternalInput")
    fcols_d = nc.dram_tensor("fcols", [128, NT * 8], dt.float32, kind="ExternalInput")
    gtab_d = nc.dram_tensor("gtab", [128, NT * 36], dt.float32, kind="ExternalInput")
    misc_d = nc.dram_tensor("misc", [128, 16], dt.float32, kind="ExternalInput")
    texi_d = nc.dram_tensor("texi", [TEXN, 1], dt.float32, kind="ExternalInput")
    basis_d = nc.dram_tensor("basis", [4, NPX], dt.float32, kind="ExternalInput")
    pxb_d = nc.dram_tensor("pxb", [128, NPX], dt.float32, kind="ExternalInput")
    pyb_d = nc.dram_tensor("pyb", [128, NPX], dt.float32, kind="ExternalInput")
    pyexp_d = nc.dram_tensor("pyexp", [128, ROWS * 9], dt.float32, kind="ExternalInput")
    pxcol_d = nc.dram_tensor("pxcol", [128, 1], dt.float32, kind="ExternalInput")
    imr_d = nc.dram_tensor("imr", [ROWS, W, 3], dt.float32, kind="ExternalOutput")
    imp_d = nc.dram_tensor("imp", [ROWS, W], dt.float32, kind="ExternalOutput")

    with TileContext(nc) as tc:
        with tc.tile_pool(name="cb", bufs=1) as cb, \
             tc.tile_pool(name="zb", bufs=1) as zbp, \
             tc.tile_pool(name="wk", bufs=2) as wk, \
             tc.tile_pool(name="ep", bufs=1) as ep, \
             tc.tile_pool(name="ps1", bufs=1, space="PSUM") as ps1, \
             tc.tile_pool(name="ps2", bufs=2, space="PSUM") as ps2, \
             tc.tile_pool(name="dr", bufs=1, space="DRAM") as dr:

            # ---- constants into SBUF ----
            coefs_sb = cb.tile([4, NT * 6 * 128], dt.float32, tag="coefs")
            nc.sync.dma_start(out=coefs_sb[:].rearrange("k (t m j) -> k t m j", t=NT, m=6),
                              in_=coefs_d[:].rearrange("t m k j -> k t m j"))
            wcols = cb.tile([128, NT * 12], dt.float32, tag="wcols")
            nc.sync.dma_start(out=wcols[:], in_=wcols_d[:])
            fcols = cb.tile([128, NT * 8], dt.float32, tag="fcols")
            nc.sync.dma_start(out=fcols[:], in_=fcols_d[:])
            gtab = cb.tile([128, NT * 36], dt.float32, tag="gtab")
            nc.sync.dma_start(out=gtab[:], in_=gtab_d[:])
            misc = cb.tile([128, 16], dt.float32, tag="misc")
            nc.sync.dma_start(out=misc[:], in_=misc_d[:])

            pyexp = cb.tile([128, ROWS * 9], dt.float32, tag="pyexp")
            nc.sync.dma_start(out=pyexp[:], in_=pyexp_d[:])
            pxcol = cb.tile([128, 1], dt.float32, tag="pxcol")
            nc.sync.dma_start(out=pxcol[:], in_=pxcol_d[:])
            m16 = cb.tile([128, CH], dt.float32, tag="m16")
            nc.vector.memset(m16[:], LOG1EM7)
            ones = cb.tile([128, 1], dt.float32, tag="ones")
            nc.vector.memset(ones[:], 1.0)

            imfeat = ep.tile([128, ROWS * 36], dt.float32, tag="imfeat")
            covT = ep.tile([128, ROWS], dt.float32, tag="covT")
            covd = dr.tile([ROWS, W], dt.float32, tag="covd")
            gathA = ep.tile([128, ROWS * 2 * 6], dt.float32, tag="gathA")
            uA = ep.tile([128, ROWS], dt.float32, tag="uA")
            vA = ep.tile([128, ROWS], dt.float32, tag="vA")

            def fcol(t, j):
                return fcols[:, t * 8 + j:t * 8 + j + 1]

            def wcol(t, j):
                return wcols[:, t * 12 + j:t * 12 + j + 1]

            # ---------------- main per-chunk loop ----------------
            for ch in range(NCHUNK):
                pxs = slice(ch * CH, (ch + 1) * CH)
                bas = wk.tile([4, CH], dt.float32, tag="bas")
                nc.sync.dma_start(out=bas[:], in_=basis_d[:, pxs])
                pxbc = wk.tile([128, CH], dt.float32, tag="pxbc")
                nc.sync.dma_start(out=pxbc[:], in_=pxb_d[:, pxs])
                pybc = wk.tile([128, CH], dt.float32, tag="pybc")
                nc.sync.dma_start(out=pybc[:], in_=pyb_d[:, pxs])

                zts = []
                sg = ps1.tile([1, CH], dt.float32, tag="sg", space="PSUM")
                in_tiles = []
                pc_tiles = []
                for t in range(NT):
                    # ---- exact w/zi maps: s_m = PX*c0 + PY*c1 (ACT fma + STT) ----
                    ss = []
                    for m in range(4):
                        u = wk.tile([128, CH], dt.float32, tag=f"u{m}", name=f"u{m}")
                        nc.scalar.activation(u[:], pxbc[:], Act.Copy,
                                             scale=wcol(t, m * 3 + 0))
                        nc.vector.scalar_tensor_tensor(
                            out=u[:], in0=pybc[:], scalar=wcol(t, m * 3 + 1),
                            in1=u[:], op0=Alu.mult, op1=Alu.add)
                        ss.append(u)
                    w1 = wk.tile([128, CH], dt.float32, tag="w1")
                    nc.vector.tensor_scalar(out=w1[:], in0=ss[1][:],
                                            scalar1=wcol(t, 1 * 3 + 2), scalar2=None,
                                            op0=Alu.add)
                    m1 = wk.tile([128, CH], dt.float32, tag="m1")
                    nc.vector.scalar_tensor_tensor(
                        out=m1[:], in0=ss[0][:], scalar=wcol(t, 0 * 3 + 2),
                        in1=w1[:], op0=Alu.add, op1=Alu.min)
                    m3 = wk.tile([128, CH], dt.float32, tag="m3")
                    nc.vector.scalar_tensor_tensor(
                        out=m3[:], in0=ss[2][:], scalar=wcol(t, 2 * 3 + 2),
                        in1=m1[:], op0=Alu.add, op1=Alu.min)
                    in01 = zbp.tile([128, CH], dt.uint8, tag=f"i{t}", name=f"i{t}")
                    nc.vector.tensor_scalar(out=in01[:], in0=m3[:], scalar1=fcol(t, 6),
                                            scalar2=0.0, op0=Alu.add, op1=Alu.is_ge)
                    in_tiles.append(in01)
                    zb = zbp.tile([128, CH], dt.float32, tag=f"z{t}")
                    nc.vector._custom_dve(zsel2_op, out=zb[:], in0=m3[:], in1=ss[3][:],
                                          s0=fcol(t, 7), s1=wcol(t, 3 * 3 + 2),
                                          imm2=float(ZFILL))
                    zts.append(zb)

                    # ---- soft-prob path (PE maps) ----
                    a_list = []
                    for e in range(3):
                        Sp = ps2.tile([128, CH], dt.float32, tag="qS", space="PSUM")
                        nc.tensor.matmul(
                            Sp[:], coefs_sb[:, (t * 6 + e) * 128:(t * 6 + e + 1) * 128],
                            bas[:], start=True, stop=True)
                        Qp = ps2.tile([128, CH], dt.float32, tag="qQ", space="PSUM")
                        nc.tensor.matmul(
                            Qp[:], coefs_sb[:, (t * 6 + e + 3) * 128:(t * 6 + e + 4) * 128],
                            bas[:], start=True, stop=True)
                        Qs = wk.tile([128, CH], dt.float32, tag="Qs")
                        nc.scalar.activation(Qs[:], Qp[:], Act.Copy)
                        a = wk.tile([128, CH], dt.float32, tag=f"a{e}")
                        nc.vector._custom_dve(seg_a_op, out=a[:], in0=Sp[:], in1=Qs[:],
                                              s0=fcol(t, e), s1=fcol(t, 3 + e), imm2=1.0)
                        a_list.append(a)
                    am = wk.tile([128, CH], dt.float32, tag="am")
                    nc.vector.tensor_tensor(out=am[:], in0=a_list[0][:], in1=a_list[1][:], op=Alu.min)
                    am2 = wk.tile([128, CH], dt.float32, tag="am2")
                    nc.vector.tensor_tensor(out=am2[:], in0=am[:], in1=a_list[2][:], op=Alu.min)
                    pp = wk.tile([128, CH], dt.float32, tag="pp")
                    nc.scalar.activation(pp[:], am2[:], Act.Exp, scale=-1.0)
                    pct = zbp.tile([128, CH], dt.float32, tag=f"pc{t}", name=f"pc{t}")
                    nc.vector.tensor_scalar(out=pct[:], in0=pp[:], scalar1=PMAX,
                                            scalar2=None, op0=Alu.min)
                    pc_tiles.append(pct)

                # ---- batched Ln pass (one ACT table switch per chunk) ----
                for t in range(NT):
                    lg = wk.tile([128, CH], dt.float32, tag="lg")
                    nc.scalar.activation(lg[:], pc_tiles[t][:], Act.Ln, scale=-1.0, bias=1.0)
                    nc.vector.copy_predicated(out=lg[:], mask=in_tiles[t][:], data=m16[:])
                    nc.tensor.matmul(sg[:], ones[:], lg[:], start=(t == 0),
                                     stop=(t == NT - 1), skip_group_check=True)

                # ---- zmax + sum(lg) partition reductions ----
                zmt = wk.tile([128, CH], dt.float32, tag="zmt")
                nc.vector.tensor_tensor(out=zmt[:], in0=zts[0][:], in1=zts[1][:], op=Alu.max)
                for t in range(2, NT):
                    nc.vector.tensor_tensor(out=zmt[:], in0=zmt[:], in1=zts[t][:], op=Alu.max)
                zmaxb = wk.tile([128, CH], dt.float32, tag="zmaxb")
                nc.gpsimd.partition_all_reduce(zmaxb[:], zmt[:], 128, bass_isa.ReduceOp.max)

                # ---- one-hot winner + feature matmul (accumulate over tiles) ----
                gall = ps1.tile([128, RCH * 36], dt.float32, tag="gall", space="PSUM")
                ohs = []
                for t in range(NT):
                    oh = zbp.tile([128, CH], dt.float32, tag=f"oh{t}", name=f"oh{t}")
                    nc.vector.tensor_tensor(out=oh[:], in0=zts[t][:], in1=zmaxb[:], op=Alu.is_equal)
                    ohs.append(oh)
                for blk in range(RCH):
                    for t in range(NT):
                        nc.tensor.matmul(gall[:, blk * 36:(blk + 1) * 36],
                                         ohs[t][:, blk * 128:(blk + 1) * 128],
                                         gtab[:, t * 36:(t + 1) * 36],
                                         start=(t == 0), stop=(t == NT - 1),
                                         skip_group_check=True)
                nc.scalar.activation(
                    imfeat[:, ch * RCH * 36:(ch + 1) * RCH * 36], gall[:], Act.Copy)

                # --- texcoords + texture gathers for this chunk's 4 rows ---
                ut = wk.tile([128, RCH], dt.float32, tag="ut")
                vt = wk.tile([128, RCH], dt.float32, tag="vt")
                tt1 = wk.tile([128, RCH], dt.float32, tag="tt1")
                idf = wk.tile([128, RCH * 2], dt.float32, tag="idf")
                idi = wk.tile([128, RCH * 2], dt.int32, tag="idi")
                imfc = imfeat[:, ch * RCH * 36:(ch + 1) * RCH * 36].rearrange(
                    "p (r j d) -> p r j d", j=4, d=9)
                for j, dst in ((6, ut), (7, vt)):
                    nc.vector.tensor_scalar(out=tt1[:], in0=imfc[:, :, 0, j],
                                            scalar1=pxcol[:, 0:1], scalar2=None, op0=Alu.mult)
                    nc.vector.tensor_tensor(out=dst[:], in0=imfc[:, :, 1, j],
                                            in1=pyexp[:].rearrange("p (r d) -> p r d", d=9)[
                                                :, ch * RCH:(ch + 1) * RCH, 0],
                                            op=Alu.mult)
                    nc.vector.tensor_tensor(out=dst[:], in0=dst[:], in1=tt1[:], op=Alu.add)
                    nc.vector.tensor_tensor(out=dst[:], in0=dst[:], in1=imfc[:, :, 2, j], op=Alu.add)
                # u = clip(u,0,1)*511 ; v = (1-clip(v,0,1))*511
                nc.vector.tensor_scalar(out=ut[:], in0=ut[:], scalar1=0.0, scalar2=1.0,
                                        op0=Alu.max, op1=Alu.min)
                nc.vector.tensor_scalar(out=ut[:], in0=ut[:], scalar1=float(TEX - 1),
                                        scalar2=None, op0=Alu.mult)
                nc.vector.tensor_scalar(out=vt[:], in0=vt[:], scalar1=0.0, scalar2=1.0,
                                        op0=Alu.max, op1=Alu.min)
                nc.vector.tensor_scalar(out=vt[:], in0=vt[:], scalar1=-1.0, scalar2=1.0,
                                        op0=Alu.mult, op1=Alu.add)
                nc.vector.tensor_scalar(out=vt[:], in0=vt[:], scalar1=float(TEX - 1),
                                        scalar2=None, op0=Alu.mult)
                nc.vector.tensor_copy(uA[:, ch * RCH:(ch + 1) * RCH], ut[:])
                nc.vector.tensor_copy(vA[:, ch * RCH:(ch + 1) * RCH], vt[:])
                # floor (exact, rounding-agnostic)
                ci = wk.tile([128, RCH], dt.int32, tag="ci")
                fx = wk.tile([128, RCH], dt.float32, tag="fx")
                x0c = wk.tile([128, RCH], dt.float32, tag="x0c")
                y0c = wk.tile([128, RCH], dt.float32, tag="y0c")
                for srcv, dstv in ((ut, x0c), (vt, y0c)):
                    nc.vector.tensor_copy(ci[:], srcv[:])
                    nc.vector.tensor_copy(dstv[:], ci[:])
                    nc.vector.tensor_tensor(out=fx[:], in0=dstv[:], in1=srcv[:], op=Alu.is_gt)
                    nc.vector.tensor_tensor(out=dstv[:], in0=dstv[:], in1=fx[:], op=Alu.subtract)
                y1c = wk.tile([128, RCH], dt.float32, tag="y1c")
                nc.vector.tensor_scalar(out=y1c[:], in0=y0c[:], scalar1=1.0,
                                        scalar2=float(TEX - 1), op0=Alu.add, op1=Alu.min)
                idv = idf[:].rearrange("p (r d) -> p r d", d=2)
                nc.vector.scalar_tensor_tensor(out=idv[:, :, 0], in0=y0c[:], scalar=float(TEX),
                                               in1=x0c[:], op0=Alu.mult, op1=Alu.add)
                nc.vector.scalar_tensor_tensor(out=idv[:, :, 1], in0=y1c[:], scalar=float(TEX),
                                               in1=x0c[:], op0=Alu.mult, op1=Alu.add)
                nc.vector.tensor_scalar(out=idf[:], in0=idf[:], scalar1=3.0, scalar2=None,
                                        op0=Alu.mult)
                nc.vector.tensor_copy(idi[:], idf[:])
                for r in range(RCH):
                    for wch in range(2):
                        gi = (ch * RCH + r) * 2 + wch
                        nc.gpsimd.indirect_dma_start(
                            out=gathA[:, gi * 6:(gi + 1) * 6], out_offset=None,
                            in_=texi_d[:],
                            in_offset=bass.IndirectOffsetOnAxis(
                                ap=idi[:, r * 2 + wch:r * 2 + wch + 1], axis=0))

                # ---- coverage + improb for this chunk ----
                cr = ep.tile([1, CH], dt.float32, tag="cr")
                nc.vector.tensor_scalar(out=cr[:], in0=zmaxb[0:1, :], scalar1=float(ZFILL),
                                        scalar2=None, op0=Alu.is_gt)
                nc.gpsimd.dma_start(
                    out=covd[ch * RCH:(ch + 1) * RCH, :], in_=cr[:])
                exr = ep.tile([1, CH], dt.float32, tag="exr")
                nc.scalar.activation(exr[:], sg[:], Act.Exp)
                impc = ep.tile([1, CH], dt.float32, tag="impc")
                nc.vector.tensor_scalar(out=impc[:], in0=exr[:], scalar1=-1.0,
                                        scalar2=1.0, op0=Alu.mult, op1=Alu.add)
                nc.sync.dma_start(
                    out=imp_d[ch * RCH:(ch + 1) * RCH, :].rearrange("r c -> r c")[None, :, :],
                    in_=impc[:].rearrange("a (r c) -> a r c", c=W))

            # ---------------- core epilogue ([col, row] layout) ----------------
            NR = ROWS
            nc.gpsimd.dma_start(out=covT[:], in_=covd[:].rearrange("r c -> c r"))
            i3 = imfeat[:].rearrange("p (r j d) -> p r j d", j=4, d=9)
            imf9 = ep.tile([128, NR * 9], dt.float32, tag="imf9")
            v9 = imf9[:].rearrange("p (r d) -> p r d", d=9)
            tmp9 = ep.tile([128, NR * 9], dt.float32, tag="tmp9")
            t9 = tmp9[:].rearrange("p (r d) -> p r d", d=9)
            nc.vector.tensor_scalar(out=t9[:, :, :], in0=i3[:, :, 0, :],
                                    scalar1=pxcol[:, 0:1], scalar2=None, op0=Alu.mult)
            nc.vector.tensor_tensor(out=v9[:, :, :], in0=i3[:, :, 1, :],
                                    in1=pyexp[:].rearrange("p (r d) -> p r d", d=9),
                                    op=Alu.mult)
            nc.vector.tensor_tensor(out=v9[:, :, :], in0=v9[:, :, :], in1=t9[:, :, :], op=Alu.add)
            nc.vector.tensor_tensor(out=v9[:, :, :], in0=v9[:, :, :], in1=i3[:, :, 2, :], op=Alu.add)

            def vsl(j, k=1):
                return imf9[:].rearrange("p (r d) -> p r d", d=9)[:, :, j:j + k]

            nc.vector.tensor_tensor(out=vsl(8)[:, :, 0], in0=vsl(8)[:, :, 0],
                                    in1=covT[:], op=Alu.mult)

            def normalize3(dst_tag, j0):
                sq = ep.tile([128, NR * 3], dt.float32, tag=dst_tag + "sq")
                s3 = sq[:].rearrange("p (r d) -> p r d", d=3)
                nc.vector.tensor_tensor(out=s3[:], in0=vsl(j0, 3)[:, :, :],
                                        in1=vsl(j0, 3)[:, :, :], op=Alu.mult)
                n2 = ep.tile([128, NR], dt.float32, tag=dst_tag + "n2")
                nc.vector.tensor_reduce(out=n2[:], in_=s3[:], axis=mybir.AxisListType.X,
                                        op=Alu.add)
                sn = ep.tile([128, NR], dt.float32, tag=dst_tag + "sn")
                nc.scalar.activation(sn[:], n2[:], Act.Sqrt)
                nc.vector.tensor_scalar(out=sn[:], in0=sn[:], scalar1=float(EPS),
                                        scalar2=None, op0=Alu.add)
                rc = ep.tile([128, NR], dt.float32, tag=dst_tag + "rc")
                nc.vector.reciprocal(rc[:], sn[:])
                out = ep.tile([128, NR * 3], dt.float32, tag=dst_tag)
                o3 = out[:].rearrange("p (r d) -> p r d", d=3)
                for k in range(3):
                    nc.vector.tensor_tensor(out=o3[:, :, k], in0=vsl(j0 + k)[:, :, 0],
                                            in1=rc[:], op=Alu.mult)
                return out, o3

            nrm, nrm3 = normalize3("nrm", 0)
            eye, eye3 = normalize3("eye", 3)

            def mcol(j):
                return misc[:, j:j + 1]

            cosT = ep.tile([128, NR], dt.float32, tag="cosT")
            nc.vector.tensor_scalar(out=cosT[:], in0=nrm3[:, :, 0], scalar1=mcol(9),
                                    scalar2=None, op0=Alu.mult)
            nc.vector.scalar_tensor_tensor(out=cosT[:], in0=nrm3[:, :, 1], scalar=mcol(10),
                                           in1=cosT[:], op0=Alu.mult, op1=Alu.add)
            nc.vector.scalar_tensor_tensor(out=cosT[:], in0=nrm3[:, :, 2], scalar=mcol(11),
                                           in1=cosT[:], op0=Alu.mult, op1=Alu.add)
            nc.vector.tensor_scalar(out=cosT[:], in0=cosT[:], scalar1=0.0, scalar2=1.0,
                                    op0=Alu.max, op1=Alu.min)
            cosA = ep.tile([128, NR], dt.float32, tag="cosA")
            rk = ep.tile([128, NR], dt.float32, tag="rk")
            for k in range(3):
                nc.vector.tensor_tensor(out=rk[:], in0=cosT[:], in1=nrm3[:, :, k], op=Alu.mult)
                nc.vector.tensor_scalar(out=rk[:], in0=rk[:], scalar1=2.0, scalar2=None,
                                        op0=Alu.mult)
                nc.vector.tensor_scalar(out=rk[:], in0=rk[:], scalar1=mcol(9 + k),
                                        scalar2=None, op0=Alu.subtract)
                nc.vector.tensor_tensor(out=rk[:], in0=rk[:], in1=eye3[:, :, k], op=Alu.mult)
                if k == 0:
                    nc.vector.tensor_copy(cosA[:], rk[:])
                else:
                    nc.vector.tensor_tensor(out=cosA[:], in0=cosA[:], in1=rk[:], op=Alu.add)
            nc.vector.tensor_scalar(out=cosA[:], in0=cosA[:], scalar1=1e-5, scalar2=1.0,
                                    op0=Alu.max, op1=Alu.min)
            nc.scalar.activation(cosA[:], cosA[:], Act.Ln)
            nc.scalar.activation(cosA[:], cosA[:], Act.Exp, scale=mcol(12))

            # texture lookup (u/v + gathers were produced per-chunk)
            uu = uA
            vv = vA
            # exact floor: cast to int and back, then subtract 1 where it rounded up
            cint = ep.tile([128, NR], dt.int32, tag="cint")
            fixt = ep.tile([128, NR], dt.float32, tag="fixt")

            def floorf(dst_tag, src):
                nc.vector.tensor_copy(cint[:], src[:])
                dst = ep.tile([128, NR], dt.float32, tag=dst_tag, name=dst_tag)
                nc.vector.tensor_copy(dst[:], cint[:])
                nc.vector.tensor_tensor(out=fixt[:], in0=dst[:], in1=src[:], op=Alu.is_gt)
                nc.vector.tensor_tensor(out=dst[:], in0=dst[:], in1=fixt[:], op=Alu.subtract)
                return dst

            x0f = floorf("x0f", uu)
            y0f = floorf("y0f", vv)
            wx = ep.tile([128, NR], dt.float32, tag="wx")
            nc.vector.tensor_tensor(out=wx[:], in0=uu[:], in1=x0f[:], op=Alu.subtract)
            wy = ep.tile([128, NR], dt.float32, tag="wy")
            nc.vector.tensor_tensor(out=wy[:], in0=vv[:], in1=y0f[:], op=Alu.subtract)
            gv = gathA[:].rearrange("p (r w d) -> p r w d", w=2, d=6)

            wxc = ep.tile([128, NR], dt.float32, tag="wxc")
            nc.vector.tensor_scalar(out=wxc[:], in0=wx[:], scalar1=-1.0, scalar2=1.0,
                                    op0=Alu.mult, op1=Alu.add)
            wyc = ep.tile([128, NR], dt.float32, tag="wyc")
            nc.vector.tensor_scalar(out=wyc[:], in0=wy[:], scalar1=-1.0, scalar2=1.0,
                                    op0=Alu.mult, op1=Alu.add)
            w00 = ep.tile([128, NR], dt.float32, tag="w00")
            nc.vector.tensor_tensor(out=w00[:], in0=wxc[:], in1=wyc[:], op=Alu.mult)
            w01 = ep.tile([128, NR], dt.float32, tag="w01")
            nc.vector.tensor_tensor(out=w01[:], in0=wx[:], in1=wyc[:], op=Alu.mult)
            w10 = ep.tile([128, NR], dt.float32, tag="w10")
            nc.vector.tensor_tensor(out=w10[:], in0=wxc[:], in1=wy[:], op=Alu.mult)
            w11 = ep.tile([128, NR], dt.float32, tag="w11")
            nc.vector.tensor_tensor(out=w11[:], in0=wx[:], in1=wy[:], op=Alu.mult)

            colorT = ep.tile([128, NR * 3], dt.float32, tag="colorT")
            c3v = colorT[:].rearrange("p (r d) -> p r d", d=3)
            tcc = ep.tile([128, NR], dt.float32, tag="tcc")
            acc = ep.tile([128, NR], dt.float32, tag="acc")
            mm = ep.tile([128, NR], dt.float32, tag="mm")
            for c in range(3):
                nc.vector.tensor_tensor(out=tcc[:], in0=gv[:, :, 0, c], in1=w00[:], op=Alu.mult)
                nc.vector.tensor_tensor(out=mm[:], in0=gv[:, :, 0, 3 + c], in1=w01[:], op=Alu.mult)
                nc.vector.tensor_tensor(out=tcc[:], in0=tcc[:], in1=mm[:], op=Alu.add)
                nc.vector.tensor_tensor(out=mm[:], in0=gv[:, :, 1, c], in1=w10[:], op=Alu.mult)
                nc.vector.tensor_tensor(out=tcc[:], in0=tcc[:], in1=mm[:], op=Alu.add)
                nc.vector.tensor_tensor(out=mm[:], in0=gv[:, :, 1, 3 + c], in1=w11[:], op=Alu.mult)
                nc.vector.tensor_tensor(out=tcc[:], in0=tcc[:], in1=mm[:], op=Alu.add)
                nc.vector.tensor_tensor(out=mm[:], in0=cosT[:], in1=tcc[:], op=Alu.mult)
                nc.vector.tensor_scalar(out=acc[:], in0=tcc[:], scalar1=mcol(c),
                                        scalar2=None, op0=Alu.mult)
                nc.vector.scalar_tensor_tensor(out=acc[:], in0=mm[:], scalar=mcol(3 + c),
                                               in1=acc[:], op0=Alu.mult, op1=Alu.add)
                nc.vector.scalar_tensor_tensor(out=acc[:], in0=cosA[:], scalar=mcol(6 + c),
                                               in1=acc[:], op0=Alu.mult, op1=Alu.add)
                nc.vector.tensor_tensor(out=acc[:], in0=acc[:], in1=vsl(8)[:, :, 0], op=Alu.mult)
                nc.vector.tensor_scalar(out=c3v[:, :, c], in0=acc[:], scalar1=0.0, scalar2=1.0,
                                        op0=Alu.max, op1=Alu.min)

            nc.sync.dma_start(out=imr_d[:].rearrange("r c k -> c r k"),
                              in_=colorT[:].rearrange("p (r k) -> p r k", k=3))

    nc.finalize()
    return nc


def kernel(points, faces, camera_rot, camera_pos, camera_proj, uv, ft, texture,
           lightdirect, material, shininess, height, width):
    from concourse.bass_utils import run_bass_kernel_spmd

    inputs = dict(points=points, faces=faces, camera_rot=camera_rot,
                  camera_pos=camera_pos, camera_proj=camera_proj, uv=uv, ft=ft,
                  texture=texture, lightdirect=lightdirect, material=material,
                  shininess=shininess)
    in_maps, normal1 = host_prep(inputs)

    if 'nc' not in _NC_CACHE:
        _NC_CACHE['nc'] = build_nc()
    nc = _NC_CACHE['nc']

    res = run_bass_kernel_spmd(nc, in_maps, core_ids=list(range(NCORES)))

    imrender = np.zeros((B, H, W, 3), dtype=f32)
    improb = np.zeros((B, H, W, 1), dtype=f32)
    for core in range(NCORES):
        b = core // BANDS
        r0 = (core % BANDS) * ROWS
        imrender[b, r0:r0 + ROWS] = res.results[core]['imr']
        improb[b, r0:r0 + ROWS, :, 0] = res.results[core]['imp']
    return imrender, improb, normal1


# revision 19
# speedup vs baseline: 1.1315x; 1.0375x over previous
"""Trainium2 Bass kernel for nn_PhongRender (DIB-R style Phong renderer).

kernel(**inputs) takes FULL unsharded inputs (as from setup_inputs()) and
returns (imrender [2,128,128,3], improb [2,128,128,1], normal1 [2,800,3]).

Sharding: 8 cores = 2 batches x 4 row-bands of 32 image rows each.
Each core rasterizes its 32x128 pixel band against all 800 faces.

Per-core device pipeline (f-layout [128 faces, 512 px] tiles):
  - w0n/w1n/w2n/zi maps: exact elementwise evaluation (ACT fma + DVE STT/TS)
    with the same f32 rounding sequence as the reference-validated host sim
    (hard decisions: inside tests, z-buffer winner).
  - S/Q soft-silhouette maps: PE matmul (K=4 affine basis), smooth path.
  - zmax / sum(log) partition reductions: GPSIMD all-reduce (exact f32).
  - winner one-hot (zbuf == zmax) -> feature interpolation via PE matmul
    against a precomputed per-face G table.
  - fragment shader + bilinear texture (indirect DMA gather) on-device.
"""
import numpy as np

f32 = np.float32

B, P, F, H, W, TEX = 2, 500, 800, 128, 128, 512
NCORES = 8
BANDS = 4                 # row-bands per batch
ROWS = H // BANDS         # 32 rows per core
NPX = ROWS * W            # 4096 pixels per core
CH = 512                  # pixels per chunk (4 image rows)
NCHUNK = NPX // CH        # 8
RCH = CH // W             # rows per chunk = 4
NT = (F + 127) // 128     # 7 face tiles
FP = NT * 128             # 896 padded faces
EPS = f32(1e-15)
MULT = 1000.0
DELTA = 7000.0
NEG_BIG = f32(-1e30)
ZFILL = f32(-1e10)
LOG1EM7 = float(np.log(np.float32(1e-7)))
PMAX = float(np.float32(1.0) - np.float32(1e-7))
TEXN = TEX * TEX * 3 + 16


def _fma(a, b, c):
    """f32 fused multiply-add (matches XLA CPU's fma contraction)."""
    return (a.astype(np.float64) * b.astype(np.float64) + c.astype(np.float64)).astype(f32)


def _cross_fma(a, b):
    """cross product with XLA-CPU's fma pattern: fma(a1,b2, -(a2*b1))."""
    return np.stack([
        _fma(a[:, 1], b[:, 2], -(a[:, 2] * b[:, 1]).astype(f32)),
        _fma(a[:, 2], b[:, 0], -(a[:, 0] * b[:, 2]).astype(f32)),
        _fma(a[:, 0], b[:, 1], -(a[:, 1] * b[:, 0]).astype(f32)),
    ], -1)


def host_prep(inputs):
    """Build per-core input tensor maps + host-computed normal1."""
    points = np.asarray(inputs['points'], dtype=f32)
    faces = np.asarray(inputs['faces'])
    camera_rot = np.asarray(inputs['camera_rot'], dtype=f32)
    camera_pos = np.asarray(inputs['camera_pos'], dtype=f32)
    camera_proj = np.asarray(inputs['camera_proj'], dtype=f32).reshape(3)
    uv = np.asarray(inputs['uv'], dtype=f32)
    ft = np.asarray(inputs['ft'])
    texture = np.asarray(inputs['texture'], dtype=f32)
    lightdirect = np.asarray(inputs['lightdirect'], dtype=f32)
    material = np.asarray(inputs['material'], dtype=f32)
    shininess = np.asarray(inputs['shininess'], dtype=f32)

    px = ((f32(2.0) * (np.arange(W, dtype=f32) + f32(0.5)) - f32(W)) / f32(W)).astype(f32)
    py = ((f32(H) - f32(2.0) * (np.arange(H, dtype=f32) + f32(0.5))) / f32(H)).astype(f32)

    cc = np.float64(MULT) ** 2 / np.float64(DELTA)

    normal1_out = np.zeros((B, F, 3), dtype=f32)
    per_batch = []
    for b in range(B):
        pts = points[b]
        pos = camera_pos[b]
        rot = camera_rot[b]
        pc = np.einsum('pj,ij->pi', (pts - pos[None, :]).astype(f32), rot).astype(f32)
        xy = (pc * camera_proj[None, :]).astype(f32)
        xy2 = (xy[:, :2] / xy[:, 2:3]).astype(f32)
        p3 = pc[faces]
        p2 = xy2[faces]
        v10 = (p3[:, 1] - p3[:, 0]).astype(f32)
        v20 = (p3[:, 2] - p3[:, 0]).astype(f32)
        normal = _cross_fma(v10, v20)
        normalz = normal[:, 2]
        nn = np.sqrt((normal * normal).sum(-1, keepdims=True).astype(f32)).astype(f32)
        normal1_out[b] = (normal / (nn + EPS)).astype(f32)

        ax, ay = p2[:, 0, 0], p2[:, 0, 1]
        bx, by = p2[:, 1, 0], p2[:, 1, 1]
        cx, cy = p2[:, 2, 0], p2[:, 2, 1]
        area = ((bx - ax) * (cy - ay) - (by - ay) * (cx - ax)).astype(f32)
        ok = np.abs(area) > f32(1e-10)
        inv = (f32(1.0) / np.where(ok, area, f32(1.0))).astype(f32)

        def edge_coef(ux, uy, vx, vy):
            ex = (vx - ux).astype(f32)
            ey = (vy - uy).astype(f32)
            return np.stack([(-ey).astype(f32), ex,
                             (ey * ux - ex * uy).astype(f32)], 0)

        cw0 = edge_coef(bx, by, cx, cy)
        cw1 = edge_coef(cx, cy, ax, ay)
        cw2 = edge_coef(ax, ay, bx, by)
        cw0n = (cw0 * inv[None]).astype(f32)
        cw1n = (cw1 * inv[None]).astype(f32)
        cw2n = (cw2 * inv[None]).astype(f32)
        z = p3[:, :, 2]
        czi = (cw0n * z[:, 0][None] + cw1n * z[:, 1][None] + cw2n * z[:, 2][None]).astype(f32)

        uvf = uv[b][ft]
        feat = np.concatenate([
            np.tile(normal[:, None, :], (1, 3, 1)),
            -p3, uvf, np.ones((F, 3, 1), dtype=f32)], axis=-1).astype(f32)
        cw = np.stack([cw0n, cw1n, cw2n], 0)  # [3k,3j,F]
        G = np.einsum('kjf,fkd->fjd', cw.astype(np.float64),
                      feat.astype(np.float64)).astype(f32)  # [F,3,9]

        def seg_tables(ux, uy, vx, vy):
            ex = (vx - ux).astype(np.float64); ey = (vy - uy).astype(np.float64)
            uxd = ux.astype(np.float64); uyd = uy.astype(np.float64)
            E = ex * ex + ey * ey
            S = np.stack([np.zeros_like(ex), 2 * cc * ex, 2 * cc * ey,
                          2 * cc * (-ex * uxd - ey * uyd)], 0).astype(f32)
            Q = np.stack([np.full_like(ex, cc), -2 * cc * uxd, -2 * cc * uyd,
                          cc * (uxd * uxd + uyd * uyd)], 0).astype(f32)
            IE = (1.0 / (2 * cc * (E + 1e-12))).astype(f32)
            CE = (cc * E).astype(f32)
            return S, Q, IE, CE

        Sab, Qab, IEab, CEab = seg_tables(ax, ay, bx, by)
        Sbc, Qbc, IEbc, CEbc = seg_tables(bx, by, cx, cy)
        Sca, Qca, IEca, CEca = seg_tables(cx, cy, ax, ay)

        okm1 = np.where(ok, f32(0.0), NEG_BIG).astype(f32)
        okm2 = np.where(ok & (normalz > 0), f32(0.0), NEG_BIG).astype(f32)

        def padF(a, fill=0.0):
            shp = list(a.shape); shp[-1] = FP
            out = np.full(shp, fill, dtype=f32)
            out[..., :F] = a
            return out

        SQ = np.stack([padF(Sab), padF(Sbc), padF(Sca),
                       padF(Qab), padF(Qbc), padF(Qca)], 0)  # [6,4,FP]
        SQ[3:, 3, F:] = f32(1e30)  # pad faces: a = huge -> lg = 0
        coefs = SQ.reshape(6, 4, NT, 128).transpose(2, 0, 1, 3).copy()  # [NT,6,4,128]

        WC = np.stack([padF(cw0n[0]), padF(cw0n[1]), padF(cw0n[2]),
                       padF(cw1n[0]), padF(cw1n[1]), padF(cw1n[2]),
                       padF(cw2n[0]), padF(cw2n[1]), padF(cw2n[2]),
                       padF(czi[0]), padF(czi[1]), padF(czi[2])], 0)  # [12, FP]
        wcols = WC.reshape(12, NT, 128).transpose(2, 1, 0).reshape(128, NT * 12).copy()

        FC = np.stack([padF(IEab), padF(IEbc), padF(IEca),
                       padF(CEab), padF(CEbc), padF(CEca),
                       padF(okm1, float(NEG_BIG)), padF(okm2, float(NEG_BIG))], 0)
        fcols = FC.reshape(8, NT, 128).transpose(2, 1, 0).reshape(128, NT * 8).copy()

        Gp = padF(G.reshape(F, 27).T).T.reshape(FP, 3, 9)
        gtab = Gp.reshape(NT, 128, 27)
        gtab = np.concatenate([gtab, np.zeros((NT, 128, 9), dtype=f32)], -1)
        gtab = gtab.transpose(1, 0, 2).reshape(128, NT * 36).copy()

        misc = np.zeros((128, 16), dtype=f32)
        mat = material[b]
        ld = lightdirect[b]
        ln = np.sqrt((ld * ld).sum().astype(f32)).astype(f32)
        ldn = (ld / (ln + EPS)).astype(f32)
        misc[:, 0:3] = mat[0][None, :]
        misc[:, 3:6] = mat[1][None, :]
        misc[:, 6:9] = mat[2][None, :]
        misc[:, 9:12] = ldn[None, :]
        misc[:, 12] = shininess[b, 0]

        ti = np.zeros(TEXN, dtype=f32)
        ti[:TEX * TEX * 3] = texture[b].transpose(1, 2, 0).reshape(-1)

        per_batch.append(dict(coefs=coefs, wcols=wcols, fcols=fcols, gtab=gtab,
                              misc=misc, texi=ti.reshape(TEXN, 1)))

    PXg = np.broadcast_to(px[None, :], (H, W)).reshape(-1).astype(f32)
    PYg = np.broadcast_to(py[:, None], (H, W)).reshape(-1).astype(f32)
    Rg = (PXg * PXg + PYg * PYg).astype(f32)

    in_maps = []
    for core in range(NCORES):
        b = core // BANDS
        band = core % BANDS
        r0 = band * ROWS
        sl = slice(r0 * W, (r0 + ROWS) * W)
        basis = np.stack([Rg[sl], PXg[sl], PYg[sl], np.ones(NPX, dtype=f32)], 0).copy()
        pyexp = np.broadcast_to(py[r0:r0 + ROWS][None, :, None], (128, ROWS, 9)).reshape(128, ROWS * 9).copy()
        pxcol = px[:, None].copy()
        t = per_batch[b]
        in_maps.append({
            'coefs': t['coefs'], 'wcols': t['wcols'], 'fcols': t['fcols'],
            'gtab': t['gtab'], 'misc': t['misc'], 'texi': t['texi'],
            'basis': basis, 'pyexp': pyexp, 'pxcol': pxcol,
            'pxb': np.broadcast_to(PXg[sl][None, :], (128, NPX)).copy(),
            'pyb': np.broadcast_to(PYg[sl][None, :], (128, NPX)).copy(),
        })
    return in_maps, normal1_out


_NC_CACHE = {}


def _register_seg_a():
    """Register the fused soft-prob custom DVE op:
    out = t*(t*CE - S) + Q with t = clip01(S*IE)   (S=in0, Q=in1, IE=s0, CE=s1)."""
    import concourse.dve_ops as dve_ops
    if any(o.name == "SEG_A_ANT" for o in dve_ops.OPS):
        return tuple(next(o for o in dve_ops.OPS if o.name == n)
                     for n in ("SEG_A_ANT", "ZSEL_ANT", "ZSEL2_ANT", "MINA2_ANT", "PSEL_ANT"))
    from concourse.dve_spec import Spec, Src0, Src1, C0, C1, C2, relu, minn
    from concourse.dve_table_gen import dve_ver_for

    def ref_seg_a(in0, in1, s0, s1, imm2):
        t = np.minimum(np.maximum(in0.astype(np.float32) * s0, 0), imm2).astype(np.float32)
        return (t * (t * s1 - in0) + in1).astype(np.float32)

    t_expr = minn(relu(Src0 * C0), C2)
    spec = Spec(body=t_expr * (t_expr * C1 - Src0) + Src1, reference=ref_seg_a)
    row = dve_ops._CUSTOM_DVE_ROW_BASE + len(dve_ops.OPS)
    op = dve_ops.DveOp("SEG_A_ANT", spec, subdim=False,
                       uops_sha={"v3": "a415bb456e75ca33"})
    dve_ops.OPS.append(op)
    dve_ops._SUB_OPCODE_FOR_NAME[op.name] = row
    dve_ops.CUSTOM_DVE_SPECS[op.name] = spec

    def ref_zsel(in0, in1, s0, s1, imm2):
        return (in0.astype(np.float32) * in1 + (in0 - imm2) * s0).astype(np.float32)

    spec2 = Spec(body=Src0 * Src1 + (Src0 - C2) * C0, reference=ref_zsel)
    row2 = dve_ops._CUSTOM_DVE_ROW_BASE + len(dve_ops.OPS)
    op2 = dve_ops.DveOp("ZSEL_ANT", spec2, subdim=False,
                        uops_sha={"v3": "4c6249d316d2ba3d"})
    dve_ops.OPS.append(op2)
    dve_ops._SUB_OPCODE_FOR_NAME[op2.name] = row2
    dve_ops.CUSTOM_DVE_SPECS[op2.name] = spec2

    from concourse.dve_spec import Zero, select

    def ref_zsel2(in0, in1, s0, s1, imm2):
        return np.where((in0.astype(np.float32) + s0) >= 0,
                        (in1.astype(np.float32) + s1).astype(np.float32),
                        np.float32(imm2)).astype(np.float32)

    spec3 = Spec(body=select((Src0 + C0) >= Zero, Src1 + C1, C2), reference=ref_zsel2)
    row3 = dve_ops._CUSTOM_DVE_ROW_BASE + len(dve_ops.OPS)
    op3 = dve_ops.DveOp("ZSEL2_ANT", spec3, subdim=False,
                        uops_sha={"v3": "c08a5da8e56941a5"})
    dve_ops.OPS.append(op3)
    dve_ops._SUB_OPCODE_FOR_NAME[op3.name] = row3
    dve_ops.CUSTOM_DVE_SPECS[op3.name] = spec3

    from concourse.dve_spec import minn

    def ref_mina2(in0, in1, s0, s1, imm2):
        return np.minimum((in0.astype(np.float32) + s0).astype(np.float32),
                          (in1.astype(np.float32) + s1).astype(np.float32)).astype(np.float32)

    spec4 = Spec(body=minn(Src0 + C0, Src1 + C1), reference=ref_mina2)
    row4 = dve_ops._CUSTOM_DVE_ROW_BASE + len(dve_ops.OPS)
    op4 = dve_ops.DveOp("MINA2_ANT", spec4, subdim=False,
                        uops_sha={"v3": "c432cfb49e0ef018"})
    dve_ops.OPS.append(op4)
    dve_ops._SUB_OPCODE_FOR_NAME[op4.name] = row4
    dve_ops.CUSTOM_DVE_SPECS[op4.name] = spec4

    def ref_psel(in0, in1, s0, s1, imm2):
        return np.where((in0.astype(np.float32) + s0) >= 0, np.float32(imm2),
                        in1.astype(np.float32)).astype(np.float32)

    spec5 = Spec(body=select((Src0 + C0) >= Zero, C2, Src1), reference=ref_psel)
    row5 = dve_ops._CUSTOM_DVE_ROW_BASE + len(dve_ops.OPS)
    op5 = dve_ops.DveOp("PSEL_ANT", spec5, subdim=False,
                        uops_sha={"v3": "77b9264861ef90d8"})
    dve_ops.OPS.append(op5)
    dve_ops._SUB_OPCODE_FOR_NAME[op5.name] = row5
    dve_ops.CUSTOM_DVE_SPECS[op5.name] = spec5
    return op, op2, op3, op4, op5


def build_nc():
    import concourse.bacc as bacc
    import concourse.bass as bass
    import concourse.mybir as mybir
    import concourse.bass_isa as bass_isa
    from concourse.tile import TileContext

    dt = mybir.dt
    Alu = mybir.AluOpType
    Act = mybir.ActivationFunctionType

    seg_a_op, zsel_op, zsel2_op, mina2_op, psel_op = _register_seg_a()
    nc = bacc.Bacc(trn_type="TRN2")
    coefs_d = nc.dram_tensor("coefs", [NT, 6, 4, 128], dt.float32, kind="ExternalInput")
    wcols_d = nc.dram_tensor("wcols", [128, NT * 12], dt.float32, kind="ExternalInput")
    fcols_d = nc.dram_tensor("fcols", [128, NT * 8], dt.float32, kind="ExternalInput")
    gtab_d = nc.dram_tensor("gtab", [128, NT * 36], dt.float32, kind="ExternalInput")
    misc_d = nc.dram_tensor("misc", [128, 16], dt.float32, kind="ExternalInput")
    texi_d = nc.dram_tensor("texi", [TEXN, 1], dt.float32, kind="ExternalInput")
    basis_d = nc.dram_tensor("basis", [4, NPX], dt.float32, kind="ExternalInput")
    pxb_d = nc.dram_tensor("pxb", [128, NPX], dt.float32, kind="ExternalInput")
    pyb_d = nc.dram_tensor("pyb", [128, NPX], dt.float32, kind="ExternalInput")
    pyexp_d = nc.dram_tensor("pyexp", [128, ROWS * 9], dt.float32, kind="ExternalInput")
    pxcol_d = nc.dram_tensor("pxcol", [128, 1], dt.float32, kind="ExternalInput")
    imr_d = nc.dram_tensor("imr", [ROWS, W, 3], dt.float32, kind="ExternalOutput")
    imp_d = nc.dram_tensor("imp", [ROWS, W], dt.float32, kind="ExternalOutput")

    with TileContext(nc) as tc:
        with tc.tile_pool(name="cb", bufs=1) as cb, \
             tc.tile_pool(name="zb", bufs=1) as zbp, \
             tc.tile_pool(name="wk", bufs=2) as wk, \
             tc.tile_pool(name="ep", bufs=1) as ep, \
             tc.tile_pool(name="ps1", bufs=1, space="PSUM") as ps1, \
             tc.tile_pool(name="ps2", bufs=2, space="PSUM") as ps2, \
             tc.tile_pool(name="dr", bufs=1, space="DRAM") as dr:

            # ---- constants into SBUF ----
            coefs_sb = cb.tile([4, NT * 6 * 128], dt.float32, tag="coefs")
            nc.sync.dma_start(out=coefs_sb[:].rearrange("k (t m j) -> k t m j", t=NT, m=6),
                              in_=coefs_d[:].rearrange("t m k j -> k t m j"))
            wcols = cb.tile([128, NT * 12], dt.float32, tag="wcols")
            nc.sync.dma_start(out=wcols[:], in_=wcols_d[:])
            fcols = cb.tile([128, NT * 8], dt.float32, tag="fcols")
            nc.sync.dma_start(out=fcols[:], in_=fcols_d[:])
            gtab = cb.tile([128, NT * 36], dt.float32, tag="gtab")
            nc.sync.dma_start(out=gtab[:], in_=gtab_d[:])
            misc = cb.tile([128, 16], dt.float32, tag="misc")
            nc.sync.dma_start(out=misc[:], in_=misc_d[:])

            pyexp = cb.tile([128, ROWS * 9], dt.float32, tag="pyexp")
            nc.sync.dma_start(out=pyexp[:], in_=pyexp_d[:])
            pxcol = cb.tile([128, 1], dt.float32, tag="pxcol")
            nc.sync.dma_start(out=pxcol[:], in_=pxcol_d[:])
            ones = cb.tile([128, 1], dt.float32, tag="ones")
            nc.vector.memset(ones[:], 1.0)

            imfeat = ep.tile([128, ROWS * 36], dt.float32, tag="imfeat")
            covT = ep.tile([128, ROWS], dt.float32, tag="covT")
            covd = dr.tile([ROWS, W], dt.float32, tag="covd")
            gathA = ep.tile([128, ROWS * 2 * 6], dt.float32, tag="gathA")
            uA = ep.tile([128, ROWS], dt.float32, tag="uA")
            vA = ep.tile([128, ROWS], dt.float32, tag="vA")

            def fcol(t, j):
                return fcols[:, t * 8 + j:t * 8 + j + 1]

            def wcol(t, j):
                return wcols[:, t * 12 + j:t * 12 + j + 1]

            # ---------------- main per-chunk loop ----------------
            for ch in range(NCHUNK):
                pxs = slice(ch * CH, (ch + 1) * CH)
                bas = wk.tile([4, CH], dt.float32, tag="bas")
                nc.sync.dma_start(out=bas[:], in_=basis_d[:, pxs])
                pxbc = wk.tile([128, CH], dt.float32, tag="pxbc")
                nc.sync.dma_start(out=pxbc[:], in_=pxb_d[:, pxs])
                pybc = wk.tile([128, CH], dt.float32, tag="pybc")
                nc.sync.dma_start(out=pybc[:], in_=pyb_d[:, pxs])

                zts = []
                sg = ps1.tile([1, CH], dt.float32, tag="sg", space="PSUM")
                in_tiles = []
                pc_tiles = []
                for t in range(NT):
                    # ---- exact w/zi maps: s_m = PX*c0 + PY*c1 (ACT fma + STT) ----
                    ss = []
                    for m in range(4):
                        u = wk.tile([128, CH], dt.float32, tag=f"u{m}", name=f"u{m}")
                        nc.scalar.activation(u[:], pxbc[:], Act.Copy,
                                             scale=wcol(t, m * 3 + 0))
                        nc.vector.scalar_tensor_tensor(
                            out=u[:], in0=pybc[:], scalar=wcol(t, m * 3 + 1),
                            in1=u[:], op0=Alu.mult, op1=Alu.add)
                        ss.append(u)
                    m1 = wk.tile([128, CH], dt.float32, tag="m1")
                    nc.vector._custom_dve(mina2_op, out=m1[:], in0=ss[0][:], in1=ss[1][:],
                                          s0=wcol(t, 0 * 3 + 2), s1=wcol(t, 1 * 3 + 2),
                                          imm2=0.0)
                    m3 = wk.tile([128, CH], dt.float32, tag="m3")
                    nc.vector.scalar_tensor_tensor(
                        out=m3[:], in0=ss[2][:], scalar=wcol(t, 2 * 3 + 2),
                        in1=m1[:], op0=Alu.add, op1=Alu.min)

                    zb = zbp.tile([128, CH], dt.float32, tag=f"z{t}")
                    nc.vector._custom_dve(zsel2_op, out=zb[:], in0=m3[:], in1=ss[3][:],
                                          s0=fcol(t, 7), s1=wcol(t, 3 * 3 + 2),
                                          imm2=float(ZFILL))
                    zts.append(zb)

                    # ---- soft-prob path (PE maps) ----
                    a_list = []
                    for e in range(3):
                        Sp = ps2.tile([128, CH], dt.float32, tag="qS", space="PSUM")
                        nc.tensor.matmul(
                            Sp[:], coefs_sb[:, (t * 6 + e) * 128:(t * 6 + e + 1) * 128],
                            bas[:], start=True, stop=True)
                        Qp = ps2.tile([128, CH], dt.float32, tag="qQ", space="PSUM")
                        nc.tensor.matmul(
                            Qp[:], coefs_sb[:, (t * 6 + e + 3) * 128:(t * 6 + e + 4) * 128],
                            bas[:], start=True, stop=True)
                        Qs = wk.tile([128, CH], dt.float32, tag="Qs")
                        nc.scalar.activation(Qs[:], Qp[:], Act.Copy)
                        a = wk.tile([128, CH], dt.float32, tag=f"a{e}")
                        nc.vector._custom_dve(seg_a_op, out=a[:], in0=Sp[:], in1=Qs[:],
                                              s0=fcol(t, e), s1=fcol(t, 3 + e), imm2=1.0)
                        a_list.append(a)
                    am = wk.tile([128, CH], dt.float32, tag="am")
                    nc.vector.tensor_tensor(out=am[:], in0=a_list[0][:], in1=a_list[1][:], op=Alu.min)
                    am2 = wk.tile([128, CH], dt.float32, tag="am2")
                    nc.vector.tensor_tensor(out=am2[:], in0=am[:], in1=a_list[2][:], op=Alu.min)
                    pp = wk.tile([128, CH], dt.float32, tag="pp")
                    nc.scalar.activation(pp[:], am2[:], Act.Exp, scale=-1.0)
                    pcr = wk.tile([128, CH], dt.float32, tag="pcr")
                    nc.vector.tensor_scalar(out=pcr[:], in0=pp[:], scalar1=PMAX,
                                            scalar2=None, op0=Alu.min)
                    pct = zbp.tile([128, CH], dt.float32, tag=f"pc{t}", name=f"pc{t}")
                    nc.vector._custom_dve(psel_op, out=pct[:], in0=m3[:], in1=pcr[:],
                                          s0=fcol(t, 6), s1=0.0, imm2=PMAX)
                    pc_tiles.append(pct)

                # ---- batched Ln pass (one ACT table switch per chunk) ----
                for t in range(NT):
                    lg = wk.tile([128, CH], dt.float32, tag="lg")
                    nc.scalar.activation(lg[:], pc_tiles[t][:], Act.Ln, scale=-1.0, bias=1.0)
                    nc.tensor.matmul(sg[:], ones[:], lg[:], start=(t == 0),
                                     stop=(t == NT - 1), skip_group_check=True)

                # ---- zmax + sum(lg) partition reductions ----
                zmt = wk.tile([128, CH], dt.float32, tag="zmt")
                nc.vector.tensor_tensor(out=zmt[:], in0=zts[0][:], in1=zts[1][:], op=Alu.max)
                for t in range(2, NT):
                    nc.vector.tensor_tensor(out=zmt[:], in0=zmt[:], in1=zts[t][:], op=Alu.max)
                zmaxb = wk.tile([128, CH], dt.float32, tag="zmaxb")
                nc.gpsimd.partition_all_reduce(zmaxb[:], zmt[:], 128, bass_isa.ReduceOp.max)

                # ---- one-hot winner + feature matmul (accumulate over tiles) ----
                gall = ps1.tile([128, RCH * 36], dt.float32, tag="gall", space="PSUM")
                ohs = []
                for t in range(NT):
                    oh = zbp.tile([128, CH], dt.float32, tag=f"oh{t}", name=f"oh{t}")
                    nc.vector.tensor_tensor(out=oh[:], in0=zts[t][:], in1=zmaxb[:], op=Alu.is_equal)
                    ohs.append(oh)
                for blk in range(RCH):
                    for t in range(NT):
                        nc.tensor.matmul(gall[:, blk * 36:(blk + 1) * 36],
                                         ohs[t][:, blk * 128:(blk + 1) * 128],
                                         gtab[:, t * 36:(t + 1) * 36],
                                         start=(t == 0), stop=(t == NT - 1),
                                         skip_group_check=True)
                nc.scalar.activation(
                    imfeat[:, ch * RCH * 36:(ch + 1) * RCH * 36], gall[:], Act.Copy)

                # --- texcoords + texture gathers for this chunk's 4 rows ---
                ut = wk.tile([128, RCH], dt.float32, tag="ut")
                vt = wk.tile([128, RCH], dt.float32, tag="vt")
                tt1 = wk.tile([128, RCH], dt.float32, tag="tt1")
                idf = wk.tile([128, RCH * 2], dt.float32, tag="idf")
                idi = wk.tile([128, RCH * 2], dt.int32, tag="idi")
                imfc = imfeat[:, ch * RCH * 36:(ch + 1) * RCH * 36].rearrange(
                    "p (r j d) -> p r j d", j=4, d=9)
                for j, dst in ((6, ut), (7, vt)):
                    nc.vector.tensor_scalar(out=tt1[:], in0=imfc[:, :, 0, j],
                                            scalar1=pxcol[:, 0:1], scalar2=None, op0=Alu.mult)
                    nc.vector.tensor_tensor(out=dst[:], in0=imfc[:, :, 1, j],
                                            in1=pyexp[:].rearrange("p (r d) -> p r d", d=9)[
                                                :, ch * RCH:(ch + 1) * RCH, 0],
                                            op=Alu.mult)
                    nc.vector.tensor_tensor(out=dst[:], in0=dst[:], in1=tt1[:], op=Alu.add)
                    nc.vector.tensor_tensor(out=dst[:], in0=dst[:], in1=imfc[:, :, 2, j], op=Alu.add)
                # u = clip(u,0,1)*511 ; v = (1-clip(v,0,1))*511
                nc.vector.tensor_scalar(out=ut[:], in0=ut[:], scalar1=0.0, scalar2=1.0,
                                        op0=Alu.max, op1=Alu.min)
                nc.vector.tensor_scalar(out=ut[:], in0=ut[:], scalar1=float(TEX - 1),
                                        scalar2=None, op0=Alu.mult)
                nc.vector.tensor_scalar(out=vt[:], in0=vt[:], scalar1=0.0, scalar2=1.0,
                                        op0=Alu.max, op1=Alu.min)
                nc.vector.tensor_scalar(out=vt[:], in0=vt[:], scalar1=-1.0, scalar2=1.0,
                                        op0=Alu.mult, op1=Alu.add)
                nc.vector.tensor_scalar(out=vt[:], in0=vt[:], scalar1=float(TEX - 1),
                                        scalar2=None, op0=Alu.mult)
                nc.vector.tensor_copy(uA[:, ch * RCH:(ch + 1) * RCH], ut[:])
                nc.vector.tensor_copy(vA[:, ch * RCH:(ch + 1) * RCH], vt[:])
                # floor (exact, rounding-agnostic)
                ci = wk.tile([128, RCH], dt.int32, tag="ci")
                fx = wk.tile([128, RCH], dt.float32, tag="fx")
                x0c = wk.tile([128, RCH], dt.float32, tag="x0c")
                y0c = wk.tile([128, RCH], dt.float32, tag="y0c")
                for srcv, dstv in ((ut, x0c), (vt, y0c)):
                    nc.vector.tensor_copy(ci[:], srcv[:])
                    nc.vector.tensor_copy(dstv[:], ci[:])
                    nc.vector.tensor_tensor(out=fx[:], in0=dstv[:], in1=srcv[:], op=Alu.is_gt)
                    nc.vector.tensor_tensor(out=dstv[:], in0=dstv[:], in1=fx[:], op=Alu.subtract)
                y1c = wk.tile([128, RCH], dt.float32, tag="y1c")
                nc.vector.tensor_scalar(out=y1c[:], in0=y0c[:], scalar1=1.0,
                                        scalar2=float(TEX - 1), op0=Alu.add, op1=Alu.min)
                idv = idf[:].rearrange("p (r d) -> p r d", d=2)
                nc.vector.scalar_tensor_tensor(out=idv[:, :, 0], in0=y0c[:], scalar=float(TEX),
                                               in1=x0c[:], op0=Alu.mult, op1=Alu.add)
                nc.vector.scalar_tensor_tensor(out=idv[:, :, 1], in0=y1c[:], scalar=float(TEX),
                                               in1=x0c[:], op0=Alu.mult, op1=Alu.add)
                nc.vector.tensor_scalar(out=idf[:], in0=idf[:], scalar1=3.0, scalar2=None,
                                        op0=Alu.mult)
                nc.vector.tensor_copy(idi[:], idf[:])
                for r in range(RCH):
                    for wch in range(2):
                        gi = (ch * RCH + r) * 2 + wch
                        nc.gpsimd.indirect_dma_start(
                            out=gathA[:, gi * 6:(gi + 1) * 6], out_offset=None,
                            in_=texi_d[:],
                            in_offset=bass.IndirectOffsetOnAxis(
                                ap=idi[:, r * 2 + wch:r * 2 + wch + 1], axis=0))

                # ---- coverage + improb for this chunk ----
                cr = ep.tile([1, CH], dt.float32, tag="cr")
                nc.vector.tensor_scalar(out=cr[:], in0=zmaxb[0:1, :], scalar1=float(ZFILL),
                                        scalar2=None, op0=Alu.is_gt)
                nc.gpsimd.dma_start(
                    out=covd[ch * RCH:(ch + 1) * RCH, :], in_=cr[:])
                exr = ep.tile([1, CH], dt.float32, tag="exr")
                nc.scalar.activation(exr[:], sg[:], Act.Exp)
                impc = ep.tile([1, CH], dt.float32, tag="impc")
                nc.vector.tensor_scalar(out=impc[:], in0=exr[:], scalar1=-1.0,
                                        scalar2=1.0, op0=Alu.mult, op1=Alu.add)
                nc.sync.dma_start(
                    out=imp_d[ch * RCH:(ch + 1) * RCH, :].rearrange("r c -> r c")[None, :, :],
                    in_=impc[:].rearrange("a (r c) -> a r c", c=W))

            # ---------------- core epilogue ([col, row] layout) ----------------
            NR = ROWS
            nc.gpsimd.dma_start(out=covT[:], in_=covd[:].rearrange("r c -> c r"))
            i3 = imfeat[:].rearrange("p (r j d) -> p r j d", j=4, d=9)
            imf9 = ep.tile([128, NR * 9], dt.float32, tag="imf9")
            v9 = imf9[:].rearrange("p (r d) -> p r d", d=9)
            tmp9 = ep.tile([128, NR * 9], dt.float32, tag="tmp9")
            t9 = tmp9[:].rearrange("p (r d) -> p r d", d=9)
            nc.vector.tensor_scalar(out=t9[:, :, :], in0=i3[:, :, 0, :],
                                    scalar1=pxcol[:, 0:1], scalar2=None, op0=Alu.mult)
            nc.vector.tensor_tensor(out=v9[:, :, :], in0=i3[:, :, 1, :],
                                    in1=pyexp[:].rearrange("p (r d) -> p r d", d=9),
                                    op=Alu.mult)
            nc.vector.tensor_tensor(out=v9[:, :, :], in0=v9[:, :, :], in1=t9[:, :, :], op=Alu.add)
            nc.vector.tensor_tensor(out=v9[:, :, :], in0=v9[:, :, :], in1=i3[:, :, 2, :], op=Alu.add)

            def vsl(j, k=1):
                return imf9[:].rearrange("p (r d) -> p r d", d=9)[:, :, j:j + k]

            nc.vector.tensor_tensor(out=vsl(8)[:, :, 0], in0=vsl(8)[:, :, 0],
                                    in1=covT[:], op=Alu.mult)

            def normalize3(dst_tag, j0):
                sq = ep.tile([128, NR * 3], dt.float32, tag=dst_tag + "sq")
                s3 = sq[:].rearrange("p (r d) -> p r d", d=3)
                nc.vector.tensor_tensor(out=s3[:], in0=vsl(j0, 3)[:, :, :],
                                        in1=vsl(j0, 3)[:, :, :], op=Alu.mult)
                n2 = ep.tile([128, NR], dt.float32, tag=dst_tag + "n2")
                nc.vector.tensor_reduce(out=n2[:], in_=s3[:], axis=mybir.AxisListType.X,
                                        op=Alu.add)
                sn = ep.tile([128, NR], dt.float32, tag=dst_tag + "sn")
                nc.scalar.activation(sn[:], n2[:], Act.Sqrt)
                nc.vector.tensor_scalar(out=sn[:], in0=sn[:], scalar1=float(EPS),
                                        scalar2=None, op0=Alu.add)
                rc = ep.tile([128, NR], dt.float32, tag=dst_tag + "rc")
                nc.vector.reciprocal(rc[:], sn[:])
                out = ep.tile([128, NR * 3], dt.float32, tag=dst_tag)
                o3 = out[:].rearrange("p (r d) -> p r d", d=3)
                for k in range(3):
                    nc.vector.tensor_tensor(out=o3[:, :, k], in0=vsl(j0 + k)[:, :, 0],
                                            in1=rc[:], op=Alu.mult)
                return out, o3

            nrm, nrm3 = normalize3("nrm", 0)
            eye, eye3 = normalize3("eye", 3)

            def mcol(j):
                return misc[:, j:j + 1]

            cosT = ep.tile([128, NR], dt.float32, tag="cosT")
            nc.vector.tensor_scalar(out=cosT[:], in0=nrm3[:, :, 0], scalar1=mcol(9),
                                    scalar2=None, op0=Alu.mult)
            nc.vector.scalar_tensor_tensor(out=cosT[:], in0=nrm3[:, :, 1], scalar=mcol(10),
                                           in1=cosT[:], op0=Alu.mult, op1=Alu.add)
            nc.vector.scalar_tensor_tensor(out=cosT[:], in0=nrm3[:, :, 2], scalar=mcol(11),
                                           in1=cosT[:], op0=Alu.mult, op1=Alu.add)
            nc.vector.tensor_scalar(out=cosT[:], in0=cosT[:], scalar1=0.0, scalar2=1.0,
                                    op0=Alu.max, op1=Alu.min)
            cosA = ep.tile([128, NR], dt.float32, tag="cosA")
            rk = ep.tile([128, NR], dt.float32, tag="rk")
            for k in range(3):
                nc.vector.tensor_tensor(out=rk[:], in0=cosT[:], in1=nrm3[:, :, k], op=Alu.mult)
                nc.vector.tensor_scalar(out=rk[:], in0=rk[:], scalar1=2.0, scalar2=None,
                                        op0=Alu.mult)
                nc.vector.tensor_scalar(out=rk[:], in0=rk[:], scalar1=mcol(9 + k),
                                        scalar2=None, op0=Alu.subtract)
                nc.vector.tensor_tensor(out=rk[:], in0=rk[:], in1=eye3[:, :, k], op=Alu.mult)
                if k == 0:
                    nc.vector.tensor_copy(cosA[:], rk[:])
                else:
                    nc.vector.tensor_tensor(out=cosA[:], in0=cosA[:], in1=rk[:], op=Alu.add)
            nc.vector.tensor_scalar(out=cosA[:], in0=cosA[:], scalar1=1e-5, scalar2=1.0,
                                    op0=Alu.max, op1=Alu.min)
            nc.scalar.activation(cosA[:], cosA[:], Act.Ln)
            nc.scalar.activation(cosA[:], cosA[:], Act.Exp, scale=mcol(12))

            # texture lookup (u/v + gathers were produced per-chunk)
            uu = uA
            vv = vA
            # exact floor: cast to int and back, then subtract 1 where it rounded up
            cint = ep.tile([128, NR], dt.int32, tag="cint")
            fixt = ep.tile([128, NR], dt.float32, tag="fixt")

            def floorf(dst_tag, src):
                nc.vector.tensor_copy(cint[:], src[:])
                dst = ep.tile([128, NR], dt.float32, tag=dst_tag, name=dst_tag)
                nc.vector.tensor_copy(dst[:], cint[:])
                nc.vector.tensor_tensor(out=fixt[:], in0=dst[:], in1=src[:], op=Alu.is_gt)
                nc.vector.tensor_tensor(out=dst[:], in0=dst[:], in1=fixt[:], op=Alu.subtract)
                return dst

            x0f = floorf("x0f", uu)
            y0f = floorf("y0f", vv)
            wx = ep.tile([128, NR], dt.float32, tag="wx")
            nc.vector.tensor_tensor(out=wx[:], in0=uu[:], in1=x0f[:], op=Alu.subtract)
            wy = ep.tile([128, NR], dt.float32, tag="wy")
            nc.vector.tensor_tensor(out=wy[:], in0=vv[:], in1=y0f[:], op=Alu.subtract)
            gv = gathA[:].rearrange("p (r w d) -> p r w d", w=2, d=6)

            wxc = ep.tile([128, NR], dt.float32, tag="wxc")
            nc.vector.tensor_scalar(out=wxc[:], in0=wx[:], scalar1=-1.0, scalar2=1.0,
                                    op0=Alu.mult, op1=Alu.add)
            wyc = ep.tile([128, NR], dt.float32, tag="wyc")
            nc.vector.tensor_scalar(out=wyc[:], in0=wy[:], scalar1=-1.0, scalar2=1.0,
                                    op0=Alu.mult, op1=Alu.add)
            w00 = ep.tile([128, NR], dt.float32, tag="w00")
            nc.vector.tensor_tensor(out=w00[:], in0=wxc[:], in1=wyc[:], op=Alu.mult)
            w01 = ep.tile([128, NR], dt.float32, tag="w01")
            nc.vector.tensor_tensor(out=w01[:], in0=wx[:], in1=wyc[:], op=Alu.mult)
            w10 = ep.tile([128, NR], dt.float32, tag="w10")
            nc.vector.tensor_tensor(out=w10[:], in0=wxc[:], in1=wy[:], op=Alu.mult)
            w11 = ep.tile([128, NR], dt.float32, tag="w11")
            nc.vector.tensor_tensor(out=w11[:], in0=wx[:], in1=wy[:], op=Alu.mult)

            colorT = ep.tile([128, NR * 3], dt.float32, tag="colorT")
            c3v = colorT[:].rearrange("p (r d) -> p r d", d=3)
            tcc = ep.tile([128, NR], dt.float32, tag="tcc")
            acc = ep.tile([128, NR], dt.float32, tag="acc")
            mm = ep.tile([128, NR], dt.float32, tag="mm")
            for c in range(3):
                nc.vector.tensor_tensor(out=tcc[:], in0=gv[:, :, 0, c], in1=w00[:], op=Alu.mult)
                nc.vector.tensor_tensor(out=mm[:], in0=gv[:, :, 0, 3 + c], in1=w01[:], op=Alu.mult)
                nc.vector.tensor_tensor(out=tcc[:], in0=tcc[:], in1=mm[:], op=Alu.add)
                nc.vector.tensor_tensor(out=mm[:], in0=gv[:, :, 1, c], in1=w10[:], op=Alu.mult)
                nc.vector.tensor_tensor(out=tcc[:], in0=tcc[:], in1=mm[:], op=Alu.add)
                nc.vector.tensor_tensor(out=mm[:], in0=gv[:, :, 1, 3 + c], in1=w11[:], op=Alu.mult)
                nc.vector.tensor_tensor(out=tcc[:], in0=tcc[:], in1=mm[:], op=Alu.add)
                nc.vector.tensor_tensor(out=mm[:], in0=cosT[:], in1=tcc[:], op=Alu.mult)
                nc.vector.tensor_scalar(out=acc[:], in0=tcc[:], scalar1=mcol(c),
                                        scalar2=None, op0=Alu.mult)
                nc.vector.scalar_tensor_tensor(out=acc[:], in0=mm[:], scalar=mcol(3 + c),
                                               in1=acc[:], op0=Alu.mult, op1=Alu.add)
                nc.vector.scalar_tensor_tensor(out=acc[:], in0=cosA[:], scalar=mcol(6 + c),
                                               in1=acc[:], op0=Alu.mult, op1=Alu.add)
                nc.vector.tensor_tensor(out=acc[:], in0=acc[:], in1=vsl(8)[:, :, 0], op=Alu.mult)
                nc.vector.tensor_scalar(out=c3v[:, :, c], in0=acc[:], scalar1=0.0, scalar2=1.0,
                                        op0=Alu.max, op1=Alu.min)

            nc.sync.dma_start(out=imr_d[:].rearrange("r c k -> c r k"),
                              in_=colorT[:].rearrange("p (r k) -> p r k", k=3))

    nc.finalize()
    return nc


def kernel(points, faces, camera_rot, camera_pos, camera_proj, uv, ft, texture,
           lightdirect, material, shininess, height, width):
    from concourse.bass_utils import run_bass_kernel_spmd

    inputs = dict(points=points, faces=faces, camera_rot=camera_rot,
                  camera_pos=camera_pos, camera_proj=camera_proj, uv=uv, ft=ft,
                  texture=texture, lightdirect=lightdirect, material=material,
                  shininess=shininess)
    in_maps, normal1 = host_prep(inputs)

    if 'nc' not in _NC_CACHE:
        _NC_CACHE['nc'] = build_nc()
    nc = _NC_CACHE['nc']

    res = run_bass_kernel_spmd(nc, in_maps, core_ids=list(range(NCORES)))

    imrender = np.zeros((B, H, W, 3), dtype=f32)
    improb = np.zeros((B, H, W, 1), dtype=f32)
    for core in range(NCORES):
        b = core // BANDS
        r0 = (core % BANDS) * ROWS
        imrender[b, r0:r0 + ROWS] = res.results[core]['imr']
        improb[b, r0:r0 + ROWS, :, 0] = res.results[core]['imp']
    return imrender, improb, normal1


# revision 20
# speedup vs baseline: 1.1757x; 1.0390x over previous
"""Trainium2 Bass kernel for nn_PhongRender (DIB-R style Phong renderer).

kernel(**inputs) takes FULL unsharded inputs (as from setup_inputs()) and
returns (imrender [2,128,128,3], improb [2,128,128,1], normal1 [2,800,3]).

Sharding: 8 cores = 2 batches x 4 row-bands of 32 image rows each.
Each core rasterizes its 32x128 pixel band against all 800 faces.

Per-core device pipeline (f-layout [128 faces, 512 px] tiles):
  - w0n/w1n/w2n/zi maps: exact elementwise evaluation (ACT fma + DVE STT/TS)
    with the same f32 rounding sequence as the reference-validated host sim
    (hard decisions: inside tests, z-buffer winner).
  - S/Q soft-silhouette maps: PE matmul (K=4 affine basis), smooth path.
  - zmax / sum(log) partition reductions: GPSIMD all-reduce (exact f32).
  - winner one-hot (zbuf == zmax) -> feature interpolation via PE matmul
    against a precomputed per-face G table.
  - fragment shader + bilinear texture (indirect DMA gather) on-device.
"""
import numpy as np

f32 = np.float32

B, P, F, H, W, TEX = 2, 500, 800, 128, 128, 512
NCORES = 8
BANDS = 4                 # row-bands per batch
ROWS = H // BANDS         # 32 rows per core
NPX = ROWS * W            # 4096 pixels per core
CH = 512                  # pixels per chunk (4 image rows)
NCHUNK = NPX // CH        # 8
RCH = CH // W             # rows per chunk = 4
NT = (F + 127) // 128     # 7 face tiles
FP = NT * 128             # 896 padded faces
EPS = f32(1e-15)
MULT = 1000.0
DELTA = 7000.0
NEG_BIG = f32(-1e30)
ZFILL = f32(-1e10)
LOG1EM7 = float(np.log(np.float32(1e-7)))
PMAX = float(np.float32(1.0) - np.float32(1e-7))
TEXN = TEX * TEX * 3 + 16


def _fma(a, b, c):
    """f32 fused multiply-add (matches XLA CPU's fma contraction)."""
    return (a.astype(np.float64) * b.astype(np.float64) + c.astype(np.float64)).astype(f32)


def _cross_fma(a, b):
    """cross product with XLA-CPU's fma pattern: fma(a1,b2, -(a2*b1))."""
    return np.stack([
        _fma(a[:, 1], b[:, 2], -(a[:, 2] * b[:, 1]).astype(f32)),
        _fma(a[:, 2], b[:, 0], -(a[:, 0] * b[:, 2]).astype(f32)),
        _fma(a[:, 0], b[:, 1], -(a[:, 1] * b[:, 0]).astype(f32)),
    ], -1)


def host_prep(inputs):
    """Build per-core input tensor maps + host-computed normal1."""
    points = np.asarray(inputs['points'], dtype=f32)
    faces = np.asarray(inputs['faces'])
    camera_rot = np.asarray(inputs['camera_rot'], dtype=f32)
    camera_pos = np.asarray(inputs['camera_pos'], dtype=f32)
    camera_proj = np.asarray(inputs['camera_proj'], dtype=f32).reshape(3)
    uv = np.asarray(inputs['uv'], dtype=f32)
    ft = np.asarray(inputs['ft'])
    texture = np.asarray(inputs['texture'], dtype=f32)
    lightdirect = np.asarray(inputs['lightdirect'], dtype=f32)
    material = np.asarray(inputs['material'], dtype=f32)
    shininess = np.asarray(inputs['shininess'], dtype=f32)

    px = ((f32(2.0) * (np.arange(W, dtype=f32) + f32(0.5)) - f32(W)) / f32(W)).astype(f32)
    py = ((f32(H) - f32(2.0) * (np.arange(H, dtype=f32) + f32(0.5))) / f32(H)).astype(f32)

    cc = np.float64(MULT) ** 2 / np.float64(DELTA)

    normal1_out = np.zeros((B, F, 3), dtype=f32)
    per_batch = []
    for b in range(B):
        pts = points[b]
        pos = camera_pos[b]
        rot = camera_rot[b]
        pc = np.einsum('pj,ij->pi', (pts - pos[None, :]).astype(f32), rot).astype(f32)
        xy = (pc * camera_proj[None, :]).astype(f32)
        xy2 = (xy[:, :2] / xy[:, 2:3]).astype(f32)
        p3 = pc[faces]
        p2 = xy2[faces]
        v10 = (p3[:, 1] - p3[:, 0]).astype(f32)
        v20 = (p3[:, 2] - p3[:, 0]).astype(f32)
        normal = _cross_fma(v10, v20)
        normalz = normal[:, 2]
        nn = np.sqrt((normal * normal).sum(-1, keepdims=True).astype(f32)).astype(f32)
        normal1_out[b] = (normal / (nn + EPS)).astype(f32)

        ax, ay = p2[:, 0, 0], p2[:, 0, 1]
        bx, by = p2[:, 1, 0], p2[:, 1, 1]
        cx, cy = p2[:, 2, 0], p2[:, 2, 1]
        area = ((bx - ax) * (cy - ay) - (by - ay) * (cx - ax)).astype(f32)
        ok = np.abs(area) > f32(1e-10)
        inv = (f32(1.0) / np.where(ok, area, f32(1.0))).astype(f32)

        def edge_coef(ux, uy, vx, vy):
            ex = (vx - ux).astype(f32)
            ey = (vy - uy).astype(f32)
            return np.stack([(-ey).astype(f32), ex,
                             (ey * ux - ex * uy).astype(f32)], 0)

        cw0 = edge_coef(bx, by, cx, cy)
        cw1 = edge_coef(cx, cy, ax, ay)
        cw2 = edge_coef(ax, ay, bx, by)
        cw0n = (cw0 * inv[None]).astype(f32)
        cw1n = (cw1 * inv[None]).astype(f32)
        cw2n = (cw2 * inv[None]).astype(f32)
        z = p3[:, :, 2]
        czi = (cw0n * z[:, 0][None] + cw1n * z[:, 1][None] + cw2n * z[:, 2][None]).astype(f32)

        uvf = uv[b][ft]
        feat = np.concatenate([
            np.tile(normal[:, None, :], (1, 3, 1)),
            -p3, uvf, np.ones((F, 3, 1), dtype=f32)], axis=-1).astype(f32)
        cw = np.stack([cw0n, cw1n, cw2n], 0)  # [3k,3j,F]
        G = np.einsum('kjf,fkd->fjd', cw.astype(np.float64),
                      feat.astype(np.float64)).astype(f32)  # [F,3,9]

        def seg_tables(ux, uy, vx, vy):
            ex = (vx - ux).astype(np.float64); ey = (vy - uy).astype(np.float64)
            uxd = ux.astype(np.float64); uyd = uy.astype(np.float64)
            E = ex * ex + ey * ey
            S = np.stack([np.zeros_like(ex), 2 * cc * ex, 2 * cc * ey,
                          2 * cc * (-ex * uxd - ey * uyd)], 0).astype(f32)
            Q = np.stack([np.full_like(ex, cc), -2 * cc * uxd, -2 * cc * uyd,
                          cc * (uxd * uxd + uyd * uyd)], 0).astype(f32)
            IE = (1.0 / (2 * cc * (E + 1e-12))).astype(f32)
            CE = (cc * E).astype(f32)
            return S, Q, IE, CE

        Sab, Qab, IEab, CEab = seg_tables(ax, ay, bx, by)
        Sbc, Qbc, IEbc, CEbc = seg_tables(bx, by, cx, cy)
        Sca, Qca, IEca, CEca = seg_tables(cx, cy, ax, ay)

        okm1 = np.where(ok, f32(0.0), NEG_BIG).astype(f32)
        okm2 = np.where(ok & (normalz > 0), f32(0.0), NEG_BIG).astype(f32)

        def padF(a, fill=0.0):
            shp = list(a.shape); shp[-1] = FP
            out = np.full(shp, fill, dtype=f32)
            out[..., :F] = a
            return out

        SQ = np.stack([padF(Sab), padF(Sbc), padF(Sca),
                       padF(Qab), padF(Qbc), padF(Qca)], 0)  # [6,4,FP]
        SQ[3:, 3, F:] = f32(1e30)  # pad faces: a = huge -> lg = 0
        coefs = SQ.reshape(6, 4, NT, 128).transpose(2, 0, 1, 3).copy()  # [NT,6,4,128]

        WC = np.stack([padF(cw0n[0]), padF(cw0n[1]), padF(cw0n[2]),
                       padF(cw1n[0]), padF(cw1n[1]), padF(cw1n[2]),
                       padF(cw2n[0]), padF(cw2n[1]), padF(cw2n[2]),
                       padF(czi[0]), padF(czi[1]), padF(czi[2])], 0)  # [12, FP]
        wcols = WC.reshape(12, NT, 128).transpose(2, 1, 0).reshape(128, NT * 12).copy()

        FC = np.stack([padF(IEab), padF(IEbc), padF(IEca),
                       padF(CEab), padF(CEbc), padF(CEca),
                       padF(okm1, float(NEG_BIG)), padF(okm2, float(NEG_BIG))], 0)
        fcols = FC.reshape(8, NT, 128).transpose(2, 1, 0).reshape(128, NT * 8).copy()

        Gp = padF(G.reshape(F, 27).T).T.reshape(FP, 3, 9)
        gtab = Gp.reshape(NT, 128, 27)
        gtab = np.concatenate([gtab, np.zeros((NT, 128, 9), dtype=f32)], -1)
        gtab = gtab.transpose(1, 0, 2).reshape(128, NT * 36).copy()

        misc = np.zeros((128, 16), dtype=f32)
        mat = material[b]
        ld = lightdirect[b]
        ln = np.sqrt((ld * ld).sum().astype(f32)).astype(f32)
        ldn = (ld / (ln + EPS)).astype(f32)
        misc[:, 0:3] = mat[0][None, :]
        misc[:, 3:6] = mat[1][None, :]
        misc[:, 6:9] = mat[2][None, :]
        misc[:, 9:12] = ldn[None, :]
        misc[:, 12] = shininess[b, 0]

        ti = np.zeros(TEXN, dtype=f32)
        ti[:TEX * TEX * 3] = texture[b].transpose(1, 2, 0).reshape(-1)

        per_batch.append(dict(coefs=coefs, wcols=wcols, fcols=fcols, gtab=gtab,
                              misc=misc, texi=ti.reshape(TEXN, 1)))

    PXg = np.broadcast_to(px[None, :], (H, W)).reshape(-1).astype(f32)
    PYg = np.broadcast_to(py[:, None], (H, W)).reshape(-1).astype(f32)
    Rg = (PXg * PXg + PYg * PYg).astype(f32)

    in_maps = []
    for core in range(NCORES):
        b = core // BANDS
        band = core % BANDS
        r0 = band * ROWS
        sl = slice(r0 * W, (r0 + ROWS) * W)
        basis = np.stack([Rg[sl], PXg[sl], PYg[sl], np.ones(NPX, dtype=f32)], 0).copy()
        pyexp = np.broadcast_to(py[r0:r0 + ROWS][None, :, None], (128, ROWS, 9)).reshape(128, ROWS * 9).copy()
        pxcol = px[:, None].copy()
        t = per_batch[b]
        in_maps.append({
            'coefs': t['coefs'], 'wcols': t['wcols'], 'fcols': t['fcols'],
            'gtab': t['gtab'], 'misc': t['misc'], 'texi': t['texi'],
            'basis': basis, 'pyexp': pyexp, 'pxcol': pxcol,
            'pxb': np.broadcast_to(PXg[sl][None, :], (128, NPX)).copy(),
            'pyb': np.broadcast_to(PYg[sl][None, :], (128, NPX)).copy(),
        })
    return in_maps, normal1_out


_NC_CACHE = {}


def _register_seg_a():
    """Register the fused soft-prob custom DVE op:
    out = t*(t*CE - S) + Q with t = clip01(S*IE)   (S=in0, Q=in1, IE=s0, CE=s1)."""
    import concourse.dve_ops as dve_ops
    if any(o.name == "SEG_A_ANT" for o in dve_ops.OPS):
        return tuple(next(o for o in dve_ops.OPS if o.name == n)
                     for n in ("SEG_A_ANT", "ZSEL_ANT", "ZSEL2_ANT", "MINA2_ANT",
                               "PSEL_ANT", "PSEL2_ANT"))
    from concourse.dve_spec import Spec, Src0, Src1, C0, C1, C2, relu, minn
    from concourse.dve_table_gen import dve_ver_for

    def ref_seg_a(in0, in1, s0, s1, imm2):
        t = np.minimum(np.maximum(in0.astype(np.float32) * s0, 0), imm2).astype(np.float32)
        return (t * (t * s1 - in0) + in1).astype(np.float32)

    t_expr = minn(relu(Src0 * C0), C2)
    spec = Spec(body=t_expr * (t_expr * C1 - Src0) + Src1, reference=ref_seg_a)
    row = dve_ops._CUSTOM_DVE_ROW_BASE + len(dve_ops.OPS)
    op = dve_ops.DveOp("SEG_A_ANT", spec, subdim=False,
                       uops_sha={"v3": "a415bb456e75ca33"})
    dve_ops.OPS.append(op)
    dve_ops._SUB_OPCODE_FOR_NAME[op.name] = row
    dve_ops.CUSTOM_DVE_SPECS[op.name] = spec

    def ref_zsel(in0, in1, s0, s1, imm2):
        return (in0.astype(np.float32) * in1 + (in0 - imm2) * s0).astype(np.float32)

    spec2 = Spec(body=Src0 * Src1 + (Src0 - C2) * C0, reference=ref_zsel)
    row2 = dve_ops._CUSTOM_DVE_ROW_BASE + len(dve_ops.OPS)
    op2 = dve_ops.DveOp("ZSEL_ANT", spec2, subdim=False,
                        uops_sha={"v3": "4c6249d316d2ba3d"})
    dve_ops.OPS.append(op2)
    dve_ops._SUB_OPCODE_FOR_NAME[op2.name] = row2
    dve_ops.CUSTOM_DVE_SPECS[op2.name] = spec2

    from concourse.dve_spec import Zero, select

    def ref_zsel2(in0, in1, s0, s1, imm2):
        return np.where((in0.astype(np.float32) + s0) >= 0,
                        (in1.astype(np.float32) + s1).astype(np.float32),
                        np.float32(imm2)).astype(np.float32)

    spec3 = Spec(body=select((Src0 + C0) >= Zero, Src1 + C1, C2), reference=ref_zsel2)
    row3 = dve_ops._CUSTOM_DVE_ROW_BASE + len(dve_ops.OPS)
    op3 = dve_ops.DveOp("ZSEL2_ANT", spec3, subdim=False,
                        uops_sha={"v3": "c08a5da8e56941a5"})
    dve_ops.OPS.append(op3)
    dve_ops._SUB_OPCODE_FOR_NAME[op3.name] = row3
    dve_ops.CUSTOM_DVE_SPECS[op3.name] = spec3

    from concourse.dve_spec import minn

    def ref_mina2(in0, in1, s0, s1, imm2):
        return np.minimum((in0.astype(np.float32) + s0).astype(np.float32),
                          (in1.astype(np.float32) + s1).astype(np.float32)).astype(np.float32)

    spec4 = Spec(body=minn(Src0 + C0, Src1 + C1), reference=ref_mina2)
    row4 = dve_ops._CUSTOM_DVE_ROW_BASE + len(dve_ops.OPS)
    op4 = dve_ops.DveOp("MINA2_ANT", spec4, subdim=False,
                        uops_sha={"v3": "c432cfb49e0ef018"})
    dve_ops.OPS.append(op4)
    dve_ops._SUB_OPCODE_FOR_NAME[op4.name] = row4
    dve_ops.CUSTOM_DVE_SPECS[op4.name] = spec4

    def ref_psel(in0, in1, s0, s1, imm2):
        return np.where((in0.astype(np.float32) + s0) >= 0, np.float32(imm2),
                        in1.astype(np.float32)).astype(np.float32)

    spec5 = Spec(body=select((Src0 + C0) >= Zero, C2, Src1), reference=ref_psel)
    row5 = dve_ops._CUSTOM_DVE_ROW_BASE + len(dve_ops.OPS)
    op5 = dve_ops.DveOp("PSEL_ANT", spec5, subdim=False,
                        uops_sha={"v3": "77b9264861ef90d8"})
    dve_ops.OPS.append(op5)
    dve_ops._SUB_OPCODE_FOR_NAME[op5.name] = row5
    dve_ops.CUSTOM_DVE_SPECS[op5.name] = spec5

    def ref_psel2(in0, in1, s0, s1, imm2):
        return np.where((in0.astype(np.float32) + s0) >= 0, np.float32(imm2),
                        np.minimum(in1.astype(np.float32), np.float32(imm2))).astype(np.float32)

    spec6 = Spec(body=select((Src0 + C0) >= Zero, C2, minn(Src1, C2)), reference=ref_psel2)
    row6 = dve_ops._CUSTOM_DVE_ROW_BASE + len(dve_ops.OPS)
    op6 = dve_ops.DveOp("PSEL2_ANT", spec6, subdim=False,
                        uops_sha={"v3": "49f2e61b1b302f84"})
    dve_ops.OPS.append(op6)
    dve_ops._SUB_OPCODE_FOR_NAME[op6.name] = row6
    dve_ops.CUSTOM_DVE_SPECS[op6.name] = spec6
    return op, op2, op3, op4, op5, op6


def build_nc():
    import concourse.bacc as bacc
    import concourse.bass as bass
    import concourse.mybir as mybir
    import concourse.bass_isa as bass_isa
    from concourse.tile import TileContext

    dt = mybir.dt
    Alu = mybir.AluOpType
    Act = mybir.ActivationFunctionType

    seg_a_op, zsel_op, zsel2_op, mina2_op, psel_op, psel2_op = _register_seg_a()
    nc = bacc.Bacc(trn_type="TRN2")
    coefs_d = nc.dram_tensor("coefs", [NT, 6, 4, 128], dt.float32, kind="ExternalInput")
    wcols_d = nc.dram_tensor("wcols", [128, NT * 12], dt.float32, kind="ExternalInput")
    fcols_d = nc.dram_tensor("fcols", [128, NT * 8], dt.float32, kind="ExternalInput")
    gtab_d = nc.dram_tensor("gtab", [128, NT * 36], dt.float32, kind="ExternalInput")
    misc_d = nc.dram_tensor("misc", [128, 16], dt.float32, kind="ExternalInput")
    texi_d = nc.dram_tensor("texi", [TEXN, 1], dt.float32, kind="ExternalInput")
    basis_d = nc.dram_tensor("basis", [4, NPX], dt.float32, kind="ExternalInput")
    pxb_d = nc.dram_tensor("pxb", [128, NPX], dt.float32, kind="ExternalInput")
    pyb_d = nc.dram_tensor("pyb", [128, NPX], dt.float32, kind="ExternalInput")
    pyexp_d = nc.dram_tensor("pyexp", [128, ROWS * 9], dt.float32, kind="ExternalInput")
    pxcol_d = nc.dram_tensor("pxcol", [128, 1], dt.float32, kind="ExternalInput")
    imr_d = nc.dram_tensor("imr", [ROWS, W, 3], dt.float32, kind="ExternalOutput")
    imp_d = nc.dram_tensor("imp", [ROWS, W], dt.float32, kind="ExternalOutput")

    with TileContext(nc) as tc:
        with tc.tile_pool(name="cb", bufs=1) as cb, \
             tc.tile_pool(name="zb", bufs=1) as zbp, \
             tc.tile_pool(name="wk", bufs=2) as wk, \
             tc.tile_pool(name="ep", bufs=1) as ep, \
             tc.tile_pool(name="ps1", bufs=1, space="PSUM") as ps1, \
             tc.tile_pool(name="ps2", bufs=2, space="PSUM") as ps2, \
             tc.tile_pool(name="dr", bufs=1, space="DRAM") as dr:

            # ---- constants into SBUF ----
            coefs_sb = cb.tile([4, NT * 6 * 128], dt.float32, tag="coefs")
            nc.sync.dma_start(out=coefs_sb[:].rearrange("k (t m j) -> k t m j", t=NT, m=6),
                              in_=coefs_d[:].rearrange("t m k j -> k t m j"))
            wcols = cb.tile([128, NT * 12], dt.float32, tag="wcols")
            nc.sync.dma_start(out=wcols[:], in_=wcols_d[:])
            fcols = cb.tile([128, NT * 8], dt.float32, tag="fcols")
            nc.sync.dma_start(out=fcols[:], in_=fcols_d[:])
            gtab = cb.tile([128, NT * 36], dt.float32, tag="gtab")
            nc.sync.dma_start(out=gtab[:], in_=gtab_d[:])
            misc = cb.tile([128, 16], dt.float32, tag="misc")
            nc.sync.dma_start(out=misc[:], in_=misc_d[:])

            pyexp = cb.tile([128, ROWS * 9], dt.float32, tag="pyexp")
            nc.sync.dma_start(out=pyexp[:], in_=pyexp_d[:])
            pxcol = cb.tile([128, 1], dt.float32, tag="pxcol")
            nc.sync.dma_start(out=pxcol[:], in_=pxcol_d[:])
            ones = cb.tile([128, 1], dt.float32, tag="ones")
            nc.vector.memset(ones[:], 1.0)

            imfeat = ep.tile([128, ROWS * 36], dt.float32, tag="imfeat")
            covT = ep.tile([128, ROWS], dt.float32, tag="covT")
            covd = dr.tile([ROWS, W], dt.float32, tag="covd")
            gathA = ep.tile([128, ROWS * 2 * 6], dt.float32, tag="gathA")
            uA = ep.tile([128, ROWS], dt.float32, tag="uA")
            vA = ep.tile([128, ROWS], dt.float32, tag="vA")

            def fcol(t, j):
                return fcols[:, t * 8 + j:t * 8 + j + 1]

            def wcol(t, j):
                return wcols[:, t * 12 + j:t * 12 + j + 1]

            # ---------------- main per-chunk loop ----------------
            for ch in range(NCHUNK):
                pxs = slice(ch * CH, (ch + 1) * CH)
                bas = wk.tile([4, CH], dt.float32, tag="bas")
                nc.sync.dma_start(out=bas[:], in_=basis_d[:, pxs])
                pxbc = wk.tile([128, CH], dt.float32, tag="pxbc")
                nc.sync.dma_start(out=pxbc[:], in_=pxb_d[:, pxs])
                pybc = wk.tile([128, CH], dt.float32, tag="pybc")
                nc.sync.dma_start(out=pybc[:], in_=pyb_d[:, pxs])

                zts = []
                sg = ps1.tile([1, CH], dt.float32, tag="sg", space="PSUM")
                in_tiles = []
                pc_tiles = []
                for t in range(NT):
                    # ---- exact w/zi maps: s_m = PX*c0 + PY*c1 (ACT fma + STT) ----
                    ss = []
                    for m in range(4):
                        u = wk.tile([128, CH], dt.float32, tag=f"u{m}", name=f"u{m}")
                        nc.scalar.activation(u[:], pxbc[:], Act.Copy,
                                             scale=wcol(t, m * 3 + 0))
                        nc.vector.scalar_tensor_tensor(
                            out=u[:], in0=pybc[:], scalar=wcol(t, m * 3 + 1),
                            in1=u[:], op0=Alu.mult, op1=Alu.add)
                        ss.append(u)
                    m1 = wk.tile([128, CH], dt.float32, tag="m1")
                    nc.vector._custom_dve(mina2_op, out=m1[:], in0=ss[0][:], in1=ss[1][:],
                                          s0=wcol(t, 0 * 3 + 2), s1=wcol(t, 1 * 3 + 2),
                                          imm2=0.0)
                    m3 = wk.tile([128, CH], dt.float32, tag="m3")
                    nc.vector.scalar_tensor_tensor(
                        out=m3[:], in0=ss[2][:], scalar=wcol(t, 2 * 3 + 2),
                        in1=m1[:], op0=Alu.add, op1=Alu.min)

                    zb = zbp.tile([128, CH], dt.float32, tag=f"z{t}")
                    nc.vector._custom_dve(zsel2_op, out=zb[:], in0=m3[:], in1=ss[3][:],
                                          s0=fcol(t, 7), s1=wcol(t, 3 * 3 + 2),
                                          imm2=float(ZFILL))
                    zts.append(zb)

                    # ---- soft-prob path (PE maps) ----
                    a_list = []
                    for e in range(3):
                        Sp = ps2.tile([128, CH], dt.float32, tag="qS", space="PSUM")
                        nc.tensor.matmul(
                            Sp[:], coefs_sb[:, (t * 6 + e) * 128:(t * 6 + e + 1) * 128],
                            bas[:], start=True, stop=True)
                        Qp = ps2.tile([128, CH], dt.float32, tag="qQ", space="PSUM")
                        nc.tensor.matmul(
                            Qp[:], coefs_sb[:, (t * 6 + e + 3) * 128:(t * 6 + e + 4) * 128],
                            bas[:], start=True, stop=True)
                        Qs = wk.tile([128, CH], dt.float32, tag="Qs")
                        nc.scalar.activation(Qs[:], Qp[:], Act.Copy)
                        a = wk.tile([128, CH], dt.float32, tag=f"a{e}")
                        nc.vector._custom_dve(seg_a_op, out=a[:], in0=Sp[:], in1=Qs[:],
                                              s0=fcol(t, e), s1=fcol(t, 3 + e), imm2=1.0)
                        a_list.append(a)
                    am = wk.tile([128, CH], dt.float32, tag="am")
                    nc.vector.tensor_tensor(out=am[:], in0=a_list[0][:], in1=a_list[1][:], op=Alu.min)
                    am2 = wk.tile([128, CH], dt.float32, tag="am2")
                    nc.vector.tensor_tensor(out=am2[:], in0=am[:], in1=a_list[2][:], op=Alu.min)
                    pp = wk.tile([128, CH], dt.float32, tag="pp")
                    nc.scalar.activation(pp[:], am2[:], Act.Exp, scale=-1.0)
                    pct = zbp.tile([128, CH], dt.float32, tag=f"pc{t}", name=f"pc{t}")
                    nc.vector._custom_dve(psel2_op, out=pct[:], in0=m3[:], in1=pp[:],
                                          s0=fcol(t, 6), s1=0.0, imm2=PMAX)
                    pc_tiles.append(pct)

                # ---- batched Ln pass (one ACT table switch per chunk) ----
                for t in range(NT):
                    lg = wk.tile([128, CH], dt.float32, tag="lg")
                    nc.scalar.activation(lg[:], pc_tiles[t][:], Act.Ln, scale=-1.0, bias=1.0)
                    nc.tensor.matmul(sg[:], ones[:], lg[:], start=(t == 0),
                                     stop=(t == NT - 1), skip_group_check=True)

                # ---- zmax + sum(lg) partition reductions ----
                zmt = wk.tile([128, CH], dt.float32, tag="zmt")
                nc.vector.tensor_tensor(out=zmt[:], in0=zts[0][:], in1=zts[1][:], op=Alu.max)
                for t in range(2, NT):
                    nc.vector.tensor_tensor(out=zmt[:], in0=zmt[:], in1=zts[t][:], op=Alu.max)
                zmaxb = wk.tile([128, CH], dt.float32, tag="zmaxb")
                nc.gpsimd.partition_all_reduce(zmaxb[:], zmt[:], 128, bass_isa.ReduceOp.max)

                # ---- one-hot winner + feature matmul (accumulate over tiles) ----
                gall = ps1.tile([128, RCH * 36], dt.float32, tag="gall", space="PSUM")
                ohs = []
                for t in range(NT):
                    oh = zbp.tile([128, CH], dt.float32, tag=f"oh{t}", name=f"oh{t}")
                    nc.vector.tensor_tensor(out=oh[:], in0=zts[t][:], in1=zmaxb[:], op=Alu.is_equal)
                    ohs.append(oh)
                for blk in range(RCH):
                    for t in range(NT):
                        nc.tensor.matmul(gall[:, blk * 36:(blk + 1) * 36],
                                         ohs[t][:, blk * 128:(blk + 1) * 128],
                                         gtab[:, t * 36:(t + 1) * 36],
                                         start=(t == 0), stop=(t == NT - 1),
                                         skip_group_check=True)
                nc.scalar.activation(
                    imfeat[:, ch * RCH * 36:(ch + 1) * RCH * 36], gall[:], Act.Copy)

                # --- texcoords + texture gathers for this chunk's 4 rows ---
                ut = wk.tile([128, RCH], dt.float32, tag="ut")
                vt = wk.tile([128, RCH], dt.float32, tag="vt")
                tt1 = wk.tile([128, RCH], dt.float32, tag="tt1")
                idf = wk.tile([128, RCH * 2], dt.float32, tag="idf")
                idi = wk.tile([128, RCH * 2], dt.int32, tag="idi")
                imfc = imfeat[:, ch * RCH * 36:(ch + 1) * RCH * 36].rearrange(
                    "p (r j d) -> p r j d", j=4, d=9)
                for j, dst in ((6, ut), (7, vt)):
                    nc.vector.tensor_scalar(out=tt1[:], in0=imfc[:, :, 0, j],
                                            scalar1=pxcol[:, 0:1], scalar2=None, op0=Alu.mult)
                    nc.vector.tensor_tensor(out=dst[:], in0=imfc[:, :, 1, j],
                                            in1=pyexp[:].rearrange("p (r d) -> p r d", d=9)[
                                                :, ch * RCH:(ch + 1) * RCH, 0],
                                            op=Alu.mult)
                    nc.vector.tensor_tensor(out=dst[:], in0=dst[:], in1=tt1[:], op=Alu.add)
                    nc.vector.tensor_tensor(out=dst[:], in0=dst[:], in1=imfc[:, :, 2, j], op=Alu.add)
                # u = clip(u,0,1)*511 ; v = (1-clip(v,0,1))*511
                nc.vector.tensor_scalar(out=ut[:], in0=ut[:], scalar1=0.0, scalar2=1.0,
                                        op0=Alu.max, op1=Alu.min)
                nc.vector.tensor_scalar(out=ut[:], in0=ut[:], scalar1=float(TEX - 1),
                                        scalar2=None, op0=Alu.mult)
                nc.vector.tensor_scalar(out=vt[:], in0=vt[:], scalar1=0.0, scalar2=1.0,
                                        op0=Alu.max, op1=Alu.min)
                nc.vector.tensor_scalar(out=vt[:], in0=vt[:], scalar1=-1.0, scalar2=1.0,
                                        op0=Alu.mult, op1=Alu.add)
                nc.vector.tensor_scalar(out=vt[:], in0=vt[:], scalar1=float(TEX - 1),
                                        scalar2=None, op0=Alu.mult)
                nc.vector.tensor_copy(uA[:, ch * RCH:(ch + 1) * RCH], ut[:])
                nc.vector.tensor_copy(vA[:, ch * RCH:(ch + 1) * RCH], vt[:])
                # floor (exact, rounding-agnostic)
                ci = wk.tile([128, RCH], dt.int32, tag="ci")
                fx = wk.tile([128, RCH], dt.float32, tag="fx")
                x0c = wk.tile([128, RCH], dt.float32, tag="x0c")
                y0c = wk.tile([128, RCH], dt.float32, tag="y0c")
                for srcv, dstv in ((ut, x0c), (vt, y0c)):
                    nc.vector.tensor_copy(ci[:], srcv[:])
                    nc.vector.tensor_copy(dstv[:], ci[:])
                    nc.vector.tensor_tensor(out=fx[:], in0=dstv[:], in1=srcv[:], op=Alu.is_gt)
                    nc.vector.tensor_tensor(out=dstv[:], in0=dstv[:], in1=fx[:], op=Alu.subtract)
                y1c = wk.tile([128, RCH], dt.float32, tag="y1c")
                nc.vector.tensor_scalar(out=y1c[:], in0=y0c[:], scalar1=1.0,
                                        scalar2=float(TEX - 1), op0=Alu.add, op1=Alu.min)
                idv = idf[:].rearrange("p (r d) -> p r d", d=2)
                nc.vector.scalar_tensor_tensor(out=idv[:, :, 0], in0=y0c[:], scalar=float(TEX),
                                               in1=x0c[:], op0=Alu.mult, op1=Alu.add)
                nc.vector.scalar_tensor_tensor(out=idv[:, :, 1], in0=y1c[:], scalar=float(TEX),
                                               in1=x0c[:], op0=Alu.mult, op1=Alu.add)
                nc.vector.tensor_scalar(out=idf[:], in0=idf[:], scalar1=3.0, scalar2=None,
                                        op0=Alu.mult)
                nc.vector.tensor_copy(idi[:], idf[:])
                for r in range(RCH):
                    for wch in range(2):
                        gi = (ch * RCH + r) * 2 + wch
                        nc.gpsimd.indirect_dma_start(
                            out=gathA[:, gi * 6:(gi + 1) * 6], out_offset=None,
                            in_=texi_d[:],
                            in_offset=bass.IndirectOffsetOnAxis(
                                ap=idi[:, r * 2 + wch:r * 2 + wch + 1], axis=0))

                # ---- coverage + improb for this chunk ----
                cr = ep.tile([1, CH], dt.float32, tag="cr")
                nc.vector.tensor_scalar(out=cr[:], in0=zmaxb[0:1, :], scalar1=float(ZFILL),
                                        scalar2=None, op0=Alu.is_gt)
                nc.gpsimd.dma_start(
                    out=covd[ch * RCH:(ch + 1) * RCH, :], in_=cr[:])
                exr = ep.tile([1, CH], dt.float32, tag="exr")
                nc.scalar.activation(exr[:], sg[:], Act.Exp)
                impc = ep.tile([1, CH], dt.float32, tag="impc")
                nc.vector.tensor_scalar(out=impc[:], in0=exr[:], scalar1=-1.0,
                                        scalar2=1.0, op0=Alu.mult, op1=Alu.add)
                nc.sync.dma_start(
                    out=imp_d[ch * RCH:(ch + 1) * RCH, :].rearrange("r c -> r c")[None, :, :],
                    in_=impc[:].rearrange("a (r c) -> a r c", c=W))

            # ---------------- core epilogue ([col, row] layout) ----------------
            NR = ROWS
            nc.gpsimd.dma_start(out=covT[:], in_=covd[:].rearrange("r c -> c r"))
            i3 = imfeat[:].rearrange("p (r j d) -> p r j d", j=4, d=9)
            imf9 = ep.tile([128, NR * 9], dt.float32, tag="imf9")
            v9 = imf9[:].rearrange("p (r d) -> p r d", d=9)
            tmp9 = ep.tile([128, NR * 9], dt.float32, tag="tmp9")
            t9 = tmp9[:].rearrange("p (r d) -> p r d", d=9)
            nc.vector.tensor_scalar(out=t9[:, :, :], in0=i3[:, :, 0, :],
                                    scalar1=pxcol[:, 0:1], scalar2=None, op0=Alu.mult)
            nc.vector.tensor_tensor(out=v9[:, :, :], in0=i3[:, :, 1, :],
                                    in1=pyexp[:].rearrange("p (r d) -> p r d", d=9),
                                    op=Alu.mult)
            nc.vector.tensor_tensor(out=v9[:, :, :], in0=v9[:, :, :], in1=t9[:, :, :], op=Alu.add)
            nc.vector.tensor_tensor(out=v9[:, :, :], in0=v9[:, :, :], in1=i3[:, :, 2, :], op=Alu.add)

            def vsl(j, k=1):
                return imf9[:].rearrange("p (r d) -> p r d", d=9)[:, :, j:j + k]

            nc.vector.tensor_tensor(out=vsl(8)[:, :, 0], in0=vsl(8)[:, :, 0],
                                    in1=covT[:], op=Alu.mult)

            def normalize3(dst_tag, j0):
                sq = ep.tile([128, NR * 3], dt.float32, tag=dst_tag + "sq")
                s3 = sq[:].rearrange("p (r d) -> p r d", d=3)
                nc.vector.tensor_tensor(out=s3[:], in0=vsl(j0, 3)[:, :, :],
                                        in1=vsl(j0, 3)[:, :, :], op=Alu.mult)
                n2 = ep.tile([128, NR], dt.float32, tag=dst_tag + "n2")
                nc.vector.tensor_reduce(out=n2[:], in_=s3[:], axis=mybir.AxisListType.X,
                                        op=Alu.add)
                sn = ep.tile([128, NR], dt.float32, tag=dst_tag + "sn")
                nc.scalar.activation(sn[:], n2[:], Act.Sqrt)
                nc.vector.tensor_scalar(out=sn[:], in0=sn[:], scalar1=float(EPS),
                                        scalar2=None, op0=Alu.add)
                rc = ep.tile([128, NR], dt.float32, tag=dst_tag + "rc")
                nc.vector.reciprocal(rc[:], sn[:])
                out = ep.tile([128, NR * 3], dt.float32, tag=dst_tag)
                o3 = out[:].rearrange("p (r d) -> p r d", d=3)
                for k in range(3):
                    nc.vector.tensor_tensor(out=o3[:, :, k], in0=vsl(j0 + k)[:, :, 0],
                                            in1=rc[:], op=Alu.mult)
                return out, o3

            nrm, nrm3 = normalize3("nrm", 0)
            eye, eye3 = normalize3("eye", 3)

            def mcol(j):
                return misc[:, j:j + 1]

            cosT = ep.tile([128, NR], dt.float32, tag="cosT")
            nc.vector.tensor_scalar(out=cosT[:], in0=nrm3[:, :, 0], scalar1=mcol(9),
                                    scalar2=None, op0=Alu.mult)
            nc.vector.scalar_tensor_tensor(out=cosT[:], in0=nrm3[:, :, 1], scalar=mcol(10),
                                           in1=cosT[:], op0=Alu.mult, op1=Alu.add)
            nc.vector.scalar_tensor_tensor(out=cosT[:], in0=nrm3[:, :, 2], scalar=mcol(11),
                                           in1=cosT[:], op0=Alu.mult, op1=Alu.add)
            nc.vector.tensor_scalar(out=cosT[:], in0=cosT[:], scalar1=0.0, scalar2=1.0,
                                    op0=Alu.max, op1=Alu.min)
            cosA = ep.tile([128, NR], dt.float32, tag="cosA")
            rk = ep.tile([128, NR], dt.float32, tag="rk")
            for k in range(3):
                nc.vector.tensor_tensor(out=rk[:], in0=cosT[:], in1=nrm3[:, :, k], op=Alu.mult)
                nc.vector.tensor_scalar(out=rk[:], in0=rk[:], scalar1=2.0, scalar2=None,
                                        op0=Alu.mult)
                nc.vector.tensor_scalar(out=rk[:], in0=rk[:], scalar1=mcol(9 + k),
                                        scalar2=None, op0=Alu.subtract)
                nc.vector.tensor_tensor(out=rk[:], in0=rk[:], in1=eye3[:, :, k], op=Alu.mult)
                if k == 0:
                    nc.vector.tensor_copy(cosA[:], rk[:])
                else:
                    nc.vector.tensor_tensor(out=cosA[:], in0=cosA[:], in1=rk[:], op=Alu.add)
            nc.vector.tensor_scalar(out=cosA[:], in0=cosA[:], scalar1=1e-5, scalar2=1.0,
                                    op0=Alu.max, op1=Alu.min)
            nc.scalar.activation(cosA[:], cosA[:], Act.Ln)
            nc.scalar.activation(cosA[:], cosA[:], Act.Exp, scale=mcol(12))

            # texture lookup (u/v + gathers were produced per-chunk)
            uu = uA
            vv = vA
            # exact floor: cast to int and back, then subtract 1 where it rounded up
            cint = ep.tile([128, NR], dt.int32, tag="cint")
            fixt = ep.tile([128, NR], dt.float32, tag="fixt")

            def floorf(dst_tag, src):
                nc.vector.tensor_copy(cint[:], src[:])
                dst = ep.tile([128, NR], dt.float32, tag=dst_tag, name=dst_tag)
                nc.vector.tensor_copy(dst[:], cint[:])
                nc.vector.tensor_tensor(out=fixt[:], in0=dst[:], in1=src[:], op=Alu.is_gt)
                nc.vector.tensor_tensor(out=dst[:], in0=dst[:], in1=fixt[:], op=Alu.subtract)
                return dst

            x0f = floorf("x0f", uu)
            y0f = floorf("y0f", vv)
            wx = ep.tile([128, NR], dt.float32, tag="wx")
            nc.vector.tensor_tensor(out=wx[:], in0=uu[:], in1=x0f[:], op=Alu.subtract)
            wy = ep.tile([128, NR], dt.float32, tag="wy")
            nc.vector.tensor_tensor(out=wy[:], in0=vv[:], in1=y0f[:], op=Alu.subtract)
            gv = gathA[:].rearrange("p (r w d) -> p r w d", w=2, d=6)

            wxc = ep.tile([128, NR], dt.float32, tag="wxc")
            nc.vector.tensor_scalar(out=wxc[:], in0=wx[:], scalar1=-1.0, scalar2=1.0,
                                    op0=Alu.mult, op1=Alu.add)
            wyc = ep.tile([128, NR], dt.float32, tag="wyc")
            nc.vector.tensor_scalar(out=wyc[:], in0=wy[:], scalar1=-1.0, scalar2=1.0,
                                    op0=Alu.mult, op1=Alu.add)
            w00 = ep.tile([128, NR], dt.float32, tag="w00")
            nc.vector.tensor_tensor(out=w00[:], in0=wxc[:], in1=wyc[:], op=Alu.mult)
            w01 = ep.tile([128, NR], dt.float32, tag="w01")
            nc.vector.tensor_tensor(out=w01[:], in0=wx[:], in1=wyc[:], op=Alu.mult)
            w10 = ep.tile([128, NR], dt.float32, tag="w10")
            nc.vector.tensor_tensor(out=w10[:], in0=wxc[:], in1=wy[:], op=Alu.mult)
            w11 = ep.tile([128, NR], dt.float32, tag="w11")
            nc.vector.tensor_tensor(out=w11[:], in0=wx[:], in1=wy[:], op=Alu.mult)

            colorT = ep.tile([128, NR * 3], dt.float32, tag="colorT")
            c3v = colorT[:].rearrange("p (r d) -> p r d", d=3)
            tcc = ep.tile([128, NR], dt.float32, tag="tcc")
            acc = ep.tile([128, NR], dt.float32, tag="acc")
            mm = ep.tile([128, NR], dt.float32, tag="mm")
            for c in range(3):
                nc.vector.tensor_tensor(out=tcc[:], in0=gv[:, :, 0, c], in1=w00[:], op=Alu.mult)
                nc.vector.tensor_tensor(out=mm[:], in0=gv[:, :, 0, 3 + c], in1=w01[:], op=Alu.mult)
                nc.vector.tensor_tensor(out=tcc[:], in0=tcc[:], in1=mm[:], op=Alu.add)
                nc.vector.tensor_tensor(out=mm[:], in0=gv[:, :, 1, c], in1=w10[:], op=Alu.mult)
                nc.vector.tensor_tensor(out=tcc[:], in0=tcc[:], in1=mm[:], op=Alu.add)
                nc.vector.tensor_tensor(out=mm[:], in0=gv[:, :, 1, 3 + c], in1=w11[:], op=Alu.mult)
                nc.vector.tensor_tensor(out=tcc[:], in0=tcc[:], in1=mm[:], op=Alu.add)
                nc.vector.tensor_tensor(out=mm[:], in0=cosT[:], in1=tcc[:], op=Alu.mult)
                nc.vector.tensor_scalar(out=acc[:], in0=tcc[:], scalar1=mcol(c),
                                        scalar2=None, op0=Alu.mult)
                nc.vector.scalar_tensor_tensor(out=acc[:], in0=mm[:], scalar=mcol(3 + c),
                                               in1=acc[:], op0=Alu.mult, op1=Alu.add)
                nc.vector.scalar_tensor_tensor(out=acc[:], in0=cosA[:], scalar=mcol(6 + c),
                                               in1=acc[:], op0=Alu.mult, op1=Alu.add)
                nc.vector.tensor_tensor(out=acc[:], in0=acc[:], in1=vsl(8)[:, :, 0], op=Alu.mult)
                nc.vector.tensor_scalar(out=c3v[:, :, c], in0=acc[:], scalar1=0.0, scalar2=1.0,
                                        op0=Alu.max, op1=Alu.min)

            nc.sync.dma_start(out=imr_d[:].rearrange("r c k -> c r k"),
                              in_=colorT[:].rearrange("p (r k) -> p r k", k=3))

    nc.finalize()
    return nc


def kernel(points, faces, camera_rot, camera_pos, camera_proj, uv, ft, texture,
           lightdirect, material, shininess, height, width):
    from concourse.bass_utils import run_bass_kernel_spmd

    inputs = dict(points=points, faces=faces, camera_rot=camera_rot,
                  camera_pos=camera_pos, camera_proj=camera_proj, uv=uv, ft=ft,
                  texture=texture, lightdirect=lightdirect, material=material,
                  shininess=shininess)
    in_maps, normal1 = host_prep(inputs)

    if 'nc' not in _NC_CACHE:
        _NC_CACHE['nc'] = build_nc()
    nc = _NC_CACHE['nc']

    res = run_bass_kernel_spmd(nc, in_maps, core_ids=list(range(NCORES)))

    imrender = np.zeros((B, H, W, 3), dtype=f32)
    improb = np.zeros((B, H, W, 1), dtype=f32)
    for core in range(NCORES):
        b = core // BANDS
        r0 = (core % BANDS) * ROWS
        imrender[b, r0:r0 + ROWS] = res.results[core]['imr']
        improb[b, r0:r0 + ROWS, :, 0] = res.results[core]['imp']
    return imrender, improb, normal1


# revision 23
# speedup vs baseline: 1.1843x; 1.0073x over previous
"""Trainium2 Bass kernel for nn_PhongRender (DIB-R style Phong renderer).

kernel(**inputs) takes FULL unsharded inputs (as from setup_inputs()) and
returns (imrender [2,128,128,3], improb [2,128,128,1], normal1 [2,800,3]).

Sharding: 8 cores = 2 batches x 4 row-bands of 32 image rows each.
Each core rasterizes its 32x128 pixel band against all 800 faces.

Per-core device pipeline (f-layout [128 faces, 512 px] tiles):
  - w0n/w1n/w2n/zi maps: exact elementwise evaluation (ACT fma + DVE STT/TS)
    with the same f32 rounding sequence as the reference-validated host sim
    (hard decisions: inside tests, z-buffer winner).
  - S/Q soft-silhouette maps: PE matmul (K=4 affine basis), smooth path.
  - zmax / sum(log) partition reductions: GPSIMD all-reduce (exact f32).
  - winner one-hot (zbuf == zmax) -> feature interpolation via PE matmul
    against a precomputed per-face G table.
  - fragment shader + bilinear texture (indirect DMA gather) on-device.
"""
import numpy as np

f32 = np.float32

B, P, F, H, W, TEX = 2, 500, 800, 128, 128, 512
NCORES = 8
BANDS = 4                 # row-bands per batch
ROWS = H // BANDS         # 32 rows per core
NPX = ROWS * W            # 4096 pixels per core
CH = 512                  # pixels per chunk (4 image rows)
NCHUNK = NPX // CH        # 8
RCH = CH // W             # rows per chunk = 4
NT = (F + 127) // 128     # 7 face tiles
FP = NT * 128             # 896 padded faces
EPS = f32(1e-15)
MULT = 1000.0
DELTA = 7000.0
NEG_BIG = f32(-1e30)
ZFILL = f32(-1e10)
LOG1EM7 = float(np.log(np.float32(1e-7)))
PMAX = float(np.float32(1.0) - np.float32(1e-7))
TEXN = TEX * TEX * 3 + 16


def _fma(a, b, c):
    """f32 fused multiply-add (matches XLA CPU's fma contraction)."""
    return (a.astype(np.float64) * b.astype(np.float64) + c.astype(np.float64)).astype(f32)


def _cross_fma(a, b):
    """cross product with XLA-CPU's fma pattern: fma(a1,b2, -(a2*b1))."""
    return np.stack([
        _fma(a[:, 1], b[:, 2], -(a[:, 2] * b[:, 1]).astype(f32)),
        _fma(a[:, 2], b[:, 0], -(a[:, 0] * b[:, 2]).astype(f32)),
        _fma(a[:, 0], b[:, 1], -(a[:, 1] * b[:, 0]).astype(f32)),
    ], -1)


def host_prep(inputs):
    """Build per-core input tensor maps + host-computed normal1."""
    points = np.asarray(inputs['points'], dtype=f32)
    faces = np.asarray(inputs['faces'])
    camera_rot = np.asarray(inputs['camera_rot'], dtype=f32)
    camera_pos = np.asarray(inputs['camera_pos'], dtype=f32)
    camera_proj = np.asarray(inputs['camera_proj'], dtype=f32).reshape(3)
    uv = np.asarray(inputs['uv'], dtype=f32)
    ft = np.asarray(inputs['ft'])
    texture = np.asarray(inputs['texture'], dtype=f32)
    lightdirect = np.asarray(inputs['lightdirect'], dtype=f32)
    material = np.asarray(inputs['material'], dtype=f32)
    shininess = np.asarray(inputs['shininess'], dtype=f32)

    px = ((f32(2.0) * (np.arange(W, dtype=f32) + f32(0.5)) - f32(W)) / f32(W)).astype(f32)
    py = ((f32(H) - f32(2.0) * (np.arange(H, dtype=f32) + f32(0.5))) / f32(H)).astype(f32)

    cc = np.float64(MULT) ** 2 / np.float64(DELTA)

    normal1_out = np.zeros((B, F, 3), dtype=f32)
    per_batch = []
    for b in range(B):
        pts = points[b]
        pos = camera_pos[b]
        rot = camera_rot[b]
        pc = np.einsum('pj,ij->pi', (pts - pos[None, :]).astype(f32), rot).astype(f32)
        xy = (pc * camera_proj[None, :]).astype(f32)
        xy2 = (xy[:, :2] / xy[:, 2:3]).astype(f32)
        p3 = pc[faces]
        p2 = xy2[faces]
        v10 = (p3[:, 1] - p3[:, 0]).astype(f32)
        v20 = (p3[:, 2] - p3[:, 0]).astype(f32)
        normal = _cross_fma(v10, v20)
        normalz = normal[:, 2]
        nn = np.sqrt((normal * normal).sum(-1, keepdims=True).astype(f32)).astype(f32)
        normal1_out[b] = (normal / (nn + EPS)).astype(f32)

        ax, ay = p2[:, 0, 0], p2[:, 0, 1]
        bx, by = p2[:, 1, 0], p2[:, 1, 1]
        cx, cy = p2[:, 2, 0], p2[:, 2, 1]
        area = ((bx - ax) * (cy - ay) - (by - ay) * (cx - ax)).astype(f32)
        ok = np.abs(area) > f32(1e-10)
        inv = (f32(1.0) / np.where(ok, area, f32(1.0))).astype(f32)

        def edge_coef(ux, uy, vx, vy):
            ex = (vx - ux).astype(f32)
            ey = (vy - uy).astype(f32)
            return np.stack([(-ey).astype(f32), ex,
                             (ey * ux - ex * uy).astype(f32)], 0)

        cw0 = edge_coef(bx, by, cx, cy)
        cw1 = edge_coef(cx, cy, ax, ay)
        cw2 = edge_coef(ax, ay, bx, by)
        cw0n = (cw0 * inv[None]).astype(f32)
        cw1n = (cw1 * inv[None]).astype(f32)
        cw2n = (cw2 * inv[None]).astype(f32)
        z = p3[:, :, 2]
        czi = (cw0n * z[:, 0][None] + cw1n * z[:, 1][None] + cw2n * z[:, 2][None]).astype(f32)

        uvf = uv[b][ft]
        feat = np.concatenate([
            np.tile(normal[:, None, :], (1, 3, 1)),
            -p3, uvf, np.ones((F, 3, 1), dtype=f32)], axis=-1).astype(f32)
        cw = np.stack([cw0n, cw1n, cw2n], 0)  # [3k,3j,F]
        G = np.einsum('kjf,fkd->fjd', cw.astype(np.float64),
                      feat.astype(np.float64)).astype(f32)  # [F,3,9]

        def seg_tables(ux, uy, vx, vy):
            ex = (vx - ux).astype(np.float64); ey = (vy - uy).astype(np.float64)
            uxd = ux.astype(np.float64); uyd = uy.astype(np.float64)
            E = ex * ex + ey * ey
            S = np.stack([np.zeros_like(ex), 2 * cc * ex, 2 * cc * ey,
                          2 * cc * (-ex * uxd - ey * uyd)], 0).astype(f32)
            Q = np.stack([np.full_like(ex, cc), -2 * cc * uxd, -2 * cc * uyd,
                          cc * (uxd * uxd + uyd * uyd)], 0).astype(f32)
            IE = (1.0 / (2 * cc * (E + 1e-12))).astype(f32)
            CE = (cc * E).astype(f32)
            return S, Q, IE, CE

        Sab, Qab, IEab, CEab = seg_tables(ax, ay, bx, by)
        Sbc, Qbc, IEbc, CEbc = seg_tables(bx, by, cx, cy)
        Sca, Qca, IEca, CEca = seg_tables(cx, cy, ax, ay)

        okm1 = np.where(ok, f32(0.0), NEG_BIG).astype(f32)
        okm2 = np.where(ok & (normalz > 0), f32(0.0), NEG_BIG).astype(f32)

        def padF(a, fill=0.0):
            shp = list(a.shape); shp[-1] = FP
            out = np.full(shp, fill, dtype=f32)
            out[..., :F] = a
            return out

        SQ = np.stack([padF(Sab), padF(Sbc), padF(Sca),
                       padF(Qab), padF(Qbc), padF(Qca)], 0)  # [6,4,FP]
        SQ[3:, 3, F:] = f32(1e30)  # pad faces: a = huge -> lg = 0
        coefs = SQ.reshape(6, 4, NT, 128).transpose(2, 0, 1, 3).copy()  # [NT,6,4,128]

        WC = np.stack([padF(cw0n[0]), padF(cw0n[1]), padF(cw0n[2]),
                       padF(cw1n[0]), padF(cw1n[1]), padF(cw1n[2]),
                       padF(cw2n[0]), padF(cw2n[1]), padF(cw2n[2]),
                       padF(czi[0]), padF(czi[1]), padF(czi[2])], 0)  # [12, FP]
        wcols = WC.reshape(12, NT, 128).transpose(2, 1, 0).reshape(128, NT * 12).copy()

        FC = np.stack([padF(IEab), padF(IEbc), padF(IEca),
                       padF(CEab), padF(CEbc), padF(CEca),
                       padF(okm1, float(NEG_BIG)), padF(okm2, float(NEG_BIG))], 0)
        fcols = FC.reshape(8, NT, 128).transpose(2, 1, 0).reshape(128, NT * 8).copy()

        Gp = padF(G.reshape(F, 27).T).T.reshape(FP, 3, 9)
        gtab = Gp.reshape(NT, 128, 27)
        gtab = np.concatenate([gtab, np.zeros((NT, 128, 9), dtype=f32)], -1)
        gtab = gtab.transpose(1, 0, 2).reshape(128, NT * 36).copy()

        misc = np.zeros((128, 16), dtype=f32)
        mat = material[b]
        ld = lightdirect[b]
        ln = np.sqrt((ld * ld).sum().astype(f32)).astype(f32)
        ldn = (ld / (ln + EPS)).astype(f32)
        misc[:, 0:3] = mat[0][None, :]
        misc[:, 3:6] = mat[1][None, :]
        misc[:, 6:9] = mat[2][None, :]
        misc[:, 9:12] = ldn[None, :]
        misc[:, 12] = shininess[b, 0]

        ti = np.zeros(TEXN, dtype=f32)
        ti[:TEX * TEX * 3] = texture[b].transpose(1, 2, 0).reshape(-1)

        per_batch.append(dict(coefs=coefs, wcols=wcols, fcols=fcols, gtab=gtab,
                              misc=misc, texi=ti.reshape(TEXN, 1)))

    PXg = np.broadcast_to(px[None, :], (H, W)).reshape(-1).astype(f32)
    PYg = np.broadcast_to(py[:, None], (H, W)).reshape(-1).astype(f32)
    Rg = (PXg * PXg + PYg * PYg).astype(f32)

    in_maps = []
    for core in range(NCORES):
        b = core // BANDS
        band = core % BANDS
        r0 = band * ROWS
        sl = slice(r0 * W, (r0 + ROWS) * W)
        basis = np.stack([Rg[sl], PXg[sl], PYg[sl], np.ones(NPX, dtype=f32)], 0).copy()
        pyexp = np.broadcast_to(py[r0:r0 + ROWS][None, :, None], (128, ROWS, 9)).reshape(128, ROWS * 9).copy()
        pxcol = px[:, None].copy()
        t = per_batch[b]
        in_maps.append({
            'coefs': t['coefs'], 'wcols': t['wcols'], 'fcols': t['fcols'],
            'gtab': t['gtab'], 'misc': t['misc'], 'texi': t['texi'],
            'basis': basis, 'pyexp': pyexp, 'pxcol': pxcol,
            'pxb': np.broadcast_to(PXg[sl][None, :], (128, NPX)).copy(),
            'pyb': np.broadcast_to(PYg[sl][None, :], (128, NPX)).copy(),
        })
    return in_maps, normal1_out


_NC_CACHE = {}


def _register_seg_a():
    """Register the fused soft-prob custom DVE op:
    out = t*(t*CE - S) + Q with t = clip01(S*IE)   (S=in0, Q=in1, IE=s0, CE=s1)."""
    import concourse.dve_ops as dve_ops
    if any(o.name == "SEG_A_ANT" for o in dve_ops.OPS):
        return tuple(next(o for o in dve_ops.OPS if o.name == n)
                     for n in ("SEG_A_ANT", "ZSEL_ANT", "ZSEL2_ANT", "MINA2_ANT",
                               "PSEL_ANT", "PSEL2_ANT"))
    from concourse.dve_spec import Spec, Src0, Src1, C0, C1, C2, relu, minn
    from concourse.dve_table_gen import dve_ver_for

    def ref_seg_a(in0, in1, s0, s1, imm2):
        t = np.minimum(np.maximum(in0.astype(np.float32) * s0, 0), imm2).astype(np.float32)
        return (t * (t * s1 - in0) + in1).astype(np.float32)

    t_expr = minn(relu(Src0 * C0), C2)
    spec = Spec(body=t_expr * (t_expr * C1 - Src0) + Src1, reference=ref_seg_a)
    row = dve_ops._CUSTOM_DVE_ROW_BASE + len(dve_ops.OPS)
    op = dve_ops.DveOp("SEG_A_ANT", spec, subdim=False,
                       uops_sha={"v3": "a415bb456e75ca33"})
    dve_ops.OPS.append(op)
    dve_ops._SUB_OPCODE_FOR_NAME[op.name] = row
    dve_ops.CUSTOM_DVE_SPECS[op.name] = spec

    def ref_zsel(in0, in1, s0, s1, imm2):
        return (in0.astype(np.float32) * in1 + (in0 - imm2) * s0).astype(np.float32)

    spec2 = Spec(body=Src0 * Src1 + (Src0 - C2) * C0, reference=ref_zsel)
    row2 = dve_ops._CUSTOM_DVE_ROW_BASE + len(dve_ops.OPS)
    op2 = dve_ops.DveOp("ZSEL_ANT", spec2, subdim=False,
                        uops_sha={"v3": "4c6249d316d2ba3d"})
    dve_ops.OPS.append(op2)
    dve_ops._SUB_OPCODE_FOR_NAME[op2.name] = row2
    dve_ops.CUSTOM_DVE_SPECS[op2.name] = spec2

    from concourse.dve_spec import Zero, select

    def ref_zsel2(in0, in1, s0, s1, imm2):
        return np.where((in0.astype(np.float32) + s0) >= 0,
                        (in1.astype(np.float32) + s1).astype(np.float32),
                        np.float32(imm2)).astype(np.float32)

    spec3 = Spec(body=select((Src0 + C0) >= Zero, Src1 + C1, C2), reference=ref_zsel2)
    row3 = dve_ops._CUSTOM_DVE_ROW_BASE + len(dve_ops.OPS)
    op3 = dve_ops.DveOp("ZSEL2_ANT", spec3, subdim=False,
                        uops_sha={"v3": "c08a5da8e56941a5"})
    dve_ops.OPS.append(op3)
    dve_ops._SUB_OPCODE_FOR_NAME[op3.name] = row3
    dve_ops.CUSTOM_DVE_SPECS[op3.name] = spec3

    from concourse.dve_spec import minn

    def ref_mina2(in0, in1, s0, s1, imm2):
        return np.minimum((in0.astype(np.float32) + s0).astype(np.float32),
                          (in1.astype(np.float32) + s1).astype(np.float32)).astype(np.float32)

    spec4 = Spec(body=minn(Src0 + C0, Src1 + C1), reference=ref_mina2)
    row4 = dve_ops._CUSTOM_DVE_ROW_BASE + len(dve_ops.OPS)
    op4 = dve_ops.DveOp("MINA2_ANT", spec4, subdim=False,
                        uops_sha={"v3": "c432cfb49e0ef018"})
    dve_ops.OPS.append(op4)
    dve_ops._SUB_OPCODE_FOR_NAME[op4.name] = row4
    dve_ops.CUSTOM_DVE_SPECS[op4.name] = spec4

    def ref_psel(in0, in1, s0, s1, imm2):
        return np.where((in0.astype(np.float32) + s0) >= 0, np.float32(imm2),
                        in1.astype(np.float32)).astype(np.float32)

    spec5 = Spec(body=select((Src0 + C0) >= Zero, C2, Src1), reference=ref_psel)
    row5 = dve_ops._CUSTOM_DVE_ROW_BASE + len(dve_ops.OPS)
    op5 = dve_ops.DveOp("PSEL_ANT", spec5, subdim=False,
                        uops_sha={"v3": "77b9264861ef90d8"})
    dve_ops.OPS.append(op5)
    dve_ops._SUB_OPCODE_FOR_NAME[op5.name] = row5
    dve_ops.CUSTOM_DVE_SPECS[op5.name] = spec5

    def ref_psel2(in0, in1, s0, s1, imm2):
        return np.where((in0.astype(np.float32) + s0) >= 0, np.float32(imm2),
                        np.minimum(in1.astype(np.float32), np.float32(imm2))).astype(np.float32)

    spec6 = Spec(body=select((Src0 + C0) >= Zero, C2, minn(Src1, C2)), reference=ref_psel2)
    row6 = dve_ops._CUSTOM_DVE_ROW_BASE + len(dve_ops.OPS)
    op6 = dve_ops.DveOp("PSEL2_ANT", spec6, subdim=False,
                        uops_sha={"v3": "49f2e61b1b302f84"})
    dve_ops.OPS.append(op6)
    dve_ops._SUB_OPCODE_FOR_NAME[op6.name] = row6
    dve_ops.CUSTOM_DVE_SPECS[op6.name] = spec6
    return op, op2, op3, op4, op5, op6


def build_nc():
    import concourse.bacc as bacc
    import concourse.bass as bass
    import concourse.mybir as mybir
    import concourse.bass_isa as bass_isa
    from concourse.tile import TileContext

    dt = mybir.dt
    Alu = mybir.AluOpType
    Act = mybir.ActivationFunctionType

    seg_a_op, zsel_op, zsel2_op, mina2_op, psel_op, psel2_op = _register_seg_a()
    nc = bacc.Bacc(trn_type="TRN2")
    coefs_d = nc.dram_tensor("coefs", [NT, 6, 4, 128], dt.float32, kind="ExternalInput")
    wcols_d = nc.dram_tensor("wcols", [128, NT * 12], dt.float32, kind="ExternalInput")
    fcols_d = nc.dram_tensor("fcols", [128, NT * 8], dt.float32, kind="ExternalInput")
    gtab_d = nc.dram_tensor("gtab", [128, NT * 36], dt.float32, kind="ExternalInput")
    misc_d = nc.dram_tensor("misc", [128, 16], dt.float32, kind="ExternalInput")
    texi_d = nc.dram_tensor("texi", [TEXN, 1], dt.float32, kind="ExternalInput")
    basis_d = nc.dram_tensor("basis", [4, NPX], dt.float32, kind="ExternalInput")
    pxb_d = nc.dram_tensor("pxb", [128, NPX], dt.float32, kind="ExternalInput")
    pyb_d = nc.dram_tensor("pyb", [128, NPX], dt.float32, kind="ExternalInput")
    pyexp_d = nc.dram_tensor("pyexp", [128, ROWS * 9], dt.float32, kind="ExternalInput")
    pxcol_d = nc.dram_tensor("pxcol", [128, 1], dt.float32, kind="ExternalInput")
    imr_d = nc.dram_tensor("imr", [ROWS, W, 3], dt.float32, kind="ExternalOutput")
    imp_d = nc.dram_tensor("imp", [ROWS, W], dt.float32, kind="ExternalOutput")

    with TileContext(nc) as tc:
        with tc.tile_pool(name="cb", bufs=1) as cb, \
             tc.tile_pool(name="zb", bufs=1) as zbp, \
             tc.tile_pool(name="wk", bufs=2) as wk, \
             tc.tile_pool(name="ep", bufs=1) as ep, \
             tc.tile_pool(name="ps1", bufs=1, space="PSUM") as ps1, \
             tc.tile_pool(name="ps2", bufs=3, space="PSUM") as ps2, \
             tc.tile_pool(name="dr", bufs=1, space="DRAM") as dr:

            # ---- constants into SBUF ----
            coefs_sb = cb.tile([4, NT * 6 * 128], dt.float32, tag="coefs")
            nc.sync.dma_start(out=coefs_sb[:].rearrange("k (t m j) -> k t m j", t=NT, m=6),
                              in_=coefs_d[:].rearrange("t m k j -> k t m j"))
            wcols = cb.tile([128, NT * 12], dt.float32, tag="wcols")
            nc.sync.dma_start(out=wcols[:], in_=wcols_d[:])
            fcols = cb.tile([128, NT * 8], dt.float32, tag="fcols")
            nc.sync.dma_start(out=fcols[:], in_=fcols_d[:])
            gtab = cb.tile([128, NT * 36], dt.float32, tag="gtab")
            nc.sync.dma_start(out=gtab[:], in_=gtab_d[:])
            misc = cb.tile([128, 16], dt.float32, tag="misc")
            nc.sync.dma_start(out=misc[:], in_=misc_d[:])

            pyexp = cb.tile([128, ROWS * 9], dt.float32, tag="pyexp")
            nc.sync.dma_start(out=pyexp[:], in_=pyexp_d[:])
            pxcol = cb.tile([128, 1], dt.float32, tag="pxcol")
            nc.sync.dma_start(out=pxcol[:], in_=pxcol_d[:])
            ones = cb.tile([128, 1], dt.float32, tag="ones")
            nc.vector.memset(ones[:], 1.0)

            imfeat = ep.tile([128, ROWS * 36], dt.float32, tag="imfeat")
            covT = ep.tile([128, ROWS], dt.float32, tag="covT")
            covd = dr.tile([ROWS, W], dt.float32, tag="covd")
            gathA = ep.tile([128, ROWS * 2 * 6], dt.float32, tag="gathA")
            uA = ep.tile([128, ROWS], dt.float32, tag="uA")
            vA = ep.tile([128, ROWS], dt.float32, tag="vA")

            def fcol(t, j):
                return fcols[:, t * 8 + j:t * 8 + j + 1]

            def wcol(t, j):
                return wcols[:, t * 12 + j:t * 12 + j + 1]

            # ---------------- main per-chunk loop ----------------
            for ch in range(NCHUNK):
                pxs = slice(ch * CH, (ch + 1) * CH)
                bas = wk.tile([4, CH], dt.float32, tag="bas")
                nc.sync.dma_start(out=bas[:], in_=basis_d[:, pxs])
                pxbc = wk.tile([128, CH], dt.float32, tag="pxbc")
                nc.sync.dma_start(out=pxbc[:], in_=pxb_d[:, pxs])
                pybc = wk.tile([128, CH], dt.float32, tag="pybc")
                nc.sync.dma_start(out=pybc[:], in_=pyb_d[:, pxs])

                zts = []
                sg = ps1.tile([1, CH], dt.float32, tag="sg", space="PSUM")
                in_tiles = []
                pc_tiles = []
                for t in range(NT):
                    # ---- exact w/zi maps: s_m = PX*c0 + PY*c1 (ACT fma + STT) ----
                    ss = []
                    for m in range(4):
                        u = wk.tile([128, CH], dt.float32, tag=f"u{m}", name=f"u{m}")
                        nc.scalar.activation(u[:], pxbc[:], Act.Copy,
                                             scale=wcol(t, m * 3 + 0))
                        nc.vector.scalar_tensor_tensor(
                            out=u[:], in0=pybc[:], scalar=wcol(t, m * 3 + 1),
                            in1=u[:], op0=Alu.mult, op1=Alu.add)
                        ss.append(u)
                    m1 = wk.tile([128, CH], dt.float32, tag="m1")
                    nc.vector._custom_dve(mina2_op, out=m1[:], in0=ss[0][:], in1=ss[1][:],
                                          s0=wcol(t, 0 * 3 + 2), s1=wcol(t, 1 * 3 + 2),
                                          imm2=0.0)
                    m3 = wk.tile([128, CH], dt.float32, tag="m3")
                    nc.vector.scalar_tensor_tensor(
                        out=m3[:], in0=ss[2][:], scalar=wcol(t, 2 * 3 + 2),
                        in1=m1[:], op0=Alu.add, op1=Alu.min)

                    zb = zbp.tile([128, CH], dt.float32, tag=f"z{t}")
                    nc.vector._custom_dve(zsel2_op, out=zb[:], in0=m3[:], in1=ss[3][:],
                                          s0=fcol(t, 7), s1=wcol(t, 3 * 3 + 2),
                                          imm2=float(ZFILL))
                    zts.append(zb)

                    # ---- soft-prob path (PE maps) ----
                    a_list = []
                    for e in range(3):
                        Sp = ps2.tile([128, CH], dt.float32, tag="qS", space="PSUM")
                        nc.tensor.matmul(
                            Sp[:], coefs_sb[:, (t * 6 + e) * 128:(t * 6 + e + 1) * 128],
                            bas[:], start=True, stop=True)
                        Qp = ps2.tile([128, CH], dt.float32, tag="qQ", space="PSUM")
                        nc.tensor.matmul(
                            Qp[:], coefs_sb[:, (t * 6 + e + 3) * 128:(t * 6 + e + 4) * 128],
                            bas[:], start=True, stop=True)
                        Qs = wk.tile([128, CH], dt.float32, tag="Qs")
                        nc.scalar.activation(Qs[:], Qp[:], Act.Copy)
                        a = wk.tile([128, CH], dt.float32, tag=f"a{e}")
                        nc.vector._custom_dve(seg_a_op, out=a[:], in0=Sp[:], in1=Qs[:],
                                              s0=fcol(t, e), s1=fcol(t, 3 + e), imm2=1.0)
                        a_list.append(a)
                    am = wk.tile([128, CH], dt.float32, tag="am")
                    nc.vector.tensor_tensor(out=am[:], in0=a_list[0][:], in1=a_list[1][:], op=Alu.min)
                    am2 = wk.tile([128, CH], dt.float32, tag="am2")
                    nc.vector.tensor_tensor(out=am2[:], in0=am[:], in1=a_list[2][:], op=Alu.min)
                    pp = wk.tile([128, CH], dt.float32, tag="pp")
                    nc.scalar.activation(pp[:], am2[:], Act.Exp, scale=-1.0)
                    pct = zbp.tile([128, CH], dt.float32, tag=f"pc{t}", name=f"pc{t}")
                    nc.vector._custom_dve(psel2_op, out=pct[:], in0=m3[:], in1=pp[:],
                                          s0=fcol(t, 6), s1=0.0, imm2=PMAX)
                    pc_tiles.append(pct)

                # ---- batched Ln pass (one ACT table switch per chunk) ----
                for t in range(NT):
                    lg = wk.tile([128, CH], dt.float32, tag="lg")
                    nc.scalar.activation(lg[:], pc_tiles[t][:], Act.Ln, scale=-1.0, bias=1.0)
                    nc.tensor.matmul(sg[:], ones[:], lg[:], start=(t == 0),
                                     stop=(t == NT - 1), skip_group_check=True)

                # ---- zmax + sum(lg) partition reductions ----
                zmt = wk.tile([128, CH], dt.float32, tag="zmt")
                nc.vector.tensor_tensor(out=zmt[:], in0=zts[0][:], in1=zts[1][:], op=Alu.max)
                for t in range(2, NT):
                    nc.vector.tensor_tensor(out=zmt[:], in0=zmt[:], in1=zts[t][:], op=Alu.max)
                zmaxb = wk.tile([128, CH], dt.float32, tag="zmaxb")
                nc.gpsimd.partition_all_reduce(zmaxb[:], zmt[:], 128, bass_isa.ReduceOp.max)

                # ---- one-hot winner + feature matmul (accumulate over tiles) ----
                gall = ps1.tile([128, RCH * 36], dt.float32, tag="gall", space="PSUM")
                ohs = []
                for t in range(NT):
                    oh = zbp.tile([128, CH], dt.float32, tag=f"oh{t}", name=f"oh{t}")
                    nc.vector.tensor_tensor(out=oh[:], in0=zts[t][:], in1=zmaxb[:], op=Alu.is_equal)
                    ohs.append(oh)
                for blk in range(RCH):
                    for t in range(NT):
                        nc.tensor.matmul(gall[:, blk * 36:(blk + 1) * 36],
                                         ohs[t][:, blk * 128:(blk + 1) * 128],
                                         gtab[:, t * 36:(t + 1) * 36],
                                         start=(t == 0), stop=(t == NT - 1),
                                         skip_group_check=True)
                nc.scalar.activation(
                    imfeat[:, ch * RCH * 36:(ch + 1) * RCH * 36], gall[:], Act.Copy)

                # --- texcoords + texture gathers for this chunk's 4 rows ---
                ut = wk.tile([128, RCH], dt.float32, tag="ut")
                vt = wk.tile([128, RCH], dt.float32, tag="vt")
                tt1 = wk.tile([128, RCH], dt.float32, tag="tt1")
                idf = wk.tile([128, RCH * 2], dt.float32, tag="idf")
                idi = wk.tile([128, RCH * 2], dt.int32, tag="idi")
                imfc = imfeat[:, ch * RCH * 36:(ch + 1) * RCH * 36].rearrange(
                    "p (r j d) -> p r j d", j=4, d=9)
                for j, dst in ((6, ut), (7, vt)):
                    nc.vector.tensor_scalar(out=tt1[:], in0=imfc[:, :, 0, j],
                                            scalar1=pxcol[:, 0:1], scalar2=None, op0=Alu.mult)
                    nc.vector.tensor_tensor(out=dst[:], in0=imfc[:, :, 1, j],
                                            in1=pyexp[:].rearrange("p (r d) -> p r d", d=9)[
                                                :, ch * RCH:(ch + 1) * RCH, 0],
                                            op=Alu.mult)
                    nc.vector.tensor_tensor(out=dst[:], in0=dst[:], in1=tt1[:], op=Alu.add)
                    nc.vector.tensor_tensor(out=dst[:], in0=dst[:], in1=imfc[:, :, 2, j], op=Alu.add)
                # u = clip(u,0,1)*511 ; v = (1-clip(v,0,1))*511
                nc.vector.tensor_scalar(out=ut[:], in0=ut[:], scalar1=0.0, scalar2=1.0,
                                        op0=Alu.max, op1=Alu.min)
                nc.vector.tensor_scalar(out=ut[:], in0=ut[:], scalar1=float(TEX - 1),
                                        scalar2=None, op0=Alu.mult)
                nc.vector.tensor_scalar(out=vt[:], in0=vt[:], scalar1=0.0, scalar2=1.0,
                                        op0=Alu.max, op1=Alu.min)
                nc.vector.tensor_scalar(out=vt[:], in0=vt[:], scalar1=-1.0, scalar2=1.0,
                                        op0=Alu.mult, op1=Alu.add)
                nc.vector.tensor_scalar(out=vt[:], in0=vt[:], scalar1=float(TEX - 1),
                                        scalar2=None, op0=Alu.mult)
                nc.vector.tensor_copy(uA[:, ch * RCH:(ch + 1) * RCH], ut[:])
                nc.vector.tensor_copy(vA[:, ch * RCH:(ch + 1) * RCH], vt[:])
                # floor (exact, rounding-agnostic)
                ci = wk.tile([128, RCH], dt.int32, tag="ci")
                fx = wk.tile([128, RCH], dt.float32, tag="fx")
                x0c = wk.tile([128, RCH], dt.float32, tag="x0c")
                y0c = wk.tile([128, RCH], dt.float32, tag="y0c")
                for srcv, dstv in ((ut, x0c), (vt, y0c)):
                    nc.vector.tensor_copy(ci[:], srcv[:])
                    nc.vector.tensor_copy(dstv[:], ci[:])
                    nc.vector.tensor_tensor(out=fx[:], in0=dstv[:], in1=srcv[:], op=Alu.is_gt)
                    nc.vector.tensor_tensor(out=dstv[:], in0=dstv[:], in1=fx[:], op=Alu.subtract)
                y1c = wk.tile([128, RCH], dt.float32, tag="y1c")
                nc.vector.tensor_scalar(out=y1c[:], in0=y0c[:], scalar1=1.0,
                                        scalar2=float(TEX - 1), op0=Alu.add, op1=Alu.min)
                idv = idf[:].rearrange("p (r d) -> p r d", d=2)
                nc.vector.scalar_tensor_tensor(out=idv[:, :, 0], in0=y0c[:], scalar=float(TEX),
                                               in1=x0c[:], op0=Alu.mult, op1=Alu.add)
                nc.vector.scalar_tensor_tensor(out=idv[:, :, 1], in0=y1c[:], scalar=float(TEX),
                                               in1=x0c[:], op0=Alu.mult, op1=Alu.add)
                nc.vector.tensor_scalar(out=idf[:], in0=idf[:], scalar1=3.0, scalar2=None,
                                        op0=Alu.mult)
                nc.vector.tensor_copy(idi[:], idf[:])
                for r in range(RCH):
                    for wch in range(2):
                        gi = (ch * RCH + r) * 2 + wch
                        nc.gpsimd.indirect_dma_start(
                            out=gathA[:, gi * 6:(gi + 1) * 6], out_offset=None,
                            in_=texi_d[:],
                            in_offset=bass.IndirectOffsetOnAxis(
                                ap=idi[:, r * 2 + wch:r * 2 + wch + 1], axis=0))

                # ---- coverage + improb for this chunk ----
                cr = ep.tile([1, CH], dt.float32, tag="cr")
                nc.vector.tensor_scalar(out=cr[:], in0=zmaxb[0:1, :], scalar1=float(ZFILL),
                                        scalar2=None, op0=Alu.is_gt)
                nc.gpsimd.dma_start(
                    out=covd[ch * RCH:(ch + 1) * RCH, :], in_=cr[:])
                exr = ep.tile([1, CH], dt.float32, tag="exr")
                nc.scalar.activation(exr[:], sg[:], Act.Exp)
                impc = ep.tile([1, CH], dt.float32, tag="impc")
                nc.vector.tensor_scalar(out=impc[:], in0=exr[:], scalar1=-1.0,
                                        scalar2=1.0, op0=Alu.mult, op1=Alu.add)
                nc.sync.dma_start(
                    out=imp_d[ch * RCH:(ch + 1) * RCH, :].rearrange("r c -> r c")[None, :, :],
                    in_=impc[:].rearrange("a (r c) -> a r c", c=W))

            # ---------------- core epilogue ([col, row] layout) ----------------
            NR = ROWS
            nc.gpsimd.dma_start(out=covT[:], in_=covd[:].rearrange("r c -> c r"))
            i3 = imfeat[:].rearrange("p (r j d) -> p r j d", j=4, d=9)
            imf9 = ep.tile([128, NR * 9], dt.float32, tag="imf9")
            v9 = imf9[:].rearrange("p (r d) -> p r d", d=9)
            tmp9 = ep.tile([128, NR * 9], dt.float32, tag="tmp9")
            t9 = tmp9[:].rearrange("p (r d) -> p r d", d=9)
            nc.vector.tensor_scalar(out=t9[:, :, :], in0=i3[:, :, 0, :],
                                    scalar1=pxcol[:, 0:1], scalar2=None, op0=Alu.mult)
            nc.vector.tensor_tensor(out=v9[:, :, :], in0=i3[:, :, 1, :],
                                    in1=pyexp[:].rearrange("p (r d) -> p r d", d=9),
                                    op=Alu.mult)
            nc.vector.tensor_tensor(out=v9[:, :, :], in0=v9[:, :, :], in1=t9[:, :, :], op=Alu.add)
            nc.vector.tensor_tensor(out=v9[:, :, :], in0=v9[:, :, :], in1=i3[:, :, 2, :], op=Alu.add)

            def vsl(j, k=1):
                return imf9[:].rearrange("p (r d) -> p r d", d=9)[:, :, j:j + k]

            nc.vector.tensor_tensor(out=vsl(8)[:, :, 0], in0=vsl(8)[:, :, 0],
                                    in1=covT[:], op=Alu.mult)

            def normalize3(dst_tag, j0):
                sq = ep.tile([128, NR * 3], dt.float32, tag=dst_tag + "sq")
                s3 = sq[:].rearrange("p (r d) -> p r d", d=3)
                nc.vector.tensor_tensor(out=s3[:], in0=vsl(j0, 3)[:, :, :],
                                        in1=vsl(j0, 3)[:, :, :], op=Alu.mult)
                n2 = ep.tile([128, NR], dt.float32, tag=dst_tag + "n2")
                nc.vector.tensor_reduce(out=n2[:], in_=s3[:], axis=mybir.AxisListType.X,
                                        op=Alu.add)
                sn = ep.tile([128, NR], dt.float32, tag=dst_tag + "sn")
                nc.scalar.activation(sn[:], n2[:], Act.Sqrt)
                nc.vector.tensor_scalar(out=sn[:], in0=sn[:], scalar1=float(EPS),
                                        scalar2=None, op0=Alu.add)
                rc = ep.tile([128, NR], dt.float32, tag=dst_tag + "rc")
                nc.vector.reciprocal(rc[:], sn[:])
                out = ep.tile([128, NR * 3], dt.float32, tag=dst_tag)
                o3 = out[:].rearrange("p (r d) -> p r d", d=3)
                for k in range(3):
                    nc.vector.tensor_tensor(out=o3[:, :, k], in0=vsl(j0 + k)[:, :, 0],
                                            in1=rc[:], op=Alu.mult)
                return out, o3

            nrm, nrm3 = normalize3("nrm", 0)
            eye, eye3 = normalize3("eye", 3)

            def mcol(j):
                return misc[:, j:j + 1]

            cosT = ep.tile([128, NR], dt.float32, tag="cosT")
            nc.vector.tensor_scalar(out=cosT[:], in0=nrm3[:, :, 0], scalar1=mcol(9),
                                    scalar2=None, op0=Alu.mult)
            nc.vector.scalar_tensor_tensor(out=cosT[:], in0=nrm3[:, :, 1], scalar=mcol(10),
                                           in1=cosT[:], op0=Alu.mult, op1=Alu.add)
            nc.vector.scalar_tensor_tensor(out=cosT[:], in0=nrm3[:, :, 2], scalar=mcol(11),
                                           in1=cosT[:], op0=Alu.mult, op1=Alu.add)
            nc.vector.tensor_scalar(out=cosT[:], in0=cosT[:], scalar1=0.0, scalar2=1.0,
                                    op0=Alu.max, op1=Alu.min)
            cosA = ep.tile([128, NR], dt.float32, tag="cosA")
            rk = ep.tile([128, NR], dt.float32, tag="rk")
            for k in range(3):
                nc.vector.tensor_tensor(out=rk[:], in0=cosT[:], in1=nrm3[:, :, k], op=Alu.mult)
                nc.vector.tensor_scalar(out=rk[:], in0=rk[:], scalar1=2.0, scalar2=None,
                                        op0=Alu.mult)
                nc.vector.tensor_scalar(out=rk[:], in0=rk[:], scalar1=mcol(9 + k),
                                        scalar2=None, op0=Alu.subtract)
                nc.vector.tensor_tensor(out=rk[:], in0=rk[:], in1=eye3[:, :, k], op=Alu.mult)
                if k == 0:
                    nc.vector.tensor_copy(cosA[:], rk[:])
                else:
                    nc.vector.tensor_tensor(out=cosA[:], in0=cosA[:], in1=rk[:], op=Alu.add)
            nc.vector.tensor_scalar(out=cosA[:], in0=cosA[:], scalar1=1e-5, scalar2=1.0,
                                    op0=Alu.max, op1=Alu.min)
            nc.scalar.activation(cosA[:], cosA[:], Act.Ln)
            nc.scalar.activation(cosA[:], cosA[:], Act.Exp, scale=mcol(12))

            # texture lookup (u/v + gathers were produced per-chunk)
            uu = uA
            vv = vA
            # exact floor: cast to int and back, then subtract 1 where it rounded up
            cint = ep.tile([128, NR], dt.int32, tag="cint")
            fixt = ep.tile([128, NR], dt.float32, tag="fixt")

            def floorf(dst_tag, src):
                nc.vector.tensor_copy(cint[:], src[:])
                dst = ep.tile([128, NR], dt.float32, tag=dst_tag, name=dst_tag)
                nc.vector.tensor_copy(dst[:], cint[:])
                nc.vector.tensor_tensor(out=fixt[:], in0=dst[:], in1=src[:], op=Alu.is_gt)
                nc.vector.tensor_tensor(out=dst[:], in0=dst[:], in1=fixt[:], op=Alu.subtract)
                return dst

            x0f = floorf("x0f", uu)
            y0f = floorf("y0f", vv)
            wx = ep.tile([128, NR], dt.float32, tag="wx")
            nc.vector.tensor_tensor(out=wx[:], in0=uu[:], in1=x0f[:], op=Alu.subtract)
            wy = ep.tile([128, NR], dt.float32, tag="wy")
            nc.vector.tensor_tensor(out=wy[:], in0=vv[:], in1=y0f[:], op=Alu.subtract)
            gv = gathA[:].rearrange("p (r w d) -> p r w d", w=2, d=6)

            wxc = ep.tile([128, NR], dt.float32, tag="wxc")
            nc.vector.tensor_scalar(out=wxc[:], in0=wx[:], scalar1=-1.0, scalar2=1.0,
                                    op0=Alu.mult, op1=Alu.add)
            wyc = ep.tile([128, NR], dt.float32, tag="wyc")
            nc.vector.tensor_scalar(out=wyc[:], in0=wy[:], scalar1=-1.0, scalar2=1.0,
                                    op0=Alu.mult, op1=Alu.add)
            w00 = ep.tile([128, NR], dt.float32, tag="w00")
            nc.vector.tensor_tensor(out=w00[:], in0=wxc[:], in1=wyc[:], op=Alu.mult)
            w01 = ep.tile([128, NR], dt.float32, tag="w01")
            nc.vector.tensor_tensor(out=w01[:], in0=wx[:], in1=wyc[:], op=Alu.mult)
            w10 = ep.tile([128, NR], dt.float32, tag="w10")
            nc.vector.tensor_tensor(out=w10[:], in0=wxc[:], in1=wy[:], op=Alu.mult)
            w11 = ep.tile([128, NR], dt.float32, tag="w11")
            nc.vector.tensor_tensor(out=w11[:], in0=wx[:], in1=wy[:], op=Alu.mult)

            colorT = ep.tile([128, NR * 3], dt.float32, tag="colorT")
            c3v = colorT[:].rearrange("p (r d) -> p r d", d=3)
            tcc = ep.tile([128, NR], dt.float32, tag="tcc")
            acc = ep.tile([128, NR], dt.float32, tag="acc")
            mm = ep.tile([128, NR], dt.float32, tag="mm")
            for c in range(3):
                nc.vector.tensor_tensor(out=tcc[:], in0=gv[:, :, 0, c], in1=w00[:], op=Alu.mult)
                nc.vector.tensor_tensor(out=mm[:], in0=gv[:, :, 0, 3 + c], in1=w01[:], op=Alu.mult)
                nc.vector.tensor_tensor(out=tcc[:], in0=tcc[:], in1=mm[:], op=Alu.add)
                nc.vector.tensor_tensor(out=mm[:], in0=gv[:, :, 1, c], in1=w10[:], op=Alu.mult)
                nc.vector.tensor_tensor(out=tcc[:], in0=tcc[:], in1=mm[:], op=Alu.add)
                nc.vector.tensor_tensor(out=mm[:], in0=gv[:, :, 1, 3 + c], in1=w11[:], op=Alu.mult)
                nc.vector.tensor_tensor(out=tcc[:], in0=tcc[:], in1=mm[:], op=Alu.add)
                nc.vector.tensor_tensor(out=mm[:], in0=cosT[:], in1=tcc[:], op=Alu.mult)
                nc.vector.tensor_scalar(out=acc[:], in0=tcc[:], scalar1=mcol(c),
                                        scalar2=None, op0=Alu.mult)
                nc.vector.scalar_tensor_tensor(out=acc[:], in0=mm[:], scalar=mcol(3 + c),
                                               in1=acc[:], op0=Alu.mult, op1=Alu.add)
                nc.vector.scalar_tensor_tensor(out=acc[:], in0=cosA[:], scalar=mcol(6 + c),
                                               in1=acc[:], op0=Alu.mult, op1=Alu.add)
                nc.vector.tensor_tensor(out=acc[:], in0=acc[:], in1=vsl(8)[:, :, 0], op=Alu.mult)
                nc.vector.tensor_scalar(out=c3v[:, :, c], in0=acc[:], scalar1=0.0, scalar2=1.0,
                                        op0=Alu.max, op1=Alu.min)

            nc.sync.dma_start(out=imr_d[:].rearrange("r c k -> c r k"),
                              in_=colorT[:].rearrange("p (r k) -> p r k", k=3))

    nc.finalize()
    return nc


def kernel(points, faces, camera_rot, camera_pos, camera_proj, uv, ft, texture,
           lightdirect, material, shininess, height, width):
    from concourse.bass_utils import run_bass_kernel_spmd

    inputs = dict(points=points, faces=faces, camera_rot=camera_rot,
                  camera_pos=camera_pos, camera_proj=camera_proj, uv=uv, ft=ft,
                  texture=texture, lightdirect=lightdirect, material=material,
                  shininess=shininess)
    in_maps, normal1 = host_prep(inputs)

    if 'nc' not in _NC_CACHE:
        _NC_CACHE['nc'] = build_nc()
    nc = _NC_CACHE['nc']

    res = run_bass_kernel_spmd(nc, in_maps, core_ids=list(range(NCORES)))

    imrender = np.zeros((B, H, W, 3), dtype=f32)
    improb = np.zeros((B, H, W, 1), dtype=f32)
    for core in range(NCORES):
        b = core // BANDS
        r0 = (core % BANDS) * ROWS
        imrender[b, r0:r0 + ROWS] = res.results[core]['imr']
        improb[b, r0:r0 + ROWS, :, 0] = res.results[core]['imp']
    return imrender, improb, normal1
